# revision 10
# baseline (speedup 1.0000x reference)
"""Trainium2 Bass kernel for nn_EncoderWithClassifier (4-layer encoder + classifier).

Sharding: 8 cores, core c handles (batch b=c//2, sequence half th=c%2, 1024 tokens).
Canonical activation layout: x^T [C=256 (2 chunks of 128 partitions), T_local=1024].

Attention: scores transposed ([s_tile, t]) via row-packed K=32 matmuls (fp32r
single-pass PE), one exp per (st,g) over 2 heads. o^T accumulation uses a
33-column lhsT (v | ones) so each o matmul also emits the softmax row-sum in
PSUM partition rows 32/96 -- no separate row-sum matmuls; two heads pack per
accumulator via tile_position (0,0)/(0,64). The S->exp->o chain is
software-pipelined one iteration ahead so the in-order PE queue never blocks
on the Act engine's exp stream.

Layer pipeline: everything outside attention (proj, LN2, FFN, next-layer LN1 +
QKV) is per-token work, split into two 512-token half-chains and emitted as
"filler" steps interleaved into the next attention segment's instruction
stream -- PE/DVE chain work executes in the shadow of the Act-bound exp
stream. The 2-rank AllGather for the next layer's remote K/V is likewise
issued from filler, and remote-s score tiles are ordered last so the
collective latency hides under local-s compute.

LayerNorm: stats via packed matmuls; per-token affine folded into the
replication matmuls (lhsT = [g] and [g; b] rows) so the apply is 2 DVE ops per
chunk; x^2 runs on the idle GpSimd engine. A manual act-table load of the
ln+exp set at program start pins one table for the whole program (the
auto-inserted per-switch loads would otherwise cost ~23us).

PSUM budget (8 banks): S [128,1024] x2 bufs = 4, "acc" 2x[128,512] = 2,
"mm" 2x[128,512] = 2.
"""
import numpy as np

import concourse.bacc as bacc
import concourse.mybir as mybir
import concourse.tile as tile
from concourse import bass_utils, library_config
from concourse.masks import make_identity

V, C, TMAX, H, L = 32000, 256, 2048, 8, 4
HS, FFN = 32, 256
CLS_H, NOUT = 512, 10
B, T = 4, 2048
TL = 1024          # tokens per core
P = 128
EPS = 1e-5
SCALE = C ** (-0.5)
N_CORES = 8
dt = mybir.dt
F32 = dt.float32
Alu = mybir.AluOpType
Act = mybir.ActivationFunctionType
X_AXIS = mybir.AxisListType.X

LN_EXP_SET = 6     # act_info.json index of natural_log_exp_and_others

_CACHE = {}


def _build_program(sim=False):
    nc = bacc.Bacc("TRN2", target_bir_lowering=False, debug=False,
                   num_devices=1 if sim else N_CORES)

    # ---------------- dram I/O ----------------
    tok = nc.dram_tensor("tok", [V, C], F32, kind="ExternalInput")
    idxw = nc.dram_tensor("idxw", [P, TL // 16], dt.int16, kind="ExternalInput")
    posr = nc.dram_tensor("posr", [P, TL // P, C], F32, kind="ExternalInput")
    remidx = nc.dram_tensor("remidx", [P, (2 * P) // 16], dt.int16,
                            kind="ExternalInput")
    wq_d = nc.dram_tensor("wq", [L, P, 2, C], F32, kind="ExternalInput")
    wk_d = nc.dram_tensor("wk", [L, P, 2, C], F32, kind="ExternalInput")
    wv_d = nc.dram_tensor("wv", [L, P, 2, C], F32, kind="ExternalInput")
    wp_d = nc.dram_tensor("wp", [L, P, 2, C], F32, kind="ExternalInput")
    w1_d = nc.dram_tensor("w1", [L, P, 2, FFN], F32, kind="ExternalInput")
    w2_d = nc.dram_tensor("w2", [L, P, 2, C], F32, kind="ExternalInput")
    vecs_d = nc.dram_tensor("vecs", [L, P, 3, 2], F32, kind="ExternalInput")
    # vecs rows: 0 bproj, 1 b1, 2 b2
    vrow_d = nc.dram_tensor("vrow", [L, 2, 2, 2, P], F32, kind="ExternalInput")
    # vrow dims: [l, row(g=0,b=1), ln_i, cc, P]  (g/b transposed to rows)
    lnfrow_d = nc.dram_tensor("lnfrow", [2, 2, P], F32, kind="ExternalInput")
    wc1_d = nc.dram_tensor("wc1", [P, 2, CLS_H], F32, kind="ExternalInput")
    bc1_d = nc.dram_tensor("bc1", [P, CLS_H // P], F32, kind="ExternalInput")
    wc2_d = nc.dram_tensor("wc2", [P, CLS_H // P, NOUT], F32, kind="ExternalInput")
    bc2_d = nc.dram_tensor("bc2", [1, NOUT], F32, kind="ExternalInput")
    out_d = nc.dram_tensor("probs", [1, NOUT], F32, kind="ExternalOutput")

    REPL = [[0, 1], [2, 3], [4, 5], [6, 7]]

    with tile.TileContext(nc) as tc:
        with (
            tc.tile_pool(name="const", bufs=1) as cp,
            tc.tile_pool(name="work", bufs=1) as wk,
            tc.tile_pool(name="exp", bufs=3) as ep,
            tc.tile_pool(name="small", bufs=1) as sp,
            tc.tile_pool(name="psS", bufs=2, space="PSUM") as psS,
            tc.tile_pool(name="psA", bufs=2, space="PSUM") as psA,
            tc.tile_pool(name="psM", bufs=2, space="PSUM") as psM,
            tc.tile_pool(name="dram", bufs=2, space="DRAM") as dp,
        ):
            nc.gpsimd.load_library(library_config.mlp)

            R32 = dt.float32r

            def mm(out, lhsT, rhs, **kw):
                """matmul with fp32r operand views: single-pass PE (4x fp32)."""
                nc.tensor.matmul(out, lhsT=lhsT.bitcast(R32),
                                 rhs=rhs.bitcast(R32), **kw)

            # pin the ln+exp act table once; Ln and Exp then never reload
            nc.scalar.add_instruction(mybir.InstLoadActFuncSet(
                name=nc.get_next_instruction_name(),
                ins=[], outs=[], act_func_set_id=LN_EXP_SET))

            # ---------------- constants / weights to SBUF ----------------
            ident = cp.tile([P, P], F32, tag="ident")
            make_identity(nc, ident[:])
            inv256 = cp.tile([P, 1], F32, tag="inv256")
            nc.vector.memset(inv256[:], 1.0 / C)
            sel4 = cp.tile([4, P], F32, tag="sel4")
            nc.gpsimd.memset(sel4[:], 0.0)
            for j in range(4):
                nc.gpsimd.memset(sel4[j:j + 1, 32 * j:32 * (j + 1)], 1.0)

            def load_const(name, dram_ap, shape, dtype=F32):
                t = cp.tile(shape, dtype, tag=name, name=name)
                nc.sync.dma_start(t[:], dram_ap)
                return t

            wq = [load_const(f"wq{l}", wq_d[l], [P, 2, C]) for l in range(L)]
            wkt = [load_const(f"wk{l}", wk_d[l], [P, 2, C]) for l in range(L)]
            wv = [load_const(f"wv{l}", wv_d[l], [P, 2, C]) for l in range(L)]
            wp = [load_const(f"wp{l}", wp_d[l], [P, 2, C]) for l in range(L)]
            w1 = [load_const(f"w1{l}", w1_d[l], [P, 2, FFN]) for l in range(L)]
            w2 = [load_const(f"w2{l}", w2_d[l], [P, 2, C]) for l in range(L)]
            vecs = [load_const(f"vec{l}", vecs_d[l], [P, 3, 2]) for l in range(L)]
            vrow = [load_const(f"vrow{l}", vrow_d[l], [2, 2, 2, P])
                    for l in range(L)]
            lnfrow = load_const("lnfrow", lnfrow_d[:], [2, 2, P])
            wc1 = load_const("wc1", wc1_d[:], [P, 2, CLS_H])
            bc1 = load_const("bc1", bc1_d[:], [P, CLS_H // P])
            wc2 = load_const("wc2", wc2_d[:], [P, CLS_H // P, NOUT])
            bc2 = load_const("bc2", bc2_d[:], [1, NOUT])
            idx_sb = load_const("idx_sb", idxw[:], [P, TL // 16], dt.int16)
            remidx_sb = load_const("remidx_sb", remidx[:], [P, (2 * P) // 16],
                                   dt.int16)

            # vecs[l] rows: 0 bproj, 1 b1, 2 b2
            def vap(l, row, cc):
                return vecs[l][:, row, cc:cc + 1]

            # persistent activations
            xT = [wk.tile([P, TL], F32, tag=f"xT{cc}", name=f"xT{cc}")
                  for cc in range(2)]

            # ---------------- embedding ----------------
            with tc.tile_pool(name="embed", bufs=1) as ebp:
                xg = ebp.tile([P, TL // P, C], F32, tag="xg")
                nc.gpsimd.dma_gather(xg[:], tok[:], idx_sb[:], TL, TL, C)
                pos_sb = ebp.tile([P, TL // P, C], F32, tag="pos_sb")
                nc.sync.dma_start(pos_sb[:], posr[:])
                nc.vector.tensor_add(xg[:], xg[:], pos_sb[:])
                for tt in range(TL // P):
                    for cc in range(2):
                        tp = psM.tile([P, P], F32, tag="mm", name="tp")
                        nc.tensor.transpose(tp[:], xg[:, tt, cc * P:(cc + 1) * P],
                                            ident[:])
                        nc.vector.tensor_copy(xT[cc][:, tt * P:(tt + 1) * P], tp[:])

            # v with trailing ones column: o matmul emits row-sums for free
            v_sb = [wk.tile([P, H, HS + 1], F32, tag=f"v{st}", name=f"v{st}")
                    for st in range(16)]
            for st in range(16):
                nc.vector.memset(v_sb[st][:, :, HS:HS + 1], 1.0)
            # [mrs; -1] rows for the g*mrs - b replication matmul
            stM = sp.tile([2, TL], F32, tag="stM", name="stM")
            nc.vector.memset(stM[1:2, :], -1.0)

            stA = sp.tile([1, TL], F32, tag="stA")   # mu
            stB = sp.tile([1, TL], F32, tag="stB")   # msq -> var -> rstd
            stT = sp.tile([1, TL], F32, tag="stT")   # musq -> ln(var)
            xsq = [sp.tile([P, TL], F32, tag=f"lnsq{cc}", name=f"lnsq{cc}")
                   for cc in range(2)]

            hT = [wk.tile([P, TL], F32, tag=f"hT{cc}", name=f"hT{cc}")
                  for cc in range(2)]
            h2T = [wk.tile([P, TL], F32, tag=f"h2T{cc}", name=f"h2T{cc}")
                   for cc in range(2)]
            fT = [wk.tile([P, TL], F32, tag=f"fT{ff}", name=f"fT{ff}")
                  for ff in range(2)]
            oT = [wk.tile([P, TL], F32, tag=f"oT{cc}", name=f"oT{cc}")
                  for cc in range(2)]
            qT = [wk.tile([P, TL], F32, tag=f"qT{mt}", name=f"qT{mt}")
                  for mt in range(2)]
            kT = [wk.tile([P, T], F32, tag=f"kT{mt}", name=f"kT{mt}")
                  for mt in range(2)]
            hR = wk.tile([P, 2, TL], F32, tag="hR", name="hR")

            # -------- per-half op-list builders (each op = one emission) -----
            def ln_half_ops(src, grow, gbrow, out, half):
                """LayerNorm of 512-token half: list of emission closures."""
                sl = slice(half * 512, (half + 1) * 512)
                ops = []
                for cc in range(2):
                    ops.append(lambda cc=cc: nc.gpsimd.tensor_mul(
                        xsq[cc][:, sl], src[cc][:, sl], src[cc][:, sl]))

                def stats(dst, inp):
                    ps = psM.tile([1, 512], F32, tag="mm", name="stat")
                    for kc in range(2):
                        mm(ps[:], lhsT=inv256[:], rhs=inp[kc][:, sl],
                           start=(kc == 0), stop=(kc == 1))
                    nc.vector.tensor_copy(dst[:, sl], ps[:])
                ops.append(lambda: stats(stA, src))
                ops.append(lambda: stats(stB, xsq))

                def rows1():
                    nc.vector.tensor_mul(stT[:, sl], stA[:, sl], stA[:, sl])
                    nc.vector.scalar_tensor_tensor(stB[:, sl], stB[:, sl], EPS,
                                                   stT[:, sl], Alu.add,
                                                   Alu.subtract)
                ops.append(rows1)

                def rows2():
                    nc.scalar.activation(stT[:, sl], stB[:, sl], Act.Ln)
                    nc.scalar.activation(stB[:, sl], stT[:, sl], Act.Exp,
                                         scale=-0.5)
                    nc.vector.tensor_mul(stM[0:1, sl], stA[:, sl], stB[:, sl])
                ops.append(rows2)

                def apply(cc):
                    rep1 = psM.tile([P, 512], F32, tag="mm", name="rep1")
                    mm(rep1[:], lhsT=grow(cc), rhs=stB[:, sl],
                       start=True, stop=True)
                    rep2 = psM.tile([P, 512], F32, tag="mm", name="rep2")
                    mm(rep2[:], lhsT=gbrow(cc), rhs=stM[:, sl],
                       start=True, stop=True)
                    nc.vector.tensor_mul(out[cc][:, sl], src[cc][:, sl], rep1[:])
                    nc.vector.tensor_sub(out[cc][:, sl], out[cc][:, sl], rep2[:])
                for cc in range(2):
                    ops.append(lambda cc=cc: apply(cc))
                return ops

            def proj_half_ops(l, half):
                sl = slice(half * 512, (half + 1) * 512)
                ops = []

                def pj(cc):
                    dpj = psM.tile([P, 512], F32, tag="mm", name="dpj")
                    for kc in range(2):
                        mm(dpj[:], lhsT=wp[l][:, kc, cc * P:(cc + 1) * P],
                           rhs=oT[kc][:, sl], start=(kc == 0), stop=(kc == 1))
                    nc.vector.scalar_tensor_tensor(xT[cc][:, sl], dpj[:],
                                                   vap(l, 0, cc), xT[cc][:, sl],
                                                   Alu.add, Alu.add)
                for cc in range(2):
                    ops.append(lambda cc=cc: pj(cc))
                return ops

            def ffn_half_ops(l, half):
                sl = slice(half * 512, (half + 1) * 512)
                ops = []

                def f1(ff):
                    fps = psM.tile([P, 512], F32, tag="mm", name="fps")
                    for kc in range(2):
                        mm(fps[:], lhsT=w1[l][:, kc, ff * P:(ff + 1) * P],
                           rhs=h2T[kc][:, sl], start=(kc == 0), stop=(kc == 1))
                    nc.vector.tensor_scalar(fT[ff][:, sl], fps[:], vap(l, 1, ff),
                                            0.0, Alu.add, Alu.max)

                def f2(cc):
                    d2 = psM.tile([P, 512], F32, tag="mm", name="d2")
                    for kc in range(2):
                        mm(d2[:], lhsT=w2[l][:, kc, cc * P:(cc + 1) * P],
                           rhs=fT[kc][:, sl], start=(kc == 0), stop=(kc == 1))
                    nc.vector.scalar_tensor_tensor(xT[cc][:, sl], d2[:],
                                                   vap(l, 2, cc), xT[cc][:, sl],
                                                   Alu.add, Alu.add)
                for ff in range(2):
                    ops.append(lambda ff=ff: f1(ff))
                for cc in range(2):
                    ops.append(lambda cc=cc: f2(cc))
                return ops

            def q_half_ops(l, half):
                sl = slice(half * 512, (half + 1) * 512)
                ops = []

                def q(mt):
                    qps = psM.tile([P, 512], F32, tag="mm", name="qps")
                    for kc in range(2):
                        mm(qps[:], lhsT=wq[l][:, kc, mt * P:(mt + 1) * P],
                           rhs=hT[kc][:, sl], start=(kc == 0), stop=(kc == 1))
                    nc.vector.tensor_copy(qT[mt][:, sl], qps[:])
                for mt in range(2):
                    ops.append(lambda mt=mt: q(mt))
                return ops

            def k_ops(l, nch):
                """kT columns nch*512.. from local (nch<2) or remote half."""
                ops = []

                def k(mt):
                    kps = psM.tile([P, 512], F32, tag="mm", name="kps")
                    for kc in range(2):
                        if nch < 2:
                            rhs = hT[kc][:, nch * 512:(nch + 1) * 512]
                        else:
                            rhs = hR[:, kc, (nch - 2) * 512:(nch - 1) * 512]
                        mm(kps[:], lhsT=wkt[l][:, kc, mt * P:(mt + 1) * P],
                           rhs=rhs, start=(kc == 0), stop=(kc == 1))
                    nc.vector.tensor_copy(kT[mt][:, nch * 512:(nch + 1) * 512],
                                          kps[:])
                for mt in range(2):
                    ops.append(lambda mt=mt: k(mt))
                return ops

            def v_ops(l, sts):
                ops = []

                def v(st):
                    vps = psM.tile([P, H, HS], F32, tag="mm", name="vps")
                    for kc in range(2):
                        if st < 8:
                            lhsT = hT[kc][:, st * P:(st + 1) * P]
                        else:
                            lhsT = hR[:, kc, (st - 8) * P:(st - 7) * P]
                        mm(vps[:], lhsT=lhsT, rhs=wv[l][:, kc, :],
                           start=(kc == 0), stop=(kc == 1))
                    nc.vector.tensor_copy(v_sb[st][:, :, 0:HS], vps[:])
                for st in sts:
                    ops.append(lambda st=st: v(st))
                return ops

            def setup_ops(l):
                """AllGather hT -> hR, then remote-half K/V."""
                ops = []
                b_in = dp.tile([2 * P, TL], F32, tag="b_in", name="b_in")
                b_out = dp.tile([4 * P, TL], F32, tag="b_out", name="b_out")

                def send():
                    for cc in range(2):
                        nc.sync.dma_start(b_in[cc * P:(cc + 1) * P, :], hT[cc][:])

                def coll():
                    if sim:
                        nc.sync.dma_start(b_out[:2 * P, :], b_in[:])
                        nc.sync.dma_start(b_out[2 * P:, :], b_in[:])
                    else:
                        nc.gpsimd.collective_compute(
                            "AllGather", Alu.bypass, replica_groups=REPL,
                            ins=[b_in[:].opt()], outs=[b_out[:].opt()])

                def gath():
                    nc.gpsimd.dma_gather(hR[:], b_out[:], remidx_sb[:],
                                         2 * P, 2 * P, TL)
                ops.extend([send, coll, gath])
                ops.extend(k_ops(l, 2))
                ops.extend(k_ops(l, 3))
                ops.extend(v_ops(l, range(8, 12)))
                ops.extend(v_ops(l, range(12, 16)))
                return ops

            def ln1_rows(l):
                return (lambda cc: vrow[l][0:1, 0, cc, :],
                        lambda cc: vrow[l][:, 0, cc, :])

            def ln2_rows(l):
                return (lambda cc: vrow[l][0:1, 1, cc, :],
                        lambda cc: vrow[l][:, 1, cc, :])

            def lnf_rows():
                return (lambda cc: lnfrow[0:1, cc, :],
                        lambda cc: lnfrow[:, cc, :])

            def chain_ops(l, half):
                """proj -> LN2 -> FFN -> next LN1 (or final LN) for one half."""
                ops = []
                ops += proj_half_ops(l, half)
                g2, gb2 = ln2_rows(l)
                ops += ln_half_ops(xT, g2, gb2, h2T, half)
                ops += ffn_half_ops(l, half)
                if l + 1 < L:
                    g1, gb1 = ln1_rows(l + 1)
                    ops += ln_half_ops(xT, g1, gb1, hT, half)
                else:
                    gf, gbf = lnf_rows()
                    ops += ln_half_ops(xT, gf, gbf, h2T, half)
                return ops

            # -------------------- attention --------------------
            def run_attn(l, tcn, filler):
                tsl = slice(tcn * 512, (tcn + 1) * 512)
                fit = iter(filler) if filler is not None else None

                def step(n=1):
                    if fit is not None:
                        for _ in range(n):
                            op = next(fit, None)
                            if op is not None:
                                op()

                acc = {}

                def emit_S(hp, st, g):
                    S = psS.tile([P, 2 * 512], F32, tag="S", name="S")
                    for jj in range(2):
                        j = 2 * g + jj
                        mm(S[:, jj * 512:(jj + 1) * 512],
                           lhsT=kT[hp][32 * j:32 * (j + 1),
                                       st * P:(st + 1) * P],
                           rhs=qT[hp][32 * j:32 * (j + 1), tsl],
                           start=True, stop=True,
                           tile_position=(32 * j, 0))
                    expT = ep.tile([P, 2 * 512], F32, tag="expT", name="expT")
                    nc.scalar.activation(expT[:], S[:], Act.Exp, scale=SCALE)
                    return expT

                def emit_O(hp, st, g, expT):
                    for jj in range(2):
                        j = 2 * g + jj
                        mm(acc[hp][g][64 * jj:64 * jj + HS + 1, :],
                           lhsT=v_sb[st][:, hp * 4 + j, :],
                           rhs=expT[:, jj * 512:(jj + 1) * 512],
                           start=(st == 0), stop=(st == 15),
                           tile_position=(0, 64 * jj))

                def norm(hp):
                    rs4 = sp.tile([4, 512], F32, tag="rs4")
                    for g in range(2):
                        for jj in range(2):
                            nc.vector.tensor_copy(
                                rs4[2 * g + jj:2 * g + jj + 1, :],
                                acc[hp][g][64 * jj + HS:64 * jj + HS + 1, :])
                    rec4 = sp.tile([4, 512], F32, tag="rec4")
                    nc.vector.reciprocal(rec4[:], rs4[:])
                    rrep = psM.tile([P, 512], F32, tag="mm", name="rrep")
                    mm(rrep[:], lhsT=sel4[:], rhs=rec4[:], start=True, stop=True)
                    for j in range(4):
                        nc.vector.tensor_mul(
                            oT[hp][32 * j:32 * (j + 1), tsl],
                            acc[hp][j // 2][64 * (j % 2):64 * (j % 2) + HS, :],
                            rrep[32 * j:32 * (j + 1), :])

                pend = None
                for hp in range(2):
                    acc[hp] = [psA.tile([P, 512], F32, tag="acc",
                                        name=f"acc{g}") for g in range(2)]
                    for st in range(16):
                        for g in range(2):
                            cur = emit_S(hp, st, g)
                            step(3)
                            if pend is not None:
                                emit_O(*pend)
                                if pend[0] == 0 and pend[1] == 15 and pend[2] == 1:
                                    norm(0)
                            step(2)
                            pend = (hp, st, g, cur)
                emit_O(*pend)
                norm(1)
                # drain leftover filler
                step(1000)

            # -------------------- prologue: layer 0 setup --------------------
            g1, gb1 = ln1_rows(0)
            for half in range(2):
                for op in (ln_half_ops(xT, g1, gb1, hT, half)
                           + q_half_ops(0, half) + k_ops(0, half)
                           + v_ops(0, range(half * 4, half * 4 + 4))):
                    op()
            for op in setup_ops(0):
                op()

            # -------------------- layers --------------------
            pending = None
            for l in range(L):
                run_attn(l, 0, pending)
                fillerB = chain_ops(l, 0)
                if l + 1 < L:
                    fillerB += q_half_ops(l + 1, 0)
                    fillerB += k_ops(l + 1, 0)
                    fillerB += v_ops(l + 1, range(0, 4))
                run_attn(l, 1, fillerB)
                pending = chain_ops(l, 1)
                if l + 1 < L:
                    pending += q_half_ops(l + 1, 1)
                    pending += k_ops(l + 1, 1)
                    pending += v_ops(l + 1, range(4, 8))
                    pending += setup_ops(l + 1)
            # final-LN half 1 chain (wrote h2T) + half-0 result sits in h2T too
            for op in pending:
                op()

            # ---------------- pool + classifier ----------------
            # final LN output lives in h2T (both halves)
            emb = sp.tile([P, 2], F32, tag="emb")
            for cc in range(2):
                nc.vector.reduce_sum(emb[:, cc:cc + 1], h2T[cc][:], axis=X_AXIS)
            be_in = dp.tile([P, 2], F32, tag="be_in", name="be_in")
            be_out = dp.tile([P, 2], F32, tag="be_out", name="be_out")
            nc.sync.dma_start(be_in[:], emb[:])
            if sim:
                nc.sync.dma_start(be_out[:], be_in[:])
            else:
                nc.gpsimd.collective_compute(
                    "AllReduce", Alu.add, replica_groups=REPL,
                    ins=[be_in[:].opt()], outs=[be_out[:].opt()])
            embr = sp.tile([P, 2], F32, tag="embr")
            nc.sync.dma_start(embr[:], be_out[:])

            h1ps = psM.tile([P, CLS_H // P], F32, tag="mm", name="h1ps")
            for mt in range(CLS_H // P):
                for kc in range(2):
                    nc.tensor.matmul(h1ps[:, mt:mt + 1],
                                     lhsT=wc1[:, kc, mt * P:(mt + 1) * P],
                                     rhs=embr[:, kc:kc + 1],
                                     start=(kc == 0), stop=(kc == 1))
            h1 = sp.tile([P, CLS_H // P], F32, tag="h1")
            nc.vector.tensor_add(h1[:], h1ps[:], bc1[:])
            nc.vector.tensor_scalar_max(h1[:], h1[:], 0.0)
            lps = psM.tile([1, NOUT], F32, tag="mm", name="lps")
            for j in range(CLS_H // P):
                nc.tensor.matmul(lps[:], lhsT=h1[:, j:j + 1], rhs=wc2[:, j, :],
                                 start=(j == 0), stop=(j == CLS_H // P - 1))
            lsb = sp.tile([1, NOUT], F32, tag="lsb")
            nc.vector.tensor_add(lsb[:], lps[:], bc2[:])
            mx = sp.tile([1, 1], F32, tag="mx")
            nc.vector.tensor_reduce(mx[:], lsb[:], axis=X_AXIS, op=Alu.max)
            nmx = sp.tile([1, 1], F32, tag="nmx")
            nc.vector.tensor_scalar_mul(nmx[:], mx[:], -1.0)
            esb = sp.tile([1, NOUT], F32, tag="esb")
            nc.scalar.activation(esb[:], lsb[:], Act.Exp, bias=nmx[:])
            ssum = sp.tile([1, 1], F32, tag="ssum")
            nc.vector.reduce_sum(ssum[:], esb[:], axis=X_AXIS)
            rsum = sp.tile([1, 1], F32, tag="rsum")
            nc.vector.reciprocal(rsum[:], ssum[:])
            probs = sp.tile([1, NOUT], F32, tag="probs")
            nc.vector.tensor_single_scalar(probs[:], esb[:], rsum[:], Alu.mult)
            nc.sync.dma_start(out_d[:], probs[:])

    nc.compile()
    return nc


def _prep_shared(inputs):
    """Host-side weight prepack (identical for all cores)."""
    f = lambda a: np.ascontiguousarray(np.asarray(a, dtype=np.float32))

    def pack_mat(w):  # [C_in, M] -> [128, C_in//128, M]
        ci, m = w.shape
        return np.ascontiguousarray(w.reshape(ci // P, P, m).transpose(1, 0, 2))

    wq3 = np.stack([pack_mat(f(inputs["Wq"][l]).transpose(1, 0, 2).reshape(C, H * HS))
                    for l in range(L)])
    wk3 = np.stack([pack_mat(f(inputs["Wk"][l]).transpose(1, 0, 2).reshape(C, H * HS))
                    for l in range(L)])
    wv3 = np.stack([pack_mat(f(inputs["Wv"][l]).transpose(1, 0, 2).reshape(C, H * HS))
                    for l in range(L)])
    wp3 = np.stack([pack_mat(f(inputs["Wproj"][l])) for l in range(L)])
    w13 = np.stack([pack_mat(f(inputs["W1"][l])) for l in range(L)])
    w23 = np.stack([pack_mat(f(inputs["W2"][l])) for l in range(L)])

    def pack_vec(v):  # [256] -> [128, 2]
        return np.ascontiguousarray(f(v).reshape(2, P).T)

    vecs = np.stack([np.stack([pack_vec(inputs[k][l]) for k in
                               ("bproj", "b1", "b2")]).transpose(1, 0, 2)
                     for l in range(L)])
    vecs = np.ascontiguousarray(vecs)
    # vrow[l, row(g/b), ln_i, cc, :]: gamma/beta as contraction rows
    vrow = np.zeros((L, 2, 2, 2, P), np.float32)
    for l in range(L):
        for ln_i, (gk, bk) in enumerate((("ln1_g", "ln1_b"),
                                         ("ln2_g", "ln2_b"))):
            g = f(inputs[gk][l]).reshape(2, P)
            b = f(inputs[bk][l]).reshape(2, P)
            for cc in range(2):
                vrow[l, 0, ln_i, cc] = g[cc]
                vrow[l, 1, ln_i, cc] = b[cc]
    lnfrow = np.zeros((2, 2, P), np.float32)
    gf = f(inputs["lnf_g"]).reshape(2, P)
    bf = f(inputs["lnf_b"]).reshape(2, P)
    for cc in range(2):
        lnfrow[0, cc] = gf[cc]
        lnfrow[1, cc] = bf[cc]
    wc1 = pack_mat(f(inputs["Wc1"]) / T)        # fold mean-pool 1/T into Wc1
    bc1 = np.ascontiguousarray(f(inputs["bc1"]).reshape(CLS_H // P, P).T)
    wc2 = np.ascontiguousarray(f(inputs["Wc2"]).reshape(CLS_H // P, P, NOUT)
                               .transpose(1, 0, 2))
    bc2 = f(inputs["bc2"]).reshape(1, NOUT)
    tokf = f(inputs["tok_emb"])
    posf = f(inputs["pos_emb"])
    return dict(wq=wq3, wk=wk3, wv=wv3, wp=wp3, w1=w13, w2=w23, vecs=vecs,
                vrow=vrow, lnfrow=lnfrow, wc1=wc1, bc1=bc1, wc2=wc2, bc2=bc2,
                tok=tokf, pos=posf)


def _wrap_idx(ids):
    """int array [n] -> dma_gather wrapped layout [128, n//16] int16."""
    n = ids.shape[0]
    w = ids.reshape(n // 16, 16).T.astype(np.int16)     # [16, n//16]
    return np.ascontiguousarray(np.tile(w, (8, 1)))     # [128, n//16]


def _make_in_maps(inputs):
    shared = _prep_shared(inputs)
    idx = np.asarray(inputs["idx"]).astype(np.int64)
    in_maps = []
    for c in range(N_CORES):
        b, th = c // 2, c % 2
        t0 = th * TL
        idx_loc = idx[b, t0:t0 + TL]
        pos_loc = shared["pos"][t0:t0 + TL]  # [TL, C]
        posr_a = np.ascontiguousarray(
            pos_loc.reshape(TL // P, P, C).transpose(1, 0, 2))
        rem = (1 - th) * 2 * P + np.arange(2 * P, dtype=np.int64)
        m = dict(tok=shared["tok"], idxw=_wrap_idx(idx_loc), posr=posr_a,
                 remidx=_wrap_idx(rem),
                 wq=shared["wq"], wk=shared["wk"], wv=shared["wv"],
                 wp=shared["wp"], w1=shared["w1"], w2=shared["w2"],
                 vecs=shared["vecs"], vrow=shared["vrow"],
                 lnfrow=shared["lnfrow"], wc1=shared["wc1"],
                 bc1=shared["bc1"], wc2=shared["wc2"], bc2=shared["bc2"])
        in_maps.append(m)
    return in_maps


def kernel(**inputs) -> np.ndarray:
    if "nc" not in _CACHE:
        _CACHE["nc"] = _build_program()
    nc = _CACHE["nc"]
    in_maps = _make_in_maps(inputs)
    res = bass_utils.run_bass_kernel_spmd(nc, in_maps, core_ids=list(range(N_CORES)))
    out = np.zeros((B, NOUT), np.float32)
    for b in range(B):
        out[b] = res.results[2 * b]["probs"][0]
    return out


# revision 11
# speedup vs baseline: 1.0611x; 1.0611x over previous
"""Trainium2 Bass kernel for nn_EncoderWithClassifier (4-layer encoder + classifier).

Sharding: 8 cores, core c handles (batch b=c//2, sequence half th=c%2, 1024 tokens).
Canonical activation layout: x^T [C=256 (2 chunks of 128 partitions), T_local=1024].

Attention: scores transposed ([s_tile, t]) via row-packed K=32 matmuls (fp32r
single-pass PE), one exp per (st,g) over 2 heads. o^T accumulation uses a
33-column lhsT (v | ones) so each o matmul also emits the softmax row-sum in
PSUM partition rows 32/96 -- no separate row-sum matmuls; two heads pack per
accumulator via tile_position (0,0)/(0,64). The S->exp->o chain is
software-pipelined one iteration ahead so the in-order PE queue never blocks
on the Act engine's exp stream.

Layer pipeline: everything outside attention (proj, LN2, FFN, next-layer LN1 +
QKV) is per-token work, split into two 512-token half-chains and emitted as
"filler" steps interleaved into the next attention segment's instruction
stream -- PE/DVE chain work executes in the shadow of the Act-bound exp
stream. The 2-rank AllGather for the next layer's remote K/V is likewise
issued from filler, and remote-s score tiles are ordered last so the
collective latency hides under local-s compute.

LayerNorm: stats via packed matmuls; per-token affine folded into the
replication matmuls (lhsT = [g] and [g; b] rows) so the apply is 2 DVE ops per
chunk; x^2 runs on the idle GpSimd engine. A manual act-table load of the
ln+exp set at program start pins one table for the whole program (the
auto-inserted per-switch loads would otherwise cost ~23us).

PSUM budget (8 banks): S [128,1024] x2 bufs = 4, "acc" 2x[128,512] = 2,
"mm" 2x[128,512] = 2.
"""
import numpy as np

import concourse.bacc as bacc
import concourse.mybir as mybir
import concourse.tile as tile
from concourse import bass_utils, library_config
from concourse.masks import make_identity

V, C, TMAX, H, L = 32000, 256, 2048, 8, 4
HS, FFN = 32, 256
CLS_H, NOUT = 512, 10
B, T = 4, 2048
TL = 1024          # tokens per core
P = 128
EPS = 1e-5
SCALE = C ** (-0.5)
N_CORES = 8
dt = mybir.dt
F32 = dt.float32
Alu = mybir.AluOpType
Act = mybir.ActivationFunctionType
X_AXIS = mybir.AxisListType.X

LN_EXP_SET = 6     # act_info.json index of natural_log_exp_and_others

_CACHE = {}


def _build_program(sim=False):
    nc = bacc.Bacc("TRN2", target_bir_lowering=False, debug=False,
                   num_devices=1 if sim else N_CORES)

    # ---------------- dram I/O ----------------
    tok = nc.dram_tensor("tok", [V, C], F32, kind="ExternalInput")
    idxw = nc.dram_tensor("idxw", [P, TL // 16], dt.int16, kind="ExternalInput")
    posr = nc.dram_tensor("posr", [P, TL // P, C], F32, kind="ExternalInput")
    remidx = nc.dram_tensor("remidx", [P, (2 * P) // 16], dt.int16,
                            kind="ExternalInput")
    wq_d = nc.dram_tensor("wq", [L, P, 2, C], F32, kind="ExternalInput")
    wk_d = nc.dram_tensor("wk", [L, P, 2, C], F32, kind="ExternalInput")
    wv_d = nc.dram_tensor("wv", [L, P, 2, C], F32, kind="ExternalInput")
    wp_d = nc.dram_tensor("wp", [L, P, 2, C], F32, kind="ExternalInput")
    w1_d = nc.dram_tensor("w1", [L, P, 2, FFN], F32, kind="ExternalInput")
    w2_d = nc.dram_tensor("w2", [L, P, 2, C], F32, kind="ExternalInput")
    vecs_d = nc.dram_tensor("vecs", [L, P, 3, 2], F32, kind="ExternalInput")
    # vecs rows: 0 bproj, 1 b1, 2 b2
    vrow_d = nc.dram_tensor("vrow", [L, 2, 2, 2, P], F32, kind="ExternalInput")
    # vrow dims: [l, row(g=0,b=1), ln_i, cc, P]  (g/b transposed to rows)
    lnfrow_d = nc.dram_tensor("lnfrow", [2, 2, P], F32, kind="ExternalInput")
    wc1_d = nc.dram_tensor("wc1", [P, 2, CLS_H], F32, kind="ExternalInput")
    bc1_d = nc.dram_tensor("bc1", [P, CLS_H // P], F32, kind="ExternalInput")
    wc2_d = nc.dram_tensor("wc2", [P, CLS_H // P, NOUT], F32, kind="ExternalInput")
    bc2_d = nc.dram_tensor("bc2", [1, NOUT], F32, kind="ExternalInput")
    out_d = nc.dram_tensor("probs", [1, NOUT], F32, kind="ExternalOutput")

    REPL = [[0, 1], [2, 3], [4, 5], [6, 7]]

    with tile.TileContext(nc) as tc:
        with (
            tc.tile_pool(name="const", bufs=1) as cp,
            tc.tile_pool(name="work", bufs=1) as wk,
            tc.tile_pool(name="exp", bufs=3) as ep,
            tc.tile_pool(name="small", bufs=1) as sp,
            tc.tile_pool(name="psS", bufs=2, space="PSUM") as psS,
            tc.tile_pool(name="psA", bufs=2, space="PSUM") as psA,
            tc.tile_pool(name="psM", bufs=2, space="PSUM") as psM,
            tc.tile_pool(name="dram", bufs=2, space="DRAM") as dp,
        ):
            nc.gpsimd.load_library(library_config.mlp)

            R32 = dt.float32r

            def mm(out, lhsT, rhs, **kw):
                """matmul with fp32r operand views: single-pass PE (4x fp32)."""
                nc.tensor.matmul(out, lhsT=lhsT.bitcast(R32),
                                 rhs=rhs.bitcast(R32), **kw)

            # pin the ln+exp act table once; Ln and Exp then never reload
            nc.scalar.add_instruction(mybir.InstLoadActFuncSet(
                name=nc.get_next_instruction_name(),
                ins=[], outs=[], act_func_set_id=LN_EXP_SET))

            # ---------------- constants / weights to SBUF ----------------
            ident = cp.tile([P, P], F32, tag="ident")
            make_identity(nc, ident[:])
            inv256 = cp.tile([P, 1], F32, tag="inv256")
            nc.vector.memset(inv256[:], 1.0 / C)
            sel4 = cp.tile([4, P], F32, tag="sel4")
            nc.gpsimd.memset(sel4[:], 0.0)
            for j in range(4):
                nc.gpsimd.memset(sel4[j:j + 1, 32 * j:32 * (j + 1)], 1.0)

            def load_const(name, dram_ap, shape, dtype=F32):
                t = cp.tile(shape, dtype, tag=name, name=name)
                nc.sync.dma_start(t[:], dram_ap)
                return t

            wq = [load_const(f"wq{l}", wq_d[l], [P, 2, C]) for l in range(L)]
            wkt = [load_const(f"wk{l}", wk_d[l], [P, 2, C]) for l in range(L)]
            wv = [load_const(f"wv{l}", wv_d[l], [P, 2, C]) for l in range(L)]
            wp = [load_const(f"wp{l}", wp_d[l], [P, 2, C]) for l in range(L)]
            w1 = [load_const(f"w1{l}", w1_d[l], [P, 2, FFN]) for l in range(L)]
            w2 = [load_const(f"w2{l}", w2_d[l], [P, 2, C]) for l in range(L)]
            vecs = [load_const(f"vec{l}", vecs_d[l], [P, 3, 2]) for l in range(L)]
            vrow = [load_const(f"vrow{l}", vrow_d[l], [2, 2, 2, P])
                    for l in range(L)]
            lnfrow = load_const("lnfrow", lnfrow_d[:], [2, 2, P])
            wc1 = load_const("wc1", wc1_d[:], [P, 2, CLS_H])
            bc1 = load_const("bc1", bc1_d[:], [P, CLS_H // P])
            wc2 = load_const("wc2", wc2_d[:], [P, CLS_H // P, NOUT])
            bc2 = load_const("bc2", bc2_d[:], [1, NOUT])
            idx_sb = load_const("idx_sb", idxw[:], [P, TL // 16], dt.int16)
            remidx_sb = load_const("remidx_sb", remidx[:], [P, (2 * P) // 16],
                                   dt.int16)

            # vecs[l] rows: 0 bproj, 1 b1, 2 b2
            def vap(l, row, cc):
                return vecs[l][:, row, cc:cc + 1]

            # persistent activations
            xT = [wk.tile([P, TL], F32, tag=f"xT{cc}", name=f"xT{cc}")
                  for cc in range(2)]

            # ---------------- embedding ----------------
            with tc.tile_pool(name="embed", bufs=1) as ebp:
                xg = ebp.tile([P, TL // P, C], F32, tag="xg")
                nc.gpsimd.dma_gather(xg[:], tok[:], idx_sb[:], TL, TL, C)
                pos_sb = ebp.tile([P, TL // P, C], F32, tag="pos_sb")
                nc.sync.dma_start(pos_sb[:], posr[:])
                nc.vector.tensor_add(xg[:], xg[:], pos_sb[:])
                for tt in range(TL // P):
                    for cc in range(2):
                        tp = psM.tile([P, P], F32, tag="mm", name="tp")
                        nc.tensor.transpose(tp[:], xg[:, tt, cc * P:(cc + 1) * P],
                                            ident[:])
                        nc.vector.tensor_copy(xT[cc][:, tt * P:(tt + 1) * P], tp[:])

            # v with trailing ones column: o matmul emits row-sums for free
            v_sb = [wk.tile([P, H, HS + 1], F32, tag=f"v{st}", name=f"v{st}")
                    for st in range(16)]
            for st in range(16):
                nc.vector.memset(v_sb[st][:, :, HS:HS + 1], 1.0)
            # [mrs; -1] rows for the g*mrs - b replication matmul
            stM = sp.tile([2, TL], F32, tag="stM", name="stM")
            nc.vector.memset(stM[1:2, :], -1.0)

            stA = sp.tile([1, TL], F32, tag="stA")   # mu
            stB = sp.tile([1, TL], F32, tag="stB")   # msq -> var -> rstd
            stT = sp.tile([1, TL], F32, tag="stT")   # musq -> ln(var)
            xsq = [sp.tile([P, TL], F32, tag=f"lnsq{cc}", name=f"lnsq{cc}")
                   for cc in range(2)]

            hT = [wk.tile([P, TL], F32, tag=f"hT{cc}", name=f"hT{cc}")
                  for cc in range(2)]
            h2T = [wk.tile([P, TL], F32, tag=f"h2T{cc}", name=f"h2T{cc}")
                   for cc in range(2)]
            fT = [wk.tile([P, TL], F32, tag=f"fT{ff}", name=f"fT{ff}")
                  for ff in range(2)]
            oT = [wk.tile([P, TL], F32, tag=f"oT{cc}", name=f"oT{cc}")
                  for cc in range(2)]
            qT = [wk.tile([P, TL], F32, tag=f"qT{mt}", name=f"qT{mt}")
                  for mt in range(2)]
            kT = [wk.tile([P, T], F32, tag=f"kT{mt}", name=f"kT{mt}")
                  for mt in range(2)]
            hR = wk.tile([P, 2, TL], F32, tag="hR", name="hR")

            # -------- per-half op-list builders (each op = one emission) -----
            def ln_half_ops(src, grow, gbrow, out, half):
                """LayerNorm of 512-token half: list of emission closures."""
                sl = slice(half * 512, (half + 1) * 512)
                ops = []
                for cc in range(2):
                    ops.append(lambda cc=cc: nc.gpsimd.tensor_mul(
                        xsq[cc][:, sl], src[cc][:, sl], src[cc][:, sl]))

                def stats(dst, inp):
                    ps = psM.tile([1, 512], F32, tag="mm", name="stat")
                    for kc in range(2):
                        mm(ps[:], lhsT=inv256[:], rhs=inp[kc][:, sl],
                           start=(kc == 0), stop=(kc == 1))
                    nc.vector.tensor_copy(dst[:, sl], ps[:])
                ops.append(lambda: stats(stA, src))
                ops.append(lambda: stats(stB, xsq))

                def rows1():
                    nc.vector.tensor_mul(stT[:, sl], stA[:, sl], stA[:, sl])
                    nc.vector.scalar_tensor_tensor(stB[:, sl], stB[:, sl], EPS,
                                                   stT[:, sl], Alu.add,
                                                   Alu.subtract)
                ops.append(rows1)

                def rows2():
                    nc.scalar.activation(stT[:, sl], stB[:, sl], Act.Ln)
                    nc.scalar.activation(stB[:, sl], stT[:, sl], Act.Exp,
                                         scale=-0.5)
                    nc.vector.tensor_mul(stM[0:1, sl], stA[:, sl], stB[:, sl])
                ops.append(rows2)

                def apply(cc):
                    rep1 = psM.tile([P, 512], F32, tag="mm", name="rep1")
                    mm(rep1[:], lhsT=grow(cc), rhs=stB[:, sl],
                       start=True, stop=True)
                    rep2 = psM.tile([P, 512], F32, tag="mm", name="rep2")
                    mm(rep2[:], lhsT=gbrow(cc), rhs=stM[:, sl],
                       start=True, stop=True)
                    nc.vector.tensor_mul(out[cc][:, sl], src[cc][:, sl], rep1[:])
                    nc.vector.tensor_sub(out[cc][:, sl], out[cc][:, sl], rep2[:])
                for cc in range(2):
                    ops.append(lambda cc=cc: apply(cc))
                return ops

            def proj_half_ops(l, half):
                sl = slice(half * 512, (half + 1) * 512)
                ops = []

                def pj(cc):
                    dpj = psM.tile([P, 512], F32, tag="mm", name="dpj")
                    for kc in range(2):
                        mm(dpj[:], lhsT=wp[l][:, kc, cc * P:(cc + 1) * P],
                           rhs=oT[kc][:, sl], start=(kc == 0), stop=(kc == 1))
                    nc.vector.scalar_tensor_tensor(xT[cc][:, sl], dpj[:],
                                                   vap(l, 0, cc), xT[cc][:, sl],
                                                   Alu.add, Alu.add)
                for cc in range(2):
                    ops.append(lambda cc=cc: pj(cc))
                return ops

            def ffn_half_ops(l, half):
                sl = slice(half * 512, (half + 1) * 512)
                ops = []

                def f1(ff):
                    fps = psM.tile([P, 512], F32, tag="mm", name="fps")
                    for kc in range(2):
                        mm(fps[:], lhsT=w1[l][:, kc, ff * P:(ff + 1) * P],
                           rhs=h2T[kc][:, sl], start=(kc == 0), stop=(kc == 1))
                    nc.vector.tensor_scalar(fT[ff][:, sl], fps[:], vap(l, 1, ff),
                                            0.0, Alu.add, Alu.max)

                def f2(cc):
                    d2 = psM.tile([P, 512], F32, tag="mm", name="d2")
                    for kc in range(2):
                        mm(d2[:], lhsT=w2[l][:, kc, cc * P:(cc + 1) * P],
                           rhs=fT[kc][:, sl], start=(kc == 0), stop=(kc == 1))
                    nc.vector.scalar_tensor_tensor(xT[cc][:, sl], d2[:],
                                                   vap(l, 2, cc), xT[cc][:, sl],
                                                   Alu.add, Alu.add)
                for ff in range(2):
                    ops.append(lambda ff=ff: f1(ff))
                for cc in range(2):
                    ops.append(lambda cc=cc: f2(cc))
                return ops

            def q_half_ops(l, half):
                sl = slice(half * 512, (half + 1) * 512)
                ops = []

                def q(mt):
                    qps = psM.tile([P, 512], F32, tag="mm", name="qps")
                    for kc in range(2):
                        mm(qps[:], lhsT=wq[l][:, kc, mt * P:(mt + 1) * P],
                           rhs=hT[kc][:, sl], start=(kc == 0), stop=(kc == 1))
                    nc.vector.tensor_copy(qT[mt][:, sl], qps[:])
                for mt in range(2):
                    ops.append(lambda mt=mt: q(mt))
                return ops

            def k_ops(l, nch):
                """kT columns nch*512.. from local (nch<2) or remote half."""
                ops = []

                def k(mt):
                    kps = psM.tile([P, 512], F32, tag="mm", name="kps")
                    for kc in range(2):
                        if nch < 2:
                            rhs = hT[kc][:, nch * 512:(nch + 1) * 512]
                        else:
                            rhs = hR[:, kc, (nch - 2) * 512:(nch - 1) * 512]
                        mm(kps[:], lhsT=wkt[l][:, kc, mt * P:(mt + 1) * P],
                           rhs=rhs, start=(kc == 0), stop=(kc == 1))
                    nc.vector.tensor_copy(kT[mt][:, nch * 512:(nch + 1) * 512],
                                          kps[:])
                for mt in range(2):
                    ops.append(lambda mt=mt: k(mt))
                return ops

            def v_ops(l, sts):
                ops = []

                def v(st):
                    vps = psM.tile([P, H, HS], F32, tag="mm", name="vps")
                    for kc in range(2):
                        if st < 8:
                            lhsT = hT[kc][:, st * P:(st + 1) * P]
                        else:
                            lhsT = hR[:, kc, (st - 8) * P:(st - 7) * P]
                        mm(vps[:], lhsT=lhsT, rhs=wv[l][:, kc, :],
                           start=(kc == 0), stop=(kc == 1))
                    nc.vector.tensor_copy(v_sb[st][:, :, 0:HS], vps[:])
                for st in sts:
                    ops.append(lambda st=st: v(st))
                return ops

            def setup_ops(l):
                """AllGather hT -> hR, then remote-half K/V."""
                ops = []
                b_in = dp.tile([2 * P, TL], F32, tag="b_in", name="b_in")
                b_out = dp.tile([4 * P, TL], F32, tag="b_out", name="b_out")

                def send():
                    for cc in range(2):
                        nc.sync.dma_start(b_in[cc * P:(cc + 1) * P, :], hT[cc][:])

                def coll():
                    if sim:
                        nc.sync.dma_start(b_out[:2 * P, :], b_in[:])
                        nc.sync.dma_start(b_out[2 * P:, :], b_in[:])
                    else:
                        nc.gpsimd.collective_compute(
                            "AllGather", Alu.bypass, replica_groups=REPL,
                            ins=[b_in[:].opt()], outs=[b_out[:].opt()])

                def gath():
                    nc.gpsimd.dma_gather(hR[:], b_out[:], remidx_sb[:],
                                         2 * P, 2 * P, TL)
                ops.extend([send, coll, gath])
                ops.extend(k_ops(l, 2))
                ops.extend(k_ops(l, 3))
                ops.extend(v_ops(l, range(8, 12)))
                ops.extend(v_ops(l, range(12, 16)))
                return ops

            def ln1_rows(l):
                return (lambda cc: vrow[l][0:1, 0, cc, :],
                        lambda cc: vrow[l][:, 0, cc, :])

            def ln2_rows(l):
                return (lambda cc: vrow[l][0:1, 1, cc, :],
                        lambda cc: vrow[l][:, 1, cc, :])

            def lnf_rows():
                return (lambda cc: lnfrow[0:1, cc, :],
                        lambda cc: lnfrow[:, cc, :])

            def chain_ops(l, half):
                """proj -> LN2 -> FFN -> next LN1 (or final LN) for one half."""
                ops = []
                ops += proj_half_ops(l, half)
                g2, gb2 = ln2_rows(l)
                ops += ln_half_ops(xT, g2, gb2, h2T, half)
                ops += ffn_half_ops(l, half)
                if l + 1 < L:
                    g1, gb1 = ln1_rows(l + 1)
                    ops += ln_half_ops(xT, g1, gb1, hT, half)
                else:
                    gf, gbf = lnf_rows()
                    ops += ln_half_ops(xT, gf, gbf, h2T, half)
                return ops

            # -------------------- attention --------------------
            def run_attn(l, tcn, filler):
                tsl = slice(tcn * 512, (tcn + 1) * 512)
                fit = iter(filler) if filler is not None else None

                def step(n=1):
                    if fit is not None:
                        for _ in range(n):
                            op = next(fit, None)
                            if op is not None:
                                op()

                acc = {}

                def emit_S(hp, st, g):
                    S = psS.tile([P, 2 * 512], F32, tag="S", name="S")
                    for jj in range(2):
                        j = 2 * g + jj
                        mm(S[:, jj * 512:(jj + 1) * 512],
                           lhsT=kT[hp][32 * j:32 * (j + 1),
                                       st * P:(st + 1) * P],
                           rhs=qT[hp][32 * j:32 * (j + 1), tsl],
                           start=True, stop=True,
                           tile_position=(32 * j, 0))
                    expT = ep.tile([P, 2 * 512], F32, tag="expT", name="expT")
                    nc.scalar.activation(expT[:], S[:], Act.Exp, scale=SCALE)
                    return expT

                def emit_O(hp, st, g, expT):
                    for jj in range(2):
                        j = 2 * g + jj
                        mm(acc[hp][g][64 * jj:64 * jj + HS + 1, :],
                           lhsT=v_sb[st][:, hp * 4 + j, :],
                           rhs=expT[:, jj * 512:(jj + 1) * 512],
                           start=(st == 0), stop=(st == 15),
                           tile_position=(0, 64 * jj))

                def norm(hp):
                    rs4 = sp.tile([4, 512], F32, tag="rs4")
                    for g in range(2):
                        for jj in range(2):
                            nc.vector.tensor_copy(
                                rs4[2 * g + jj:2 * g + jj + 1, :],
                                acc[hp][g][64 * jj + HS:64 * jj + HS + 1, :])
                    rec4 = sp.tile([4, 512], F32, tag="rec4")
                    nc.vector.reciprocal(rec4[:], rs4[:])
                    rrep = psM.tile([P, 512], F32, tag="mm", name="rrep")
                    mm(rrep[:], lhsT=sel4[:], rhs=rec4[:], start=True, stop=True)
                    for j in range(4):
                        nc.vector.tensor_mul(
                            oT[hp][32 * j:32 * (j + 1), tsl],
                            acc[hp][j // 2][64 * (j % 2):64 * (j % 2) + HS, :],
                            rrep[32 * j:32 * (j + 1), :])

                pend = None
                for hp in range(2):
                    acc[hp] = [psA.tile([P, 512], F32, tag="acc",
                                        name=f"acc{g}") for g in range(2)]
                    for st in range(16):
                        for g in range(2):
                            cur = emit_S(hp, st, g)
                            step(2)
                            if pend is not None:
                                emit_O(*pend)
                                if pend[0] == 0 and pend[1] == 15 and pend[2] == 1:
                                    norm(0)
                            step(1)
                            pend = (hp, st, g, cur)
                emit_O(*pend)
                norm(1)
                # drain leftover filler
                step(1000)

            # -------------------- prologue: layer 0 setup --------------------
            g1, gb1 = ln1_rows(0)
            for half in range(2):
                for op in (ln_half_ops(xT, g1, gb1, hT, half)
                           + q_half_ops(0, half) + k_ops(0, half)
                           + v_ops(0, range(half * 4, half * 4 + 4))):
                    op()
            for op in setup_ops(0):
                op()

            # -------------------- layers --------------------
            pending = None
            for l in range(L):
                run_attn(l, 0, pending)
                fillerB = chain_ops(l, 0)
                if l + 1 < L:
                    fillerB += q_half_ops(l + 1, 0)
                    fillerB += k_ops(l + 1, 0)
                    fillerB += v_ops(l + 1, range(0, 4))
                run_attn(l, 1, fillerB)
                pending = chain_ops(l, 1)
                if l + 1 < L:
                    pending += q_half_ops(l + 1, 1)
                    pending += k_ops(l + 1, 1)
                    pending += v_ops(l + 1, range(4, 8))
                    pending += setup_ops(l + 1)
            # final-LN half 1 chain (wrote h2T) + half-0 result sits in h2T too
            for op in pending:
                op()

            # ---------------- pool + classifier ----------------
            # final LN output lives in h2T (both halves)
            emb = sp.tile([P, 2], F32, tag="emb")
            for cc in range(2):
                nc.vector.reduce_sum(emb[:, cc:cc + 1], h2T[cc][:], axis=X_AXIS)
            be_in = dp.tile([P, 2], F32, tag="be_in", name="be_in")
            be_out = dp.tile([P, 2], F32, tag="be_out", name="be_out")
            nc.sync.dma_start(be_in[:], emb[:])
            if sim:
                nc.sync.dma_start(be_out[:], be_in[:])
            else:
                nc.gpsimd.collective_compute(
                    "AllReduce", Alu.add, replica_groups=REPL,
                    ins=[be_in[:].opt()], outs=[be_out[:].opt()])
            embr = sp.tile([P, 2], F32, tag="embr")
            nc.sync.dma_start(embr[:], be_out[:])

            h1ps = psM.tile([P, CLS_H // P], F32, tag="mm", name="h1ps")
            for mt in range(CLS_H // P):
                for kc in range(2):
                    nc.tensor.matmul(h1ps[:, mt:mt + 1],
                                     lhsT=wc1[:, kc, mt * P:(mt + 1) * P],
                                     rhs=embr[:, kc:kc + 1],
                                     start=(kc == 0), stop=(kc == 1))
            h1 = sp.tile([P, CLS_H // P], F32, tag="h1")
            nc.vector.tensor_add(h1[:], h1ps[:], bc1[:])
            nc.vector.tensor_scalar_max(h1[:], h1[:], 0.0)
            lps = psM.tile([1, NOUT], F32, tag="mm", name="lps")
            for j in range(CLS_H // P):
                nc.tensor.matmul(lps[:], lhsT=h1[:, j:j + 1], rhs=wc2[:, j, :],
                                 start=(j == 0), stop=(j == CLS_H // P - 1))
            lsb = sp.tile([1, NOUT], F32, tag="lsb")
            nc.vector.tensor_add(lsb[:], lps[:], bc2[:])
            mx = sp.tile([1, 1], F32, tag="mx")
            nc.vector.tensor_reduce(mx[:], lsb[:], axis=X_AXIS, op=Alu.max)
            nmx = sp.tile([1, 1], F32, tag="nmx")
            nc.vector.tensor_scalar_mul(nmx[:], mx[:], -1.0)
            esb = sp.tile([1, NOUT], F32, tag="esb")
            nc.scalar.activation(esb[:], lsb[:], Act.Exp, bias=nmx[:])
            ssum = sp.tile([1, 1], F32, tag="ssum")
            nc.vector.reduce_sum(ssum[:], esb[:], axis=X_AXIS)
            rsum = sp.tile([1, 1], F32, tag="rsum")
            nc.vector.reciprocal(rsum[:], ssum[:])
            probs = sp.tile([1, NOUT], F32, tag="probs")
            nc.vector.tensor_single_scalar(probs[:], esb[:], rsum[:], Alu.mult)
            nc.sync.dma_start(out_d[:], probs[:])

    nc.compile()
    return nc


def _prep_shared(inputs):
    """Host-side weight prepack (identical for all cores)."""
    f = lambda a: np.ascontiguousarray(np.asarray(a, dtype=np.float32))

    def pack_mat(w):  # [C_in, M] -> [128, C_in//128, M]
        ci, m = w.shape
        return np.ascontiguousarray(w.reshape(ci // P, P, m).transpose(1, 0, 2))

    wq3 = np.stack([pack_mat(f(inputs["Wq"][l]).transpose(1, 0, 2).reshape(C, H * HS))
                    for l in range(L)])
    wk3 = np.stack([pack_mat(f(inputs["Wk"][l]).transpose(1, 0, 2).reshape(C, H * HS))
                    for l in range(L)])
    wv3 = np.stack([pack_mat(f(inputs["Wv"][l]).transpose(1, 0, 2).reshape(C, H * HS))
                    for l in range(L)])
    wp3 = np.stack([pack_mat(f(inputs["Wproj"][l])) for l in range(L)])
    w13 = np.stack([pack_mat(f(inputs["W1"][l])) for l in range(L)])
    w23 = np.stack([pack_mat(f(inputs["W2"][l])) for l in range(L)])

    def pack_vec(v):  # [256] -> [128, 2]
        return np.ascontiguousarray(f(v).reshape(2, P).T)

    vecs = np.stack([np.stack([pack_vec(inputs[k][l]) for k in
                               ("bproj", "b1", "b2")]).transpose(1, 0, 2)
                     for l in range(L)])
    vecs = np.ascontiguousarray(vecs)
    # vrow[l, row(g/b), ln_i, cc, :]: gamma/beta as contraction rows
    vrow = np.zeros((L, 2, 2, 2, P), np.float32)
    for l in range(L):
        for ln_i, (gk, bk) in enumerate((("ln1_g", "ln1_b"),
                                         ("ln2_g", "ln2_b"))):
            g = f(inputs[gk][l]).reshape(2, P)
            b = f(inputs[bk][l]).reshape(2, P)
            for cc in range(2):
                vrow[l, 0, ln_i, cc] = g[cc]
                vrow[l, 1, ln_i, cc] = b[cc]
    lnfrow = np.zeros((2, 2, P), np.float32)
    gf = f(inputs["lnf_g"]).reshape(2, P)
    bf = f(inputs["lnf_b"]).reshape(2, P)
    for cc in range(2):
        lnfrow[0, cc] = gf[cc]
        lnfrow[1, cc] = bf[cc]
    wc1 = pack_mat(f(inputs["Wc1"]) / T)        # fold mean-pool 1/T into Wc1
    bc1 = np.ascontiguousarray(f(inputs["bc1"]).reshape(CLS_H // P, P).T)
    wc2 = np.ascontiguousarray(f(inputs["Wc2"]).reshape(CLS_H // P, P, NOUT)
                               .transpose(1, 0, 2))
    bc2 = f(inputs["bc2"]).reshape(1, NOUT)
    tokf = f(inputs["tok_emb"])
    posf = f(inputs["pos_emb"])
    return dict(wq=wq3, wk=wk3, wv=wv3, wp=wp3, w1=w13, w2=w23, vecs=vecs,
                vrow=vrow, lnfrow=lnfrow, wc1=wc1, bc1=bc1, wc2=wc2, bc2=bc2,
                tok=tokf, pos=posf)


def _wrap_idx(ids):
    """int array [n] -> dma_gather wrapped layout [128, n//16] int16."""
    n = ids.shape[0]
    w = ids.reshape(n // 16, 16).T.astype(np.int16)     # [16, n//16]
    return np.ascontiguousarray(np.tile(w, (8, 1)))     # [128, n//16]


def _make_in_maps(inputs):
    shared = _prep_shared(inputs)
    idx = np.asarray(inputs["idx"]).astype(np.int64)
    in_maps = []
    for c in range(N_CORES):
        b, th = c // 2, c % 2
        t0 = th * TL
        idx_loc = idx[b, t0:t0 + TL]
        pos_loc = shared["pos"][t0:t0 + TL]  # [TL, C]
        posr_a = np.ascontiguousarray(
            pos_loc.reshape(TL // P, P, C).transpose(1, 0, 2))
        rem = (1 - th) * 2 * P + np.arange(2 * P, dtype=np.int64)
        m = dict(tok=shared["tok"], idxw=_wrap_idx(idx_loc), posr=posr_a,
                 remidx=_wrap_idx(rem),
                 wq=shared["wq"], wk=shared["wk"], wv=shared["wv"],
                 wp=shared["wp"], w1=shared["w1"], w2=shared["w2"],
                 vecs=shared["vecs"], vrow=shared["vrow"],
                 lnfrow=shared["lnfrow"], wc1=shared["wc1"],
                 bc1=shared["bc1"], wc2=shared["wc2"], bc2=shared["bc2"])
        in_maps.append(m)
    return in_maps


def kernel(**inputs) -> np.ndarray:
    if "nc" not in _CACHE:
        _CACHE["nc"] = _build_program()
    nc = _CACHE["nc"]
    in_maps = _make_in_maps(inputs)
    res = bass_utils.run_bass_kernel_spmd(nc, in_maps, core_ids=list(range(N_CORES)))
    out = np.zeros((B, NOUT), np.float32)
    for b in range(B):
        out[b] = res.results[2 * b]["probs"][0]
    return out


# revision 12
# speedup vs baseline: 1.0868x; 1.0243x over previous
"""Trainium2 Bass kernel for nn_EncoderWithClassifier (4-layer encoder + classifier).

Sharding: 8 cores, core c handles (batch b=c//2, sequence half th=c%2, 1024 tokens).
Canonical activation layout: x^T [C=256 (2 chunks of 128 partitions), T_local=1024].

Attention: scores transposed ([s_tile, t]) via row-packed K=32 matmuls (fp32r
single-pass PE), one exp per (st,g) over 2 heads. o^T accumulation uses a
33-column lhsT (v | ones) so each o matmul also emits the softmax row-sum in
PSUM partition rows 32/96 -- no separate row-sum matmuls; two heads pack per
accumulator via tile_position (0,0)/(0,64). The S->exp->o chain is
software-pipelined one iteration ahead so the in-order PE queue never blocks
on the Act engine's exp stream.

Layer pipeline: everything outside attention (proj, LN2, FFN, next-layer LN1 +
QKV) is per-token work, split into two 512-token half-chains and emitted as
"filler" steps interleaved into the next attention segment's instruction
stream -- PE/DVE chain work executes in the shadow of the Act-bound exp
stream. The 2-rank AllGather for the next layer's remote K/V is likewise
issued from filler, and remote-s score tiles are ordered last so the
collective latency hides under local-s compute.

LayerNorm: stats via packed matmuls; per-token affine folded into the
replication matmuls (lhsT = [g] and [g; b] rows) so the apply is 2 DVE ops per
chunk; x^2 runs on the idle GpSimd engine. A manual act-table load of the
ln+exp set at program start pins one table for the whole program (the
auto-inserted per-switch loads would otherwise cost ~23us).

PSUM budget (8 banks): S [128,1024] x2 bufs = 4, "acc" 2x[128,512] = 2,
"mm" 2x[128,512] = 2.
"""
import numpy as np

import concourse.bacc as bacc
import concourse.mybir as mybir
import concourse.tile as tile
from concourse import bass_utils, library_config
from concourse.masks import make_identity

V, C, TMAX, H, L = 32000, 256, 2048, 8, 4
HS, FFN = 32, 256
CLS_H, NOUT = 512, 10
B, T = 4, 2048
TL = 1024          # tokens per core
P = 128
EPS = 1e-5
SCALE = C ** (-0.5)
N_CORES = 8
dt = mybir.dt
F32 = dt.float32
Alu = mybir.AluOpType
Act = mybir.ActivationFunctionType
X_AXIS = mybir.AxisListType.X

LN_EXP_SET = 6     # act_info.json index of natural_log_exp_and_others

_CACHE = {}


def _build_program(sim=False):
    nc = bacc.Bacc("TRN2", target_bir_lowering=False, debug=False,
                   num_devices=1 if sim else N_CORES)

    # ---------------- dram I/O ----------------
    tok = nc.dram_tensor("tok", [V, C], F32, kind="ExternalInput")
    idxw = nc.dram_tensor("idxw", [P, TL // 16], dt.int16, kind="ExternalInput")
    posr = nc.dram_tensor("posr", [P, TL // P, C], F32, kind="ExternalInput")
    remidx = nc.dram_tensor("remidx", [P, (2 * P) // 16], dt.int16,
                            kind="ExternalInput")
    wq_d = nc.dram_tensor("wq", [L, P, 2, C], F32, kind="ExternalInput")
    wk_d = nc.dram_tensor("wk", [L, P, 2, C], F32, kind="ExternalInput")
    wv_d = nc.dram_tensor("wv", [L, P, 2, C], F32, kind="ExternalInput")
    wp_d = nc.dram_tensor("wp", [L, P, 2, C], F32, kind="ExternalInput")
    w1_d = nc.dram_tensor("w1", [L, P, 2, FFN], F32, kind="ExternalInput")
    w2_d = nc.dram_tensor("w2", [L, P, 2, C], F32, kind="ExternalInput")
    vecs_d = nc.dram_tensor("vecs", [L, P, 3, 2], F32, kind="ExternalInput")
    # vecs rows: 0 bproj, 1 b1, 2 b2
    vrow_d = nc.dram_tensor("vrow", [L, 2, 2, 2, P], F32, kind="ExternalInput")
    # vrow dims: [l, row(g=0,b=1), ln_i, cc, P]  (g/b transposed to rows)
    lnfrow_d = nc.dram_tensor("lnfrow", [2, 2, P], F32, kind="ExternalInput")
    wc1_d = nc.dram_tensor("wc1", [P, 2, CLS_H], F32, kind="ExternalInput")
    bc1_d = nc.dram_tensor("bc1", [P, CLS_H // P], F32, kind="ExternalInput")
    wc2_d = nc.dram_tensor("wc2", [P, CLS_H // P, NOUT], F32, kind="ExternalInput")
    bc2_d = nc.dram_tensor("bc2", [1, NOUT], F32, kind="ExternalInput")
    out_d = nc.dram_tensor("probs", [1, NOUT], F32, kind="ExternalOutput")

    REPL = [[0, 1], [2, 3], [4, 5], [6, 7]]

    with tile.TileContext(nc) as tc:
        with (
            tc.tile_pool(name="const", bufs=1) as cp,
            tc.tile_pool(name="work", bufs=1) as wk,
            tc.tile_pool(name="exp", bufs=3) as ep,
            tc.tile_pool(name="small", bufs=1) as sp,
            tc.tile_pool(name="psS", bufs=2, space="PSUM") as psS,
            tc.tile_pool(name="psA", bufs=2, space="PSUM") as psA,
            tc.tile_pool(name="psM", bufs=2, space="PSUM") as psM,
            tc.tile_pool(name="dram", bufs=2, space="DRAM") as dp,
        ):
            nc.gpsimd.load_library(library_config.mlp)

            R32 = dt.float32r

            def mm(out, lhsT, rhs, **kw):
                """matmul with fp32r operand views: single-pass PE (4x fp32)."""
                nc.tensor.matmul(out, lhsT=lhsT.bitcast(R32),
                                 rhs=rhs.bitcast(R32), **kw)

            # pin the ln+exp act table once; Ln and Exp then never reload
            nc.scalar.add_instruction(mybir.InstLoadActFuncSet(
                name=nc.get_next_instruction_name(),
                ins=[], outs=[], act_func_set_id=LN_EXP_SET))

            # ---------------- constants / weights to SBUF ----------------
            ident = cp.tile([P, P], F32, tag="ident")
            make_identity(nc, ident[:])
            inv256 = cp.tile([P, 1], F32, tag="inv256")
            nc.vector.memset(inv256[:], 1.0 / C)
            sel4 = cp.tile([4, P], F32, tag="sel4")
            nc.gpsimd.memset(sel4[:], 0.0)
            for j in range(4):
                nc.gpsimd.memset(sel4[j:j + 1, 32 * j:32 * (j + 1)], 1.0)

            def load_const(name, dram_ap, shape, dtype=F32):
                t = cp.tile(shape, dtype, tag=name, name=name)
                nc.sync.dma_start(t[:], dram_ap)
                return t

            wq = [load_const(f"wq{l}", wq_d[l], [P, 2, C]) for l in range(L)]
            wkt = [load_const(f"wk{l}", wk_d[l], [P, 2, C]) for l in range(L)]
            wv = [load_const(f"wv{l}", wv_d[l], [P, 2, C]) for l in range(L)]
            wp = [load_const(f"wp{l}", wp_d[l], [P, 2, C]) for l in range(L)]
            w1 = [load_const(f"w1{l}", w1_d[l], [P, 2, FFN]) for l in range(L)]
            w2 = [load_const(f"w2{l}", w2_d[l], [P, 2, C]) for l in range(L)]
            vecs = [load_const(f"vec{l}", vecs_d[l], [P, 3, 2]) for l in range(L)]
            vrow = [load_const(f"vrow{l}", vrow_d[l], [2, 2, 2, P])
                    for l in range(L)]
            lnfrow = load_const("lnfrow", lnfrow_d[:], [2, 2, P])
            wc1 = load_const("wc1", wc1_d[:], [P, 2, CLS_H])
            bc1 = load_const("bc1", bc1_d[:], [P, CLS_H // P])
            wc2 = load_const("wc2", wc2_d[:], [P, CLS_H // P, NOUT])
            bc2 = load_const("bc2", bc2_d[:], [1, NOUT])
            idx_sb = load_const("idx_sb", idxw[:], [P, TL // 16], dt.int16)
            remidx_sb = load_const("remidx_sb", remidx[:], [P, (2 * P) // 16],
                                   dt.int16)

            # vecs[l] rows: 0 bproj, 1 b1, 2 b2
            def vap(l, row, cc):
                return vecs[l][:, row, cc:cc + 1]

            # persistent activations
            xT = [wk.tile([P, TL], F32, tag=f"xT{cc}", name=f"xT{cc}")
                  for cc in range(2)]

            # ---------------- embedding ----------------
            with tc.tile_pool(name="embed", bufs=1) as ebp:
                xg = ebp.tile([P, TL // P, C], F32, tag="xg")
                nc.gpsimd.dma_gather(xg[:], tok[:], idx_sb[:], TL, TL, C)
                pos_sb = ebp.tile([P, TL // P, C], F32, tag="pos_sb")
                nc.sync.dma_start(pos_sb[:], posr[:])
                nc.vector.tensor_add(xg[:], xg[:], pos_sb[:])
                for tt in range(TL // P):
                    for cc in range(2):
                        tp = psM.tile([P, P], F32, tag="mm", name="tp")
                        nc.tensor.transpose(tp[:], xg[:, tt, cc * P:(cc + 1) * P],
                                            ident[:])
                        nc.vector.tensor_copy(xT[cc][:, tt * P:(tt + 1) * P], tp[:])

            # v with trailing ones column: o matmul emits row-sums for free
            v_sb = [wk.tile([P, H, HS + 1], F32, tag=f"v{st}", name=f"v{st}")
                    for st in range(16)]
            for st in range(16):
                nc.vector.memset(v_sb[st][:, :, HS:HS + 1], 1.0)
            # [mrs; -1] rows for the g*mrs - b replication matmul
            stM = sp.tile([2, TL], F32, tag="stM", name="stM")
            nc.vector.memset(stM[1:2, :], -1.0)

            stA = sp.tile([1, TL], F32, tag="stA")   # mu
            stB = sp.tile([1, TL], F32, tag="stB")   # msq -> var -> rstd
            stT = sp.tile([1, TL], F32, tag="stT")   # musq -> ln(var)
            xsq = [sp.tile([P, TL], F32, tag=f"lnsq{cc}", name=f"lnsq{cc}")
                   for cc in range(2)]

            hT = [wk.tile([P, TL], F32, tag=f"hT{cc}", name=f"hT{cc}")
                  for cc in range(2)]
            h2T = [wk.tile([P, TL], F32, tag=f"h2T{cc}", name=f"h2T{cc}")
                   for cc in range(2)]
            fT = [wk.tile([P, TL], F32, tag=f"fT{ff}", name=f"fT{ff}")
                  for ff in range(2)]
            oT = [wk.tile([P, TL], F32, tag=f"oT{cc}", name=f"oT{cc}")
                  for cc in range(2)]
            qT = [wk.tile([P, TL], F32, tag=f"qT{mt}", name=f"qT{mt}")
                  for mt in range(2)]
            kT = [wk.tile([P, T], F32, tag=f"kT{mt}", name=f"kT{mt}")
                  for mt in range(2)]
            hR = wk.tile([P, 2, TL], F32, tag="hR", name="hR")

            # -------- per-half op-list builders (each op = one emission) -----
            def ln_half_ops(src, grow, gbrow, out, half):
                """LayerNorm of 512-token half: list of emission closures."""
                sl = slice(half * 512, (half + 1) * 512)
                ops = []
                for cc in range(2):
                    ops.append(lambda cc=cc: nc.gpsimd.tensor_mul(
                        xsq[cc][:, sl], src[cc][:, sl], src[cc][:, sl]))

                def stats(dst, inp):
                    ps = psM.tile([1, 512], F32, tag="mm", name="stat")
                    for kc in range(2):
                        mm(ps[:], lhsT=inv256[:], rhs=inp[kc][:, sl],
                           start=(kc == 0), stop=(kc == 1))
                    nc.vector.tensor_copy(dst[:, sl], ps[:])
                ops.append(lambda: stats(stA, src))
                ops.append(lambda: stats(stB, xsq))

                def rows1():
                    nc.vector.tensor_mul(stT[:, sl], stA[:, sl], stA[:, sl])
                    nc.vector.scalar_tensor_tensor(stB[:, sl], stB[:, sl], EPS,
                                                   stT[:, sl], Alu.add,
                                                   Alu.subtract)
                ops.append(rows1)

                def rows2():
                    nc.scalar.activation(stT[:, sl], stB[:, sl], Act.Ln)
                    nc.scalar.activation(stB[:, sl], stT[:, sl], Act.Exp,
                                         scale=-0.5)
                    nc.vector.tensor_mul(stM[0:1, sl], stA[:, sl], stB[:, sl])
                ops.append(rows2)

                def apply(cc):
                    rep1 = psM.tile([P, 512], F32, tag="mm", name="rep1")
                    mm(rep1[:], lhsT=grow(cc), rhs=stB[:, sl],
                       start=True, stop=True)
                    rep2 = psM.tile([P, 512], F32, tag="mm", name="rep2")
                    mm(rep2[:], lhsT=gbrow(cc), rhs=stM[:, sl],
                       start=True, stop=True)
                    nc.vector.tensor_mul(out[cc][:, sl], src[cc][:, sl], rep1[:])
                    nc.vector.tensor_sub(out[cc][:, sl], out[cc][:, sl], rep2[:])
                for cc in range(2):
                    ops.append(lambda cc=cc: apply(cc))
                return ops

            def proj_half_ops(l, half):
                sl = slice(half * 512, (half + 1) * 512)
                ops = []

                def pj(cc):
                    dpj = psM.tile([P, 512], F32, tag="mm", name="dpj")
                    for kc in range(2):
                        mm(dpj[:], lhsT=wp[l][:, kc, cc * P:(cc + 1) * P],
                           rhs=oT[kc][:, sl], start=(kc == 0), stop=(kc == 1))
                    nc.vector.scalar_tensor_tensor(xT[cc][:, sl], dpj[:],
                                                   vap(l, 0, cc), xT[cc][:, sl],
                                                   Alu.add, Alu.add)
                for cc in range(2):
                    ops.append(lambda cc=cc: pj(cc))
                return ops

            def ffn_half_ops(l, half):
                sl = slice(half * 512, (half + 1) * 512)
                ops = []

                def f1(ff):
                    fps = psM.tile([P, 512], F32, tag="mm", name="fps")
                    for kc in range(2):
                        mm(fps[:], lhsT=w1[l][:, kc, ff * P:(ff + 1) * P],
                           rhs=h2T[kc][:, sl], start=(kc == 0), stop=(kc == 1))
                    nc.vector.tensor_scalar(fT[ff][:, sl], fps[:], vap(l, 1, ff),
                                            0.0, Alu.add, Alu.max)

                def f2(cc):
                    d2 = psM.tile([P, 512], F32, tag="mm", name="d2")
                    for kc in range(2):
                        mm(d2[:], lhsT=w2[l][:, kc, cc * P:(cc + 1) * P],
                           rhs=fT[kc][:, sl], start=(kc == 0), stop=(kc == 1))
                    nc.vector.scalar_tensor_tensor(xT[cc][:, sl], d2[:],
                                                   vap(l, 2, cc), xT[cc][:, sl],
                                                   Alu.add, Alu.add)
                for ff in range(2):
                    ops.append(lambda ff=ff: f1(ff))
                for cc in range(2):
                    ops.append(lambda cc=cc: f2(cc))
                return ops

            def q_half_ops(l, half):
                sl = slice(half * 512, (half + 1) * 512)
                ops = []

                def q(mt):
                    qps = psM.tile([P, 512], F32, tag="mm", name="qps")
                    for kc in range(2):
                        mm(qps[:], lhsT=wq[l][:, kc, mt * P:(mt + 1) * P],
                           rhs=hT[kc][:, sl], start=(kc == 0), stop=(kc == 1))
                    nc.vector.tensor_copy(qT[mt][:, sl], qps[:])
                for mt in range(2):
                    ops.append(lambda mt=mt: q(mt))
                return ops

            def k_ops(l, nch):
                """kT columns nch*512.. from local (nch<2) or remote half."""
                ops = []

                def k(mt):
                    kps = psM.tile([P, 512], F32, tag="mm", name="kps")
                    for kc in range(2):
                        if nch < 2:
                            rhs = hT[kc][:, nch * 512:(nch + 1) * 512]
                        else:
                            rhs = hR[:, kc, (nch - 2) * 512:(nch - 1) * 512]
                        mm(kps[:], lhsT=wkt[l][:, kc, mt * P:(mt + 1) * P],
                           rhs=rhs, start=(kc == 0), stop=(kc == 1))
                    nc.vector.tensor_copy(kT[mt][:, nch * 512:(nch + 1) * 512],
                                          kps[:])
                for mt in range(2):
                    ops.append(lambda mt=mt: k(mt))
                return ops

            def v_ops(l, sts):
                ops = []

                def v(st):
                    vps = psM.tile([P, H, HS], F32, tag="mm", name="vps")
                    for kc in range(2):
                        if st < 8:
                            lhsT = hT[kc][:, st * P:(st + 1) * P]
                        else:
                            lhsT = hR[:, kc, (st - 8) * P:(st - 7) * P]
                        mm(vps[:], lhsT=lhsT, rhs=wv[l][:, kc, :],
                           start=(kc == 0), stop=(kc == 1))
                    nc.vector.tensor_copy(v_sb[st][:, :, 0:HS], vps[:])
                for st in sts:
                    ops.append(lambda st=st: v(st))
                return ops

            def setup_ops(l):
                """AllGather hT -> hR, then remote-half K/V."""
                ops = []
                b_in = dp.tile([2 * P, TL], F32, tag="b_in", name="b_in")
                b_out = dp.tile([4 * P, TL], F32, tag="b_out", name="b_out")

                def send():
                    for cc in range(2):
                        nc.sync.dma_start(b_in[cc * P:(cc + 1) * P, :], hT[cc][:])

                def coll():
                    if sim:
                        nc.sync.dma_start(b_out[:2 * P, :], b_in[:])
                        nc.sync.dma_start(b_out[2 * P:, :], b_in[:])
                    else:
                        nc.gpsimd.collective_compute(
                            "AllGather", Alu.bypass, replica_groups=REPL,
                            ins=[b_in[:].opt()], outs=[b_out[:].opt()])

                def gath():
                    nc.gpsimd.dma_gather(hR[:], b_out[:], remidx_sb[:],
                                         2 * P, 2 * P, TL)
                ops.extend([send, coll, gath])
                ops.extend(k_ops(l, 2))
                ops.extend(k_ops(l, 3))
                ops.extend(v_ops(l, range(8, 12)))
                ops.extend(v_ops(l, range(12, 16)))
                return ops

            def ln1_rows(l):
                return (lambda cc: vrow[l][0:1, 0, cc, :],
                        lambda cc: vrow[l][:, 0, cc, :])

            def ln2_rows(l):
                return (lambda cc: vrow[l][0:1, 1, cc, :],
                        lambda cc: vrow[l][:, 1, cc, :])

            def lnf_rows():
                return (lambda cc: lnfrow[0:1, cc, :],
                        lambda cc: lnfrow[:, cc, :])

            def chain_ops(l, half):
                """proj -> LN2 -> FFN -> next LN1 (or final LN) for one half."""
                ops = []
                ops += proj_half_ops(l, half)
                g2, gb2 = ln2_rows(l)
                ops += ln_half_ops(xT, g2, gb2, h2T, half)
                ops += ffn_half_ops(l, half)
                if l + 1 < L:
                    g1, gb1 = ln1_rows(l + 1)
                    ops += ln_half_ops(xT, g1, gb1, hT, half)
                else:
                    gf, gbf = lnf_rows()
                    ops += ln_half_ops(xT, gf, gbf, h2T, half)
                return ops

            # -------------------- attention --------------------
            def run_attn(l, tcn, filler):
                tsl = slice(tcn * 512, (tcn + 1) * 512)
                fit = iter(filler) if filler is not None else None

                def step(n=1):
                    if fit is not None:
                        for _ in range(n):
                            op = next(fit, None)
                            if op is not None:
                                op()

                acc = {}

                def emit_S(hp, st, g):
                    S = psS.tile([P, 2 * 512], F32, tag="S", name="S")
                    for jj in range(2):
                        j = 2 * g + jj
                        mm(S[:, jj * 512:(jj + 1) * 512],
                           lhsT=kT[hp][32 * j:32 * (j + 1),
                                       st * P:(st + 1) * P],
                           rhs=qT[hp][32 * j:32 * (j + 1), tsl],
                           start=True, stop=True,
                           tile_position=(32 * j, 0))
                    expT = ep.tile([P, 2 * 512], F32, tag="expT", name="expT")
                    nc.scalar.activation(expT[:], S[:], Act.Exp, scale=SCALE)
                    return expT

                def emit_O(hp, st, g, expT):
                    for jj in range(2):
                        j = 2 * g + jj
                        mm(acc[hp][g][64 * jj:64 * jj + HS + 1, :],
                           lhsT=v_sb[st][:, hp * 4 + j, :],
                           rhs=expT[:, jj * 512:(jj + 1) * 512],
                           start=(st == 0), stop=(st == 15),
                           tile_position=(0, 64 * jj))

                def norm(hp):
                    rs4 = sp.tile([4, 512], F32, tag="rs4")
                    for g in range(2):
                        for jj in range(2):
                            nc.vector.tensor_copy(
                                rs4[2 * g + jj:2 * g + jj + 1, :],
                                acc[hp][g][64 * jj + HS:64 * jj + HS + 1, :])
                    rec4 = sp.tile([4, 512], F32, tag="rec4")
                    nc.vector.reciprocal(rec4[:], rs4[:])
                    rrep = psM.tile([P, 512], F32, tag="mm", name="rrep")
                    mm(rrep[:], lhsT=sel4[:], rhs=rec4[:], start=True, stop=True)
                    for j in range(4):
                        nc.vector.tensor_mul(
                            oT[hp][32 * j:32 * (j + 1), tsl],
                            acc[hp][j // 2][64 * (j % 2):64 * (j % 2) + HS, :],
                            rrep[32 * j:32 * (j + 1), :])

                pend = None
                for hp in range(2):
                    acc[hp] = [psA.tile([P, 512], F32, tag="acc",
                                        name=f"acc{g}") for g in range(2)]
                    for st in range(16):
                        for g in range(2):
                            cur = emit_S(hp, st, g)
                            step()
                            if pend is not None:
                                emit_O(*pend)
                                if pend[0] == 0 and pend[1] == 15 and pend[2] == 1:
                                    norm(0)
                            step()
                            pend = (hp, st, g, cur)
                emit_O(*pend)
                norm(1)
                # drain leftover filler
                step(1000)

            # -------------------- prologue: layer 0 setup --------------------
            g1, gb1 = ln1_rows(0)
            for half in range(2):
                for op in (ln_half_ops(xT, g1, gb1, hT, half)
                           + q_half_ops(0, half) + k_ops(0, half)
                           + v_ops(0, range(half * 4, half * 4 + 4))):
                    op()
            for op in setup_ops(0):
                op()

            # -------------------- layers --------------------
            pending = None
            for l in range(L):
                run_attn(l, 0, pending)
                fillerB = chain_ops(l, 0)
                if l + 1 < L:
                    fillerB += q_half_ops(l + 1, 0)
                    fillerB += k_ops(l + 1, 0)
                    fillerB += v_ops(l + 1, range(0, 4))
                run_attn(l, 1, fillerB)
                pending = chain_ops(l, 1)
                if l + 1 < L:
                    pending += q_half_ops(l + 1, 1)
                    pending += k_ops(l + 1, 1)
                    pending += v_ops(l + 1, range(4, 8))
                    pending += setup_ops(l + 1)
            # final-LN half 1 chain (wrote h2T) + half-0 result sits in h2T too
            for op in pending:
                op()

            # ---------------- pool + classifier ----------------
            # final LN output lives in h2T (both halves)
            emb = sp.tile([P, 2], F32, tag="emb")
            for cc in range(2):
                nc.vector.reduce_sum(emb[:, cc:cc + 1], h2T[cc][:], axis=X_AXIS)
            be_in = dp.tile([P, 2], F32, tag="be_in", name="be_in")
            be_out = dp.tile([P, 2], F32, tag="be_out", name="be_out")
            nc.sync.dma_start(be_in[:], emb[:])
            if sim:
                nc.sync.dma_start(be_out[:], be_in[:])
            else:
                nc.gpsimd.collective_compute(
                    "AllReduce", Alu.add, replica_groups=REPL,
                    ins=[be_in[:].opt()], outs=[be_out[:].opt()])
            embr = sp.tile([P, 2], F32, tag="embr")
            nc.sync.dma_start(embr[:], be_out[:])

            h1ps = psM.tile([P, CLS_H // P], F32, tag="mm", name="h1ps")
            for mt in range(CLS_H // P):
                for kc in range(2):
                    nc.tensor.matmul(h1ps[:, mt:mt + 1],
                                     lhsT=wc1[:, kc, mt * P:(mt + 1) * P],
                                     rhs=embr[:, kc:kc + 1],
                                     start=(kc == 0), stop=(kc == 1))
            h1 = sp.tile([P, CLS_H // P], F32, tag="h1")
            nc.vector.tensor_add(h1[:], h1ps[:], bc1[:])
            nc.vector.tensor_scalar_max(h1[:], h1[:], 0.0)
            lps = psM.tile([1, NOUT], F32, tag="mm", name="lps")
            for j in range(CLS_H // P):
                nc.tensor.matmul(lps[:], lhsT=h1[:, j:j + 1], rhs=wc2[:, j, :],
                                 start=(j == 0), stop=(j == CLS_H // P - 1))
            lsb = sp.tile([1, NOUT], F32, tag="lsb")
            nc.vector.tensor_add(lsb[:], lps[:], bc2[:])
            mx = sp.tile([1, 1], F32, tag="mx")
            nc.vector.tensor_reduce(mx[:], lsb[:], axis=X_AXIS, op=Alu.max)
            nmx = sp.tile([1, 1], F32, tag="nmx")
            nc.vector.tensor_scalar_mul(nmx[:], mx[:], -1.0)
            esb = sp.tile([1, NOUT], F32, tag="esb")
            nc.scalar.activation(esb[:], lsb[:], Act.Exp, bias=nmx[:])
            ssum = sp.tile([1, 1], F32, tag="ssum")
            nc.vector.reduce_sum(ssum[:], esb[:], axis=X_AXIS)
            rsum = sp.tile([1, 1], F32, tag="rsum")
            nc.vector.reciprocal(rsum[:], ssum[:])
            probs = sp.tile([1, NOUT], F32, tag="probs")
            nc.vector.tensor_single_scalar(probs[:], esb[:], rsum[:], Alu.mult)
            nc.sync.dma_start(out_d[:], probs[:])

    nc.compile()
    return nc


def _prep_shared(inputs):
    """Host-side weight prepack (identical for all cores)."""
    f = lambda a: np.ascontiguousarray(np.asarray(a, dtype=np.float32))

    def pack_mat(w):  # [C_in, M] -> [128, C_in//128, M]
        ci, m = w.shape
        return np.ascontiguousarray(w.reshape(ci // P, P, m).transpose(1, 0, 2))

    wq3 = np.stack([pack_mat(f(inputs["Wq"][l]).transpose(1, 0, 2).reshape(C, H * HS))
                    for l in range(L)])
    wk3 = np.stack([pack_mat(f(inputs["Wk"][l]).transpose(1, 0, 2).reshape(C, H * HS))
                    for l in range(L)])
    wv3 = np.stack([pack_mat(f(inputs["Wv"][l]).transpose(1, 0, 2).reshape(C, H * HS))
                    for l in range(L)])
    wp3 = np.stack([pack_mat(f(inputs["Wproj"][l])) for l in range(L)])
    w13 = np.stack([pack_mat(f(inputs["W1"][l])) for l in range(L)])
    w23 = np.stack([pack_mat(f(inputs["W2"][l])) for l in range(L)])

    def pack_vec(v):  # [256] -> [128, 2]
        return np.ascontiguousarray(f(v).reshape(2, P).T)

    vecs = np.stack([np.stack([pack_vec(inputs[k][l]) for k in
                               ("bproj", "b1", "b2")]).transpose(1, 0, 2)
                     for l in range(L)])
    vecs = np.ascontiguousarray(vecs)
    # vrow[l, row(g/b), ln_i, cc, :]: gamma/beta as contraction rows
    vrow = np.zeros((L, 2, 2, 2, P), np.float32)
    for l in range(L):
        for ln_i, (gk, bk) in enumerate((("ln1_g", "ln1_b"),
                                         ("ln2_g", "ln2_b"))):
            g = f(inputs[gk][l]).reshape(2, P)
            b = f(inputs[bk][l]).reshape(2, P)
            for cc in range(2):
                vrow[l, 0, ln_i, cc] = g[cc]
                vrow[l, 1, ln_i, cc] = b[cc]
    lnfrow = np.zeros((2, 2, P), np.float32)
    gf = f(inputs["lnf_g"]).reshape(2, P)
    bf = f(inputs["lnf_b"]).reshape(2, P)
    for cc in range(2):
        lnfrow[0, cc] = gf[cc]
        lnfrow[1, cc] = bf[cc]
    wc1 = pack_mat(f(inputs["Wc1"]) / T)        # fold mean-pool 1/T into Wc1
    bc1 = np.ascontiguousarray(f(inputs["bc1"]).reshape(CLS_H // P, P).T)
    wc2 = np.ascontiguousarray(f(inputs["Wc2"]).reshape(CLS_H // P, P, NOUT)
                               .transpose(1, 0, 2))
    bc2 = f(inputs["bc2"]).reshape(1, NOUT)
    tokf = f(inputs["tok_emb"])
    posf = f(inputs["pos_emb"])
    return dict(wq=wq3, wk=wk3, wv=wv3, wp=wp3, w1=w13, w2=w23, vecs=vecs,
                vrow=vrow, lnfrow=lnfrow, wc1=wc1, bc1=bc1, wc2=wc2, bc2=bc2,
                tok=tokf, pos=posf)


def _wrap_idx(ids):
    """int array [n] -> dma_gather wrapped layout [128, n//16] int16."""
    n = ids.shape[0]
    w = ids.reshape(n // 16, 16).T.astype(np.int16)     # [16, n//16]
    return np.ascontiguousarray(np.tile(w, (8, 1)))     # [128, n//16]


def _make_in_maps(inputs):
    shared = _prep_shared(inputs)
    idx = np.asarray(inputs["idx"]).astype(np.int64)
    in_maps = []
    for c in range(N_CORES):
        b, th = c // 2, c % 2
        t0 = th * TL
        idx_loc = idx[b, t0:t0 + TL]
        pos_loc = shared["pos"][t0:t0 + TL]  # [TL, C]
        posr_a = np.ascontiguousarray(
            pos_loc.reshape(TL // P, P, C).transpose(1, 0, 2))
        rem = (1 - th) * 2 * P + np.arange(2 * P, dtype=np.int64)
        m = dict(tok=shared["tok"], idxw=_wrap_idx(idx_loc), posr=posr_a,
                 remidx=_wrap_idx(rem),
                 wq=shared["wq"], wk=shared["wk"], wv=shared["wv"],
                 wp=shared["wp"], w1=shared["w1"], w2=shared["w2"],
                 vecs=shared["vecs"], vrow=shared["vrow"],
                 lnfrow=shared["lnfrow"], wc1=shared["wc1"],
                 bc1=shared["bc1"], wc2=shared["wc2"], bc2=shared["bc2"])
        in_maps.append(m)
    return in_maps


def kernel(**inputs) -> np.ndarray:
    if "nc" not in _CACHE:
        _CACHE["nc"] = _build_program()
    nc = _CACHE["nc"]
    in_maps = _make_in_maps(inputs)
    res = bass_utils.run_bass_kernel_spmd(nc, in_maps, core_ids=list(range(N_CORES)))
    out = np.zeros((B, NOUT), np.float32)
    for b in range(B):
        out[b] = res.results[2 * b]["probs"][0]
    return out


# revision 13
# speedup vs baseline: 1.0902x; 1.0031x over previous
"""Trainium2 Bass kernel for nn_EncoderWithClassifier (4-layer encoder + classifier).

Sharding: 8 cores, core c handles (batch b=c//2, sequence half th=c%2, 1024 tokens).
Canonical activation layout: x^T [C=256 (2 chunks of 128 partitions), T_local=1024].

Attention: scores transposed ([s_tile, t]) via row-packed K=32 matmuls (fp32r
single-pass PE), one exp per (st,g) over 2 heads. o^T accumulation uses a
33-column lhsT (v | ones) so each o matmul also emits the softmax row-sum in
PSUM partition rows 32/96 -- no separate row-sum matmuls; two heads pack per
accumulator via tile_position (0,0)/(0,64). The S->exp->o chain is
software-pipelined one iteration ahead so the in-order PE queue never blocks
on the Act engine's exp stream.

Layer pipeline: everything outside attention (proj, LN2, FFN, next-layer LN1 +
QKV) is per-token work, split into two 512-token half-chains and emitted as
"filler" steps interleaved into the next attention segment's instruction
stream -- PE/DVE chain work executes in the shadow of the Act-bound exp
stream. The 2-rank AllGather for the next layer's remote K/V is likewise
issued from filler, and remote-s score tiles are ordered last so the
collective latency hides under local-s compute.

LayerNorm: stats via packed matmuls; per-token affine folded into the
replication matmuls (lhsT = [g] and [g; b] rows) so the apply is 2 DVE ops per
chunk; x^2 runs on the idle GpSimd engine. A manual act-table load of the
ln+exp set at program start pins one table for the whole program (the
auto-inserted per-switch loads would otherwise cost ~23us).

PSUM budget (8 banks): S [128,1024] x2 bufs = 4, "acc" 2x[128,512] = 2,
"mm" 2x[128,512] = 2.
"""
import numpy as np

import concourse.bacc as bacc
import concourse.mybir as mybir
import concourse.tile as tile
from concourse import bass_utils, library_config
from concourse.masks import make_identity

V, C, TMAX, H, L = 32000, 256, 2048, 8, 4
HS, FFN = 32, 256
CLS_H, NOUT = 512, 10
B, T = 4, 2048
TL = 1024          # tokens per core
P = 128
EPS = 1e-5
SCALE = C ** (-0.5)
N_CORES = 8
dt = mybir.dt
F32 = dt.float32
Alu = mybir.AluOpType
Act = mybir.ActivationFunctionType
X_AXIS = mybir.AxisListType.X

LN_EXP_SET = 6     # act_info.json index of natural_log_exp_and_others

_CACHE = {}


def _build_program(sim=False):
    nc = bacc.Bacc("TRN2", target_bir_lowering=False, debug=False,
                   num_devices=1 if sim else N_CORES)

    # ---------------- dram I/O ----------------
    tok = nc.dram_tensor("tok", [V, C], F32, kind="ExternalInput")
    idxw = nc.dram_tensor("idxw", [P, TL // 16], dt.int16, kind="ExternalInput")
    posr = nc.dram_tensor("posr", [P, TL // P, C], F32, kind="ExternalInput")
    remidx = nc.dram_tensor("remidx", [P, (2 * P) // 16], dt.int16,
                            kind="ExternalInput")
    wq_d = nc.dram_tensor("wq", [L, P, 2, C], F32, kind="ExternalInput")
    wk_d = nc.dram_tensor("wk", [L, P, 2, C], F32, kind="ExternalInput")
    wv_d = nc.dram_tensor("wv", [L, P, 2, C], F32, kind="ExternalInput")
    wp_d = nc.dram_tensor("wp", [L, P, 2, C], F32, kind="ExternalInput")
    w1_d = nc.dram_tensor("w1", [L, P, 2, FFN], F32, kind="ExternalInput")
    w2_d = nc.dram_tensor("w2", [L, P, 2, C], F32, kind="ExternalInput")
    vecs_d = nc.dram_tensor("vecs", [L, P, 3, 2], F32, kind="ExternalInput")
    # vecs rows: 0 bproj, 1 b1, 2 b2
    vrow_d = nc.dram_tensor("vrow", [L, 2, 2, 2, P], F32, kind="ExternalInput")
    # vrow dims: [l, row(g=0,b=1), ln_i, cc, P]  (g/b transposed to rows)
    lnfrow_d = nc.dram_tensor("lnfrow", [2, 2, P], F32, kind="ExternalInput")
    wc1_d = nc.dram_tensor("wc1", [P, 2, CLS_H], F32, kind="ExternalInput")
    bc1_d = nc.dram_tensor("bc1", [P, CLS_H // P], F32, kind="ExternalInput")
    wc2_d = nc.dram_tensor("wc2", [P, CLS_H // P, NOUT], F32, kind="ExternalInput")
    bc2_d = nc.dram_tensor("bc2", [1, NOUT], F32, kind="ExternalInput")
    out_d = nc.dram_tensor("probs", [1, NOUT], F32, kind="ExternalOutput")

    REPL = [[0, 1], [2, 3], [4, 5], [6, 7]]

    with tile.TileContext(nc) as tc:
        with (
            tc.tile_pool(name="const", bufs=1) as cp,
            tc.tile_pool(name="work", bufs=1) as wk,
            tc.tile_pool(name="exp", bufs=3) as ep,
            tc.tile_pool(name="small", bufs=1) as sp,
            tc.tile_pool(name="psS", bufs=2, space="PSUM") as psS,
            tc.tile_pool(name="psA", bufs=2, space="PSUM") as psA,
            tc.tile_pool(name="psM", bufs=2, space="PSUM") as psM,
            tc.tile_pool(name="dram", bufs=2, space="DRAM") as dp,
        ):
            nc.gpsimd.load_library(library_config.mlp)

            R32 = dt.float32r

            def mm(out, lhsT, rhs, **kw):
                """matmul with fp32r operand views: single-pass PE (4x fp32)."""
                nc.tensor.matmul(out, lhsT=lhsT.bitcast(R32),
                                 rhs=rhs.bitcast(R32), **kw)

            # pin the ln+exp act table once; Ln and Exp then never reload
            nc.scalar.add_instruction(mybir.InstLoadActFuncSet(
                name=nc.get_next_instruction_name(),
                ins=[], outs=[], act_func_set_id=LN_EXP_SET))

            # ---------------- constants / weights to SBUF ----------------
            ident = cp.tile([P, P], F32, tag="ident")
            make_identity(nc, ident[:])
            inv256 = cp.tile([P, 1], F32, tag="inv256")
            nc.vector.memset(inv256[:], 1.0 / C)
            sel4 = cp.tile([4, P], F32, tag="sel4")
            nc.gpsimd.memset(sel4[:], 0.0)
            for j in range(4):
                nc.gpsimd.memset(sel4[j:j + 1, 32 * j:32 * (j + 1)], 1.0)

            def load_const(name, dram_ap, shape, dtype=F32):
                t = cp.tile(shape, dtype, tag=name, name=name)
                nc.sync.dma_start(t[:], dram_ap)
                return t

            # embed/layer-0-critical tensors first: the DMA queue drains in
            # order and the embed gather + LN1(0) + qkv(0) are the first users
            idx_sb = load_const("idx_sb", idxw[:], [P, TL // 16], dt.int16)
            remidx_sb = load_const("remidx_sb", remidx[:], [P, (2 * P) // 16],
                                   dt.int16)
            vrow = [load_const(f"vrow{l}", vrow_d[l], [2, 2, 2, P])
                    for l in range(L)]
            wq, wkt, wv, wp, w1, w2, vecs = [], [], [], [], [], [], []
            for l in range(L):
                wq.append(load_const(f"wq{l}", wq_d[l], [P, 2, C]))
                wkt.append(load_const(f"wk{l}", wk_d[l], [P, 2, C]))
                wv.append(load_const(f"wv{l}", wv_d[l], [P, 2, C]))
                wp.append(load_const(f"wp{l}", wp_d[l], [P, 2, C]))
                w1.append(load_const(f"w1{l}", w1_d[l], [P, 2, FFN]))
                w2.append(load_const(f"w2{l}", w2_d[l], [P, 2, C]))
                vecs.append(load_const(f"vec{l}", vecs_d[l], [P, 3, 2]))
            lnfrow = load_const("lnfrow", lnfrow_d[:], [2, 2, P])
            wc1 = load_const("wc1", wc1_d[:], [P, 2, CLS_H])
            bc1 = load_const("bc1", bc1_d[:], [P, CLS_H // P])
            wc2 = load_const("wc2", wc2_d[:], [P, CLS_H // P, NOUT])
            bc2 = load_const("bc2", bc2_d[:], [1, NOUT])

            # vecs[l] rows: 0 bproj, 1 b1, 2 b2
            def vap(l, row, cc):
                return vecs[l][:, row, cc:cc + 1]

            # persistent activations
            xT = [wk.tile([P, TL], F32, tag=f"xT{cc}", name=f"xT{cc}")
                  for cc in range(2)]

            # ---------------- embedding ----------------
            with tc.tile_pool(name="embed", bufs=1) as ebp:
                xg = ebp.tile([P, TL // P, C], F32, tag="xg")
                nc.gpsimd.dma_gather(xg[:], tok[:], idx_sb[:], TL, TL, C)
                pos_sb = ebp.tile([P, TL // P, C], F32, tag="pos_sb")
                nc.sync.dma_start(pos_sb[:], posr[:])
                nc.vector.tensor_add(xg[:], xg[:], pos_sb[:])
                for tt in range(TL // P):
                    for cc in range(2):
                        tp = psM.tile([P, P], F32, tag="mm", name="tp")
                        nc.tensor.transpose(tp[:], xg[:, tt, cc * P:(cc + 1) * P],
                                            ident[:])
                        nc.vector.tensor_copy(xT[cc][:, tt * P:(tt + 1) * P], tp[:])

            # v with trailing ones column: o matmul emits row-sums for free
            v_sb = [wk.tile([P, H, HS + 1], F32, tag=f"v{st}", name=f"v{st}")
                    for st in range(16)]
            for st in range(16):
                nc.vector.memset(v_sb[st][:, :, HS:HS + 1], 1.0)
            # [mrs; -1] rows for the g*mrs - b replication matmul
            stM = sp.tile([2, TL], F32, tag="stM", name="stM")
            nc.vector.memset(stM[1:2, :], -1.0)

            stA = sp.tile([1, TL], F32, tag="stA")   # mu
            stB = sp.tile([1, TL], F32, tag="stB")   # msq -> var -> rstd
            stT = sp.tile([1, TL], F32, tag="stT")   # musq -> ln(var)
            xsq = [sp.tile([P, TL], F32, tag=f"lnsq{cc}", name=f"lnsq{cc}")
                   for cc in range(2)]

            hT = [wk.tile([P, TL], F32, tag=f"hT{cc}", name=f"hT{cc}")
                  for cc in range(2)]
            h2T = [wk.tile([P, TL], F32, tag=f"h2T{cc}", name=f"h2T{cc}")
                   for cc in range(2)]
            fT = [wk.tile([P, TL], F32, tag=f"fT{ff}", name=f"fT{ff}")
                  for ff in range(2)]
            oT = [wk.tile([P, TL], F32, tag=f"oT{cc}", name=f"oT{cc}")
                  for cc in range(2)]
            qT = [wk.tile([P, TL], F32, tag=f"qT{mt}", name=f"qT{mt}")
                  for mt in range(2)]
            kT = [wk.tile([P, T], F32, tag=f"kT{mt}", name=f"kT{mt}")
                  for mt in range(2)]
            hR = wk.tile([P, 2, TL], F32, tag="hR", name="hR")

            # -------- per-half op-list builders (each op = one emission) -----
            def ln_half_ops(src, grow, gbrow, out, half):
                """LayerNorm of 512-token half: list of emission closures."""
                sl = slice(half * 512, (half + 1) * 512)
                ops = []
                for cc in range(2):
                    ops.append(lambda cc=cc: nc.gpsimd.tensor_mul(
                        xsq[cc][:, sl], src[cc][:, sl], src[cc][:, sl]))

                def stats(dst, inp):
                    ps = psM.tile([1, 512], F32, tag="mm", name="stat")
                    for kc in range(2):
                        mm(ps[:], lhsT=inv256[:], rhs=inp[kc][:, sl],
                           start=(kc == 0), stop=(kc == 1))
                    nc.vector.tensor_copy(dst[:, sl], ps[:])
                ops.append(lambda: stats(stA, src))
                ops.append(lambda: stats(stB, xsq))

                def rows1():
                    nc.vector.tensor_mul(stT[:, sl], stA[:, sl], stA[:, sl])
                    nc.vector.scalar_tensor_tensor(stB[:, sl], stB[:, sl], EPS,
                                                   stT[:, sl], Alu.add,
                                                   Alu.subtract)
                ops.append(rows1)

                def rows2():
                    nc.scalar.activation(stT[:, sl], stB[:, sl], Act.Ln)
                    nc.scalar.activation(stB[:, sl], stT[:, sl], Act.Exp,
                                         scale=-0.5)
                    nc.vector.tensor_mul(stM[0:1, sl], stA[:, sl], stB[:, sl])
                ops.append(rows2)

                def apply(cc):
                    rep1 = psM.tile([P, 512], F32, tag="mm", name="rep1")
                    mm(rep1[:], lhsT=grow(cc), rhs=stB[:, sl],
                       start=True, stop=True)
                    rep2 = psM.tile([P, 512], F32, tag="mm", name="rep2")
                    mm(rep2[:], lhsT=gbrow(cc), rhs=stM[:, sl],
                       start=True, stop=True)
                    nc.vector.tensor_mul(out[cc][:, sl], src[cc][:, sl], rep1[:])
                    nc.vector.tensor_sub(out[cc][:, sl], out[cc][:, sl], rep2[:])
                for cc in range(2):
                    ops.append(lambda cc=cc: apply(cc))
                return ops

            def proj_half_ops(l, half):
                sl = slice(half * 512, (half + 1) * 512)
                ops = []

                def pj(cc):
                    dpj = psM.tile([P, 512], F32, tag="mm", name="dpj")
                    for kc in range(2):
                        mm(dpj[:], lhsT=wp[l][:, kc, cc * P:(cc + 1) * P],
                           rhs=oT[kc][:, sl], start=(kc == 0), stop=(kc == 1))
                    nc.vector.scalar_tensor_tensor(xT[cc][:, sl], dpj[:],
                                                   vap(l, 0, cc), xT[cc][:, sl],
                                                   Alu.add, Alu.add)
                for cc in range(2):
                    ops.append(lambda cc=cc: pj(cc))
                return ops

            def ffn_half_ops(l, half):
                sl = slice(half * 512, (half + 1) * 512)
                ops = []

                def f1(ff):
                    fps = psM.tile([P, 512], F32, tag="mm", name="fps")
                    for kc in range(2):
                        mm(fps[:], lhsT=w1[l][:, kc, ff * P:(ff + 1) * P],
                           rhs=h2T[kc][:, sl], start=(kc == 0), stop=(kc == 1))
                    nc.vector.tensor_scalar(fT[ff][:, sl], fps[:], vap(l, 1, ff),
                                            0.0, Alu.add, Alu.max)

                def f2(cc):
                    d2 = psM.tile([P, 512], F32, tag="mm", name="d2")
                    for kc in range(2):
                        mm(d2[:], lhsT=w2[l][:, kc, cc * P:(cc + 1) * P],
                           rhs=fT[kc][:, sl], start=(kc == 0), stop=(kc == 1))
                    nc.vector.scalar_tensor_tensor(xT[cc][:, sl], d2[:],
                                                   vap(l, 2, cc), xT[cc][:, sl],
                                                   Alu.add, Alu.add)
                for ff in range(2):
                    ops.append(lambda ff=ff: f1(ff))
                for cc in range(2):
                    ops.append(lambda cc=cc: f2(cc))
                return ops

            def q_half_ops(l, half):
                sl = slice(half * 512, (half + 1) * 512)
                ops = []

                def q(mt):
                    qps = psM.tile([P, 512], F32, tag="mm", name="qps")
                    for kc in range(2):
                        mm(qps[:], lhsT=wq[l][:, kc, mt * P:(mt + 1) * P],
                           rhs=hT[kc][:, sl], start=(kc == 0), stop=(kc == 1))
                    nc.vector.tensor_copy(qT[mt][:, sl], qps[:])
                for mt in range(2):
                    ops.append(lambda mt=mt: q(mt))
                return ops

            def k_ops(l, nch):
                """kT columns nch*512.. from local (nch<2) or remote half."""
                ops = []

                def k(mt):
                    kps = psM.tile([P, 512], F32, tag="mm", name="kps")
                    for kc in range(2):
                        if nch < 2:
                            rhs = hT[kc][:, nch * 512:(nch + 1) * 512]
                        else:
                            rhs = hR[:, kc, (nch - 2) * 512:(nch - 1) * 512]
                        mm(kps[:], lhsT=wkt[l][:, kc, mt * P:(mt + 1) * P],
                           rhs=rhs, start=(kc == 0), stop=(kc == 1))
                    nc.vector.tensor_copy(kT[mt][:, nch * 512:(nch + 1) * 512],
                                          kps[:])
                for mt in range(2):
                    ops.append(lambda mt=mt: k(mt))
                return ops

            def v_ops(l, sts):
                ops = []

                def v(st):
                    vps = psM.tile([P, H, HS], F32, tag="mm", name="vps")
                    for kc in range(2):
                        if st < 8:
                            lhsT = hT[kc][:, st * P:(st + 1) * P]
                        else:
                            lhsT = hR[:, kc, (st - 8) * P:(st - 7) * P]
                        mm(vps[:], lhsT=lhsT, rhs=wv[l][:, kc, :],
                           start=(kc == 0), stop=(kc == 1))
                    nc.vector.tensor_copy(v_sb[st][:, :, 0:HS], vps[:])
                for st in sts:
                    ops.append(lambda st=st: v(st))
                return ops

            def setup_ops(l):
                """AllGather hT -> hR, then remote-half K/V."""
                ops = []
                b_in = dp.tile([2 * P, TL], F32, tag="b_in", name="b_in")
                b_out = dp.tile([4 * P, TL], F32, tag="b_out", name="b_out")

                def send():
                    for cc in range(2):
                        nc.sync.dma_start(b_in[cc * P:(cc + 1) * P, :], hT[cc][:])

                def coll():
                    if sim:
                        nc.sync.dma_start(b_out[:2 * P, :], b_in[:])
                        nc.sync.dma_start(b_out[2 * P:, :], b_in[:])
                    else:
                        nc.gpsimd.collective_compute(
                            "AllGather", Alu.bypass, replica_groups=REPL,
                            ins=[b_in[:].opt()], outs=[b_out[:].opt()])

                def gath():
                    nc.gpsimd.dma_gather(hR[:], b_out[:], remidx_sb[:],
                                         2 * P, 2 * P, TL)
                ops.extend([send, coll, gath])
                ops.extend(k_ops(l, 2))
                ops.extend(k_ops(l, 3))
                ops.extend(v_ops(l, range(8, 12)))
                ops.extend(v_ops(l, range(12, 16)))
                return ops

            def ln1_rows(l):
                return (lambda cc: vrow[l][0:1, 0, cc, :],
                        lambda cc: vrow[l][:, 0, cc, :])

            def ln2_rows(l):
                return (lambda cc: vrow[l][0:1, 1, cc, :],
                        lambda cc: vrow[l][:, 1, cc, :])

            def lnf_rows():
                return (lambda cc: lnfrow[0:1, cc, :],
                        lambda cc: lnfrow[:, cc, :])

            def chain_ops(l, half):
                """proj -> LN2 -> FFN -> next LN1 (or final LN) for one half."""
                ops = []
                ops += proj_half_ops(l, half)
                g2, gb2 = ln2_rows(l)
                ops += ln_half_ops(xT, g2, gb2, h2T, half)
                ops += ffn_half_ops(l, half)
                if l + 1 < L:
                    g1, gb1 = ln1_rows(l + 1)
                    ops += ln_half_ops(xT, g1, gb1, hT, half)
                else:
                    gf, gbf = lnf_rows()
                    ops += ln_half_ops(xT, gf, gbf, h2T, half)
                return ops

            # -------------------- attention --------------------
            def run_attn(l, tcn, filler):
                tsl = slice(tcn * 512, (tcn + 1) * 512)
                fit = iter(filler) if filler is not None else None

                def step(n=1):
                    if fit is not None:
                        for _ in range(n):
                            op = next(fit, None)
                            if op is not None:
                                op()

                acc = {}

                def emit_S(hp, st, g):
                    S = psS.tile([P, 2 * 512], F32, tag="S", name="S")
                    for jj in range(2):
                        j = 2 * g + jj
                        mm(S[:, jj * 512:(jj + 1) * 512],
                           lhsT=kT[hp][32 * j:32 * (j + 1),
                                       st * P:(st + 1) * P],
                           rhs=qT[hp][32 * j:32 * (j + 1), tsl],
                           start=True, stop=True,
                           tile_position=(32 * j, 0))
                    expT = ep.tile([P, 2 * 512], F32, tag="expT", name="expT")
                    nc.scalar.activation(expT[:], S[:], Act.Exp, scale=SCALE)
                    return expT

                def emit_O(hp, st, g, expT):
                    for jj in range(2):
                        j = 2 * g + jj
                        mm(acc[hp][g][64 * jj:64 * jj + HS + 1, :],
                           lhsT=v_sb[st][:, hp * 4 + j, :],
                           rhs=expT[:, jj * 512:(jj + 1) * 512],
                           start=(st == 0), stop=(st == 15),
                           tile_position=(0, 64 * jj))

                def norm(hp):
                    rs4 = sp.tile([4, 512], F32, tag="rs4")
                    for g in range(2):
                        for jj in range(2):
                            nc.vector.tensor_copy(
                                rs4[2 * g + jj:2 * g + jj + 1, :],
                                acc[hp][g][64 * jj + HS:64 * jj + HS + 1, :])
                    rec4 = sp.tile([4, 512], F32, tag="rec4")
                    nc.vector.reciprocal(rec4[:], rs4[:])
                    rrep = psM.tile([P, 512], F32, tag="mm", name="rrep")
                    mm(rrep[:], lhsT=sel4[:], rhs=rec4[:], start=True, stop=True)
                    for j in range(4):
                        nc.vector.tensor_mul(
                            oT[hp][32 * j:32 * (j + 1), tsl],
                            acc[hp][j // 2][64 * (j % 2):64 * (j % 2) + HS, :],
                            rrep[32 * j:32 * (j + 1), :])

                pend = None
                for hp in range(2):
                    acc[hp] = [psA.tile([P, 512], F32, tag="acc",
                                        name=f"acc{g}") for g in range(2)]
                    for st in range(16):
                        for g in range(2):
                            cur = emit_S(hp, st, g)
                            step()
                            if pend is not None:
                                emit_O(*pend)
                                if pend[0] == 0 and pend[1] == 15 and pend[2] == 1:
                                    norm(0)
                            step()
                            pend = (hp, st, g, cur)
                emit_O(*pend)
                norm(1)
                # drain leftover filler
                step(1000)

            # -------------------- prologue: layer 0 setup --------------------
            g1, gb1 = ln1_rows(0)
            for half in range(2):
                for op in (ln_half_ops(xT, g1, gb1, hT, half)
                           + q_half_ops(0, half) + k_ops(0, half)
                           + v_ops(0, range(half * 4, half * 4 + 4))):
                    op()
            for op in setup_ops(0):
                op()

            # -------------------- layers --------------------
            pending = None
            for l in range(L):
                run_attn(l, 0, pending)
                fillerB = chain_ops(l, 0)
                if l + 1 < L:
                    fillerB += q_half_ops(l + 1, 0)
                    fillerB += k_ops(l + 1, 0)
                    fillerB += v_ops(l + 1, range(0, 4))
                run_attn(l, 1, fillerB)
                pending = chain_ops(l, 1)
                if l + 1 < L:
                    pending += q_half_ops(l + 1, 1)
                    pending += k_ops(l + 1, 1)
                    pending += v_ops(l + 1, range(4, 8))
                    pending += setup_ops(l + 1)
            # final-LN half 1 chain (wrote h2T) + half-0 result sits in h2T too
            for op in pending:
                op()

            # ---------------- pool + classifier ----------------
            # final LN output lives in h2T (both halves)
            emb = sp.tile([P, 2], F32, tag="emb")
            for cc in range(2):
                nc.vector.reduce_sum(emb[:, cc:cc + 1], h2T[cc][:], axis=X_AXIS)
            be_in = dp.tile([P, 2], F32, tag="be_in", name="be_in")
            be_out = dp.tile([P, 2], F32, tag="be_out", name="be_out")
            nc.sync.dma_start(be_in[:], emb[:])
            if sim:
                nc.sync.dma_start(be_out[:], be_in[:])
            else:
                nc.gpsimd.collective_compute(
                    "AllReduce", Alu.add, replica_groups=REPL,
                    ins=[be_in[:].opt()], outs=[be_out[:].opt()])
            embr = sp.tile([P, 2], F32, tag="embr")
            nc.sync.dma_start(embr[:], be_out[:])

            h1ps = psM.tile([P, CLS_H // P], F32, tag="mm", name="h1ps")
            for mt in range(CLS_H // P):
                for kc in range(2):
                    nc.tensor.matmul(h1ps[:, mt:mt + 1],
                                     lhsT=wc1[:, kc, mt * P:(mt + 1) * P],
                                     rhs=embr[:, kc:kc + 1],
                                     start=(kc == 0), stop=(kc == 1))
            h1 = sp.tile([P, CLS_H // P], F32, tag="h1")
            nc.vector.tensor_add(h1[:], h1ps[:], bc1[:])
            nc.vector.tensor_scalar_max(h1[:], h1[:], 0.0)
            lps = psM.tile([1, NOUT], F32, tag="mm", name="lps")
            for j in range(CLS_H // P):
                nc.tensor.matmul(lps[:], lhsT=h1[:, j:j + 1], rhs=wc2[:, j, :],
                                 start=(j == 0), stop=(j == CLS_H // P - 1))
            lsb = sp.tile([1, NOUT], F32, tag="lsb")
            nc.vector.tensor_add(lsb[:], lps[:], bc2[:])
            mx = sp.tile([1, 1], F32, tag="mx")
            nc.vector.tensor_reduce(mx[:], lsb[:], axis=X_AXIS, op=Alu.max)
            nmx = sp.tile([1, 1], F32, tag="nmx")
            nc.vector.tensor_scalar_mul(nmx[:], mx[:], -1.0)
            esb = sp.tile([1, NOUT], F32, tag="esb")
            nc.scalar.activation(esb[:], lsb[:], Act.Exp, bias=nmx[:])
            ssum = sp.tile([1, 1], F32, tag="ssum")
            nc.vector.reduce_sum(ssum[:], esb[:], axis=X_AXIS)
            rsum = sp.tile([1, 1], F32, tag="rsum")
            nc.vector.reciprocal(rsum[:], ssum[:])
            probs = sp.tile([1, NOUT], F32, tag="probs")
            nc.vector.tensor_single_scalar(probs[:], esb[:], rsum[:], Alu.mult)
            nc.sync.dma_start(out_d[:], probs[:])

    nc.compile()
    return nc


def _prep_shared(inputs):
    """Host-side weight prepack (identical for all cores)."""
    f = lambda a: np.ascontiguousarray(np.asarray(a, dtype=np.float32))

    def pack_mat(w):  # [C_in, M] -> [128, C_in//128, M]
        ci, m = w.shape
        return np.ascontiguousarray(w.reshape(ci // P, P, m).transpose(1, 0, 2))

    wq3 = np.stack([pack_mat(f(inputs["Wq"][l]).transpose(1, 0, 2).reshape(C, H * HS))
                    for l in range(L)])
    wk3 = np.stack([pack_mat(f(inputs["Wk"][l]).transpose(1, 0, 2).reshape(C, H * HS))
                    for l in range(L)])
    wv3 = np.stack([pack_mat(f(inputs["Wv"][l]).transpose(1, 0, 2).reshape(C, H * HS))
                    for l in range(L)])
    wp3 = np.stack([pack_mat(f(inputs["Wproj"][l])) for l in range(L)])
    w13 = np.stack([pack_mat(f(inputs["W1"][l])) for l in range(L)])
    w23 = np.stack([pack_mat(f(inputs["W2"][l])) for l in range(L)])

    def pack_vec(v):  # [256] -> [128, 2]
        return np.ascontiguousarray(f(v).reshape(2, P).T)

    vecs = np.stack([np.stack([pack_vec(inputs[k][l]) for k in
                               ("bproj", "b1", "b2")]).transpose(1, 0, 2)
                     for l in range(L)])
    vecs = np.ascontiguousarray(vecs)
    # vrow[l, row(g/b), ln_i, cc, :]: gamma/beta as contraction rows
    vrow = np.zeros((L, 2, 2, 2, P), np.float32)
    for l in range(L):
        for ln_i, (gk, bk) in enumerate((("ln1_g", "ln1_b"),
                                         ("ln2_g", "ln2_b"))):
            g = f(inputs[gk][l]).reshape(2, P)
            b = f(inputs[bk][l]).reshape(2, P)
            for cc in range(2):
                vrow[l, 0, ln_i, cc] = g[cc]
                vrow[l, 1, ln_i, cc] = b[cc]
    lnfrow = np.zeros((2, 2, P), np.float32)
    gf = f(inputs["lnf_g"]).reshape(2, P)
    bf = f(inputs["lnf_b"]).reshape(2, P)
    for cc in range(2):
        lnfrow[0, cc] = gf[cc]
        lnfrow[1, cc] = bf[cc]
    wc1 = pack_mat(f(inputs["Wc1"]) / T)        # fold mean-pool 1/T into Wc1
    bc1 = np.ascontiguousarray(f(inputs["bc1"]).reshape(CLS_H // P, P).T)
    wc2 = np.ascontiguousarray(f(inputs["Wc2"]).reshape(CLS_H // P, P, NOUT)
                               .transpose(1, 0, 2))
    bc2 = f(inputs["bc2"]).reshape(1, NOUT)
    tokf = f(inputs["tok_emb"])
    posf = f(inputs["pos_emb"])
    return dict(wq=wq3, wk=wk3, wv=wv3, wp=wp3, w1=w13, w2=w23, vecs=vecs,
                vrow=vrow, lnfrow=lnfrow, wc1=wc1, bc1=bc1, wc2=wc2, bc2=bc2,
                tok=tokf, pos=posf)


def _wrap_idx(ids):
    """int array [n] -> dma_gather wrapped layout [128, n//16] int16."""
    n = ids.shape[0]
    w = ids.reshape(n // 16, 16).T.astype(np.int16)     # [16, n//16]
    return np.ascontiguousarray(np.tile(w, (8, 1)))     # [128, n//16]


def _make_in_maps(inputs):
    shared = _prep_shared(inputs)
    idx = np.asarray(inputs["idx"]).astype(np.int64)
    in_maps = []
    for c in range(N_CORES):
        b, th = c // 2, c % 2
        t0 = th * TL
        idx_loc = idx[b, t0:t0 + TL]
        pos_loc = shared["pos"][t0:t0 + TL]  # [TL, C]
        posr_a = np.ascontiguousarray(
            pos_loc.reshape(TL // P, P, C).transpose(1, 0, 2))
        rem = (1 - th) * 2 * P + np.arange(2 * P, dtype=np.int64)
        m = dict(tok=shared["tok"], idxw=_wrap_idx(idx_loc), posr=posr_a,
                 remidx=_wrap_idx(rem),
                 wq=shared["wq"], wk=shared["wk"], wv=shared["wv"],
                 wp=shared["wp"], w1=shared["w1"], w2=shared["w2"],
                 vecs=shared["vecs"], vrow=shared["vrow"],
                 lnfrow=shared["lnfrow"], wc1=shared["wc1"],
                 bc1=shared["bc1"], wc2=shared["wc2"], bc2=shared["bc2"])
        in_maps.append(m)
    return in_maps


def kernel(**inputs) -> np.ndarray:
    if "nc" not in _CACHE:
        _CACHE["nc"] = _build_program()
    nc = _CACHE["nc"]
    in_maps = _make_in_maps(inputs)
    res = bass_utils.run_bass_kernel_spmd(nc, in_maps, core_ids=list(range(N_CORES)))
    out = np.zeros((B, NOUT), np.float32)
    for b in range(B):
        out[b] = res.results[2 * b]["probs"][0]
    return out


# revision 17
# speedup vs baseline: 1.1131x; 1.0210x over previous
"""Trainium2 Bass kernel for nn_EncoderWithClassifier (4-layer encoder + classifier).

Sharding: 8 cores, core c handles (batch b=c//2, sequence half th=c%2, 1024 tokens).
Canonical activation layout: x^T [C=256 (2 chunks of 128 partitions), T_local=1024].

Attention: scores transposed ([s_tile, t]) via row-packed K=32 matmuls (fp32r
single-pass PE), one exp per (st,g) over 2 heads. o^T accumulation uses a
33-column lhsT (v | ones) so each o matmul also emits the softmax row-sum in
PSUM partition rows 32/96 -- no separate row-sum matmuls; two heads pack per
accumulator via tile_position (0,0)/(0,64). The S->exp->o chain is
software-pipelined one iteration ahead so the in-order PE queue never blocks
on the Act engine's exp stream.

Layer pipeline: everything outside attention (proj, LN2, FFN, next-layer LN1 +
QKV) is per-token work, split into two 512-token half-chains and emitted as
"filler" steps interleaved into the next attention segment's instruction
stream -- PE/DVE chain work executes in the shadow of the Act-bound exp
stream. The 2-rank AllGather for the next layer's remote K/V is likewise
issued from filler, and remote-s score tiles are ordered last so the
collective latency hides under local-s compute.

LayerNorm: stats via packed matmuls; per-token affine folded into the
replication matmuls (lhsT = [g] and [g; b] rows) so the apply is 2 DVE ops per
chunk; x^2 runs on the idle GpSimd engine. A manual act-table load of the
ln+exp set at program start pins one table for the whole program (the
auto-inserted per-switch loads would otherwise cost ~23us).

PSUM budget (8 banks): S [128,1024] x2 bufs = 4, "acc" 2x[128,512] = 2,
"mm" 2x[128,512] = 2.
"""
import numpy as np

import concourse.bacc as bacc
import concourse.mybir as mybir
import concourse.tile as tile
from concourse import bass_utils, library_config
from concourse.masks import make_identity

V, C, TMAX, H, L = 32000, 256, 2048, 8, 4
HS, FFN = 32, 256
CLS_H, NOUT = 512, 10
B, T = 4, 2048
TL = 1024          # tokens per core
P = 128
EPS = 1e-5
SCALE = C ** (-0.5)
N_CORES = 8
dt = mybir.dt
F32 = dt.float32
Alu = mybir.AluOpType
Act = mybir.ActivationFunctionType
X_AXIS = mybir.AxisListType.X

LN_EXP_SET = 6     # act_info.json index of natural_log_exp_and_others

_CACHE = {}


def _build_program(sim=False):
    nc = bacc.Bacc("TRN2", target_bir_lowering=False, debug=False,
                   num_devices=1 if sim else N_CORES)

    # ---------------- dram I/O ----------------
    tok = nc.dram_tensor("tok", [V, C], F32, kind="ExternalInput")
    idxw = nc.dram_tensor("idxw", [P, TL // 16], dt.int16, kind="ExternalInput")
    posr = nc.dram_tensor("posr", [P, TL // P, C], F32, kind="ExternalInput")
    remidx = nc.dram_tensor("remidx", [P, (2 * P) // 16], dt.int16,
                            kind="ExternalInput")
    wq_d = nc.dram_tensor("wq", [P, L, 2, C], F32, kind="ExternalInput")
    wk_d = nc.dram_tensor("wk", [P, L, 2, C], F32, kind="ExternalInput")
    wv_d = nc.dram_tensor("wv", [P, L, 2, C], F32, kind="ExternalInput")
    wp_d = nc.dram_tensor("wp", [P, L, 2, C], F32, kind="ExternalInput")
    w1_d = nc.dram_tensor("w1", [P, L, 2, FFN], F32, kind="ExternalInput")
    w2_d = nc.dram_tensor("w2", [P, L, 2, C], F32, kind="ExternalInput")
    vecs_d = nc.dram_tensor("vecs", [P, L, 3, 2], F32, kind="ExternalInput")
    # vecs rows: 0 bproj, 1 b1, 2 b2
    vrow_d = nc.dram_tensor("vrow", [2, L, 2, 2, P], F32, kind="ExternalInput")
    # vrow dims: [row(g=0,b=1), l, ln_i, cc, P]  (g/b transposed to rows)
    lnfrow_d = nc.dram_tensor("lnfrow", [2, 2, P], F32, kind="ExternalInput")
    wc1_d = nc.dram_tensor("wc1", [P, 2, CLS_H], F32, kind="ExternalInput")
    bc1_d = nc.dram_tensor("bc1", [P, CLS_H // P], F32, kind="ExternalInput")
    wc2_d = nc.dram_tensor("wc2", [P, CLS_H // P, NOUT], F32, kind="ExternalInput")
    bc2_d = nc.dram_tensor("bc2", [1, NOUT], F32, kind="ExternalInput")
    out_d = nc.dram_tensor("probs", [1, NOUT], F32, kind="ExternalOutput")

    REPL = [[0, 1], [2, 3], [4, 5], [6, 7]]

    with tile.TileContext(nc) as tc:
        with (
            tc.tile_pool(name="const", bufs=1) as cp,
            tc.tile_pool(name="work", bufs=1) as wk,
            tc.tile_pool(name="exp", bufs=3) as ep,
            tc.tile_pool(name="small", bufs=1) as sp,
            tc.tile_pool(name="psS", bufs=2, space="PSUM") as psS,
            tc.tile_pool(name="psA", bufs=2, space="PSUM") as psA,
            tc.tile_pool(name="psM", bufs=2, space="PSUM") as psM,
            tc.tile_pool(name="dram", bufs=2, space="DRAM") as dp,
        ):
            nc.gpsimd.load_library(library_config.mlp)

            R32 = dt.float32r

            def mm(out, lhsT, rhs, **kw):
                """matmul with fp32r operand views: single-pass PE (4x fp32)."""
                nc.tensor.matmul(out, lhsT=lhsT.bitcast(R32),
                                 rhs=rhs.bitcast(R32), **kw)

            # pin the ln+exp act table once; Ln and Exp then never reload
            nc.scalar.add_instruction(mybir.InstLoadActFuncSet(
                name=nc.get_next_instruction_name(),
                ins=[], outs=[], act_func_set_id=LN_EXP_SET))

            # ---------------- constants / weights to SBUF ----------------
            ident = cp.tile([P, P], F32, tag="ident")
            make_identity(nc, ident[:])
            inv256 = cp.tile([P, 1], F32, tag="inv256")
            nc.vector.memset(inv256[:], 1.0 / C)
            sel4 = cp.tile([4, P], F32, tag="sel4")
            nc.gpsimd.memset(sel4[:], 0.0)
            for j in range(4):
                nc.gpsimd.memset(sel4[j:j + 1, 32 * j:32 * (j + 1)], 1.0)

            def load_const(name, dram_ap, shape, dtype=F32, eng=None):
                t = cp.tile(shape, dtype, tag=name, name=name)
                (eng or nc.sync).dma_start(t[:], dram_ap)
                return t

            # SP queue carries only the embed-critical loads; all weights go
            # as single coalesced DMAs on the Activation HWDGE queue so the
            # embed gather + pos load aren't stuck behind them.
            idx_sb = load_const("idx_sb", idxw[:], [P, TL // 16], dt.int16)
            remidx_sb = load_const("remidx_sb", remidx[:], [P, (2 * P) // 16],
                                   dt.int16)
            A = nc.scalar
            vrow_all = load_const("vrow_all", vrow_d[:], [2, L, 2, 2, P], eng=A)
            wq_all = load_const("wq_all", wq_d[:], [P, L, 2, C], eng=A)
            wk_all = load_const("wk_all", wk_d[:], [P, L, 2, C], eng=A)
            wv_all = load_const("wv_all", wv_d[:], [P, L, 2, C], eng=A)
            wp_all = load_const("wp_all", wp_d[:], [P, L, 2, C], eng=A)
            w1_all = load_const("w1_all", w1_d[:], [P, L, 2, FFN], eng=A)
            w2_all = load_const("w2_all", w2_d[:], [P, L, 2, C], eng=A)
            vecs_all = load_const("vecs_all", vecs_d[:], [P, L, 3, 2], eng=A)
            lnfrow = load_const("lnfrow", lnfrow_d[:], [2, 2, P], eng=A)
            wc1 = load_const("wc1", wc1_d[:], [P, 2, CLS_H], eng=A)
            bc1 = load_const("bc1", bc1_d[:], [P, CLS_H // P], eng=A)
            wc2 = load_const("wc2", wc2_d[:], [P, CLS_H // P, NOUT], eng=A)
            bc2 = load_const("bc2", bc2_d[:], [1, NOUT], eng=A)
            wq = [wq_all[:, l] for l in range(L)]
            wkt = [wk_all[:, l] for l in range(L)]
            wv = [wv_all[:, l] for l in range(L)]
            wp = [wp_all[:, l] for l in range(L)]
            w1 = [w1_all[:, l] for l in range(L)]
            w2 = [w2_all[:, l] for l in range(L)]
            vrow = [vrow_all[:, l] for l in range(L)]

            # vecs rows: 0 bproj, 1 b1, 2 b2
            def vap(l, row, cc):
                return vecs_all[:, l, row, cc:cc + 1]

            # persistent activations
            xT = [wk.tile([P, TL], F32, tag=f"xT{cc}", name=f"xT{cc}")
                  for cc in range(2)]

            # ---------------- embedding ----------------
            with tc.tile_pool(name="embed", bufs=1) as ebp:
                xg = ebp.tile([P, TL // P, C], F32, tag="xg")
                nc.gpsimd.dma_gather(xg[:], tok[:], idx_sb[:], TL, TL, C)
                pos_sb = ebp.tile([P, TL // P, C], F32, tag="pos_sb")
                nc.sync.dma_start(pos_sb[:], posr[:])
                nc.vector.tensor_add(xg[:], xg[:], pos_sb[:])
                for tt in range(TL // P):
                    for cc in range(2):
                        tp = psM.tile([P, P], F32, tag="mm", name="tp")
                        nc.tensor.transpose(tp[:], xg[:, tt, cc * P:(cc + 1) * P],
                                            ident[:])
                        nc.vector.tensor_copy(xT[cc][:, tt * P:(tt + 1) * P], tp[:])

            # v with trailing ones column: o matmul emits row-sums for free
            v_sb = [wk.tile([P, H, HS + 1], F32, tag=f"v{st}", name=f"v{st}")
                    for st in range(16)]
            for st in range(16):
                nc.vector.memset(v_sb[st][:, :, HS:HS + 1], 1.0)
            # [mrs; -1] rows for the g*mrs - b replication matmul
            stM = sp.tile([2, TL], F32, tag="stM", name="stM")
            nc.vector.memset(stM[1:2, :], -1.0)

            stA = sp.tile([1, TL], F32, tag="stA")   # mu
            stB = sp.tile([1, TL], F32, tag="stB")   # msq -> var -> rstd
            stT = sp.tile([1, TL], F32, tag="stT")   # musq -> ln(var)
            xsq = [sp.tile([P, TL], F32, tag=f"lnsq{cc}", name=f"lnsq{cc}")
                   for cc in range(2)]

            hT = [wk.tile([P, TL], F32, tag=f"hT{cc}", name=f"hT{cc}")
                  for cc in range(2)]
            h2T = [wk.tile([P, TL], F32, tag=f"h2T{cc}", name=f"h2T{cc}")
                   for cc in range(2)]
            fT = [wk.tile([P, TL], F32, tag=f"fT{ff}", name=f"fT{ff}")
                  for ff in range(2)]
            oT = [wk.tile([P, TL], F32, tag=f"oT{cc}", name=f"oT{cc}")
                  for cc in range(2)]
            qT = [wk.tile([P, TL], F32, tag=f"qT{mt}", name=f"qT{mt}")
                  for mt in range(2)]
            kT = [wk.tile([P, T], F32, tag=f"kT{mt}", name=f"kT{mt}")
                  for mt in range(2)]
            hR = wk.tile([P, 2, TL], F32, tag="hR", name="hR")

            # -------- per-half op-list builders (each op = one emission) -----
            def ln_half_ops(src, grow, gbrow, out, half):
                """LayerNorm of 512-token half: list of emission closures."""
                sl = slice(half * 512, (half + 1) * 512)
                ops = []
                for cc in range(2):
                    ops.append(lambda cc=cc: nc.gpsimd.tensor_mul(
                        xsq[cc][:, sl], src[cc][:, sl], src[cc][:, sl]))

                def stats(dst, inp):
                    ps = psM.tile([1, 512], F32, tag="mm", name="stat")
                    for kc in range(2):
                        mm(ps[:], lhsT=inv256[:], rhs=inp[kc][:, sl],
                           start=(kc == 0), stop=(kc == 1))
                    nc.vector.tensor_copy(dst[:, sl], ps[:])
                ops.append(lambda: stats(stA, src))
                ops.append(lambda: stats(stB, xsq))

                def rows1():
                    nc.vector.tensor_mul(stT[:, sl], stA[:, sl], stA[:, sl])
                    nc.vector.scalar_tensor_tensor(stB[:, sl], stB[:, sl], EPS,
                                                   stT[:, sl], Alu.add,
                                                   Alu.subtract)
                ops.append(rows1)

                def rows2():
                    nc.scalar.activation(stT[:, sl], stB[:, sl], Act.Ln)
                    nc.scalar.activation(stB[:, sl], stT[:, sl], Act.Exp,
                                         scale=-0.5)
                    nc.vector.tensor_mul(stM[0:1, sl], stA[:, sl], stB[:, sl])
                ops.append(rows2)

                def apply(cc):
                    rep1 = psM.tile([P, 512], F32, tag="mm", name="rep1")
                    mm(rep1[:], lhsT=grow(cc), rhs=stB[:, sl],
                       start=True, stop=True)
                    rep2 = psM.tile([P, 512], F32, tag="mm", name="rep2")
                    mm(rep2[:], lhsT=gbrow(cc), rhs=stM[:, sl],
                       start=True, stop=True)
                    nc.vector.tensor_mul(out[cc][:, sl], src[cc][:, sl], rep1[:])
                    nc.vector.tensor_sub(out[cc][:, sl], out[cc][:, sl], rep2[:])
                for cc in range(2):
                    ops.append(lambda cc=cc: apply(cc))
                return ops

            def proj_half_ops(l, half):
                sl = slice(half * 512, (half + 1) * 512)
                ops = []

                def pj(cc):
                    dpj = psM.tile([P, 512], F32, tag="mm", name="dpj")
                    for kc in range(2):
                        mm(dpj[:], lhsT=wp[l][:, kc, cc * P:(cc + 1) * P],
                           rhs=oT[kc][:, sl], start=(kc == 0), stop=(kc == 1))
                    nc.vector.scalar_tensor_tensor(xT[cc][:, sl], dpj[:],
                                                   vap(l, 0, cc), xT[cc][:, sl],
                                                   Alu.add, Alu.add)
                for cc in range(2):
                    ops.append(lambda cc=cc: pj(cc))
                return ops

            def ffn_half_ops(l, half):
                sl = slice(half * 512, (half + 1) * 512)
                ops = []

                def f1(ff):
                    fps = psM.tile([P, 512], F32, tag="mm", name="fps")
                    for kc in range(2):
                        mm(fps[:], lhsT=w1[l][:, kc, ff * P:(ff + 1) * P],
                           rhs=h2T[kc][:, sl], start=(kc == 0), stop=(kc == 1))
                    nc.vector.tensor_scalar(fT[ff][:, sl], fps[:], vap(l, 1, ff),
                                            0.0, Alu.add, Alu.max)

                def f2(cc):
                    d2 = psM.tile([P, 512], F32, tag="mm", name="d2")
                    for kc in range(2):
                        mm(d2[:], lhsT=w2[l][:, kc, cc * P:(cc + 1) * P],
                           rhs=fT[kc][:, sl], start=(kc == 0), stop=(kc == 1))
                    nc.vector.scalar_tensor_tensor(xT[cc][:, sl], d2[:],
                                                   vap(l, 2, cc), xT[cc][:, sl],
                                                   Alu.add, Alu.add)
                for ff in range(2):
                    ops.append(lambda ff=ff: f1(ff))
                for cc in range(2):
                    ops.append(lambda cc=cc: f2(cc))
                return ops

            def q_half_ops(l, half):
                sl = slice(half * 512, (half + 1) * 512)
                ops = []

                def q(mt):
                    qps = psM.tile([P, 512], F32, tag="mm", name="qps")
                    for kc in range(2):
                        mm(qps[:], lhsT=wq[l][:, kc, mt * P:(mt + 1) * P],
                           rhs=hT[kc][:, sl], start=(kc == 0), stop=(kc == 1))
                    nc.vector.tensor_copy(qT[mt][:, sl], qps[:])
                for mt in range(2):
                    ops.append(lambda mt=mt: q(mt))
                return ops

            def k_ops(l, nch):
                """kT columns nch*512.. from local (nch<2) or remote half."""
                ops = []

                def k(mt):
                    kps = psM.tile([P, 512], F32, tag="mm", name="kps")
                    for kc in range(2):
                        if nch < 2:
                            rhs = hT[kc][:, nch * 512:(nch + 1) * 512]
                        else:
                            rhs = hR[:, kc, (nch - 2) * 512:(nch - 1) * 512]
                        mm(kps[:], lhsT=wkt[l][:, kc, mt * P:(mt + 1) * P],
                           rhs=rhs, start=(kc == 0), stop=(kc == 1))
                    nc.vector.tensor_copy(kT[mt][:, nch * 512:(nch + 1) * 512],
                                          kps[:])
                for mt in range(2):
                    ops.append(lambda mt=mt: k(mt))
                return ops

            def v_ops(l, sts):
                ops = []

                def v(st):
                    vps = psM.tile([P, H, HS], F32, tag="mm", name="vps")
                    for kc in range(2):
                        if st < 8:
                            lhsT = hT[kc][:, st * P:(st + 1) * P]
                        else:
                            lhsT = hR[:, kc, (st - 8) * P:(st - 7) * P]
                        mm(vps[:], lhsT=lhsT, rhs=wv[l][:, kc, :],
                           start=(kc == 0), stop=(kc == 1))
                    nc.vector.tensor_copy(v_sb[st][:, :, 0:HS], vps[:])
                for st in sts:
                    ops.append(lambda st=st: v(st))
                return ops

            def setup_ops(l):
                """AllGather hT -> hR, then remote-half K/V."""
                ops = []
                b_in = dp.tile([2 * P, TL], F32, tag="b_in", name="b_in")
                b_out = dp.tile([4 * P, TL], F32, tag="b_out", name="b_out")

                def send():
                    for cc in range(2):
                        nc.sync.dma_start(b_in[cc * P:(cc + 1) * P, :], hT[cc][:])

                def coll():
                    if sim:
                        nc.sync.dma_start(b_out[:2 * P, :], b_in[:])
                        nc.sync.dma_start(b_out[2 * P:, :], b_in[:])
                    else:
                        nc.gpsimd.collective_compute(
                            "AllGather", Alu.bypass, replica_groups=REPL,
                            ins=[b_in[:].opt()], outs=[b_out[:].opt()])

                def gath():
                    nc.gpsimd.dma_gather(hR[:], b_out[:], remidx_sb[:],
                                         2 * P, 2 * P, TL)
                ops.extend([send, coll, gath])
                ops.extend(k_ops(l, 2))
                ops.extend(k_ops(l, 3))
                ops.extend(v_ops(l, range(8, 12)))
                ops.extend(v_ops(l, range(12, 16)))
                return ops

            def ln1_rows(l):
                return (lambda cc: vrow[l][0:1, 0, cc, :],
                        lambda cc: vrow[l][:, 0, cc, :])

            def ln2_rows(l):
                return (lambda cc: vrow[l][0:1, 1, cc, :],
                        lambda cc: vrow[l][:, 1, cc, :])

            def lnf_rows():
                return (lambda cc: lnfrow[0:1, cc, :],
                        lambda cc: lnfrow[:, cc, :])

            def chain_ops(l, half):
                """proj -> LN2 -> FFN -> next LN1 (or final LN) for one half."""
                ops = []
                ops += proj_half_ops(l, half)
                g2, gb2 = ln2_rows(l)
                ops += ln_half_ops(xT, g2, gb2, h2T, half)
                ops += ffn_half_ops(l, half)
                if l + 1 < L:
                    g1, gb1 = ln1_rows(l + 1)
                    ops += ln_half_ops(xT, g1, gb1, hT, half)
                else:
                    gf, gbf = lnf_rows()
                    ops += ln_half_ops(xT, gf, gbf, h2T, half)
                return ops

            # -------------------- attention --------------------
            def run_attn(l, tcn, filler):
                tsl = slice(tcn * 512, (tcn + 1) * 512)
                fit = iter(filler) if filler is not None else None

                def step(n=1):
                    if fit is not None:
                        for _ in range(n):
                            op = next(fit, None)
                            if op is not None:
                                op()

                acc = {}

                def emit_S(hp, st, g):
                    S = psS.tile([P, 2 * 512], F32, tag="S", name="S")
                    for jj in range(2):
                        j = 2 * g + jj
                        mm(S[:, jj * 512:(jj + 1) * 512],
                           lhsT=kT[hp][32 * j:32 * (j + 1),
                                       st * P:(st + 1) * P],
                           rhs=qT[hp][32 * j:32 * (j + 1), tsl],
                           start=True, stop=True,
                           tile_position=(32 * j, 0))
                    expT = ep.tile([P, 2 * 512], F32, tag="expT", name="expT")
                    nc.scalar.activation(expT[:], S[:], Act.Exp, scale=SCALE)
                    return expT

                def emit_O(hp, st, g, expT):
                    for jj in range(2):
                        j = 2 * g + jj
                        mm(acc[hp][g][64 * jj:64 * jj + HS + 1, :],
                           lhsT=v_sb[st][:, hp * 4 + j, :],
                           rhs=expT[:, jj * 512:(jj + 1) * 512],
                           start=(st == 0), stop=(st == 15),
                           tile_position=(0, 64 * jj))

                def norm(hp):
                    rs4 = sp.tile([4, 512], F32, tag="rs4")
                    for g in range(2):
                        for jj in range(2):
                            nc.vector.tensor_copy(
                                rs4[2 * g + jj:2 * g + jj + 1, :],
                                acc[hp][g][64 * jj + HS:64 * jj + HS + 1, :])
                    rec4 = sp.tile([4, 512], F32, tag="rec4")
                    nc.vector.reciprocal(rec4[:], rs4[:])
                    rrep = psM.tile([P, 512], F32, tag="mm", name="rrep")
                    mm(rrep[:], lhsT=sel4[:], rhs=rec4[:], start=True, stop=True)
                    for j in range(4):
                        nc.vector.tensor_mul(
                            oT[hp][32 * j:32 * (j + 1), tsl],
                            acc[hp][j // 2][64 * (j % 2):64 * (j % 2) + HS, :],
                            rrep[32 * j:32 * (j + 1), :])

                pend = None
                for hp in range(2):
                    acc[hp] = [psA.tile([P, 512], F32, tag="acc",
                                        name=f"acc{g}") for g in range(2)]
                    for st in range(16):
                        for g in range(2):
                            cur = emit_S(hp, st, g)
                            step()
                            if pend is not None:
                                emit_O(*pend)
                                if pend[0] == 0 and pend[1] == 15 and pend[2] == 1:
                                    norm(0)
                            step()
                            pend = (hp, st, g, cur)
                emit_O(*pend)
                norm(1)
                # drain leftover filler
                step(1000)

            # -------------------- prologue: layer 0 setup --------------------
            g1, gb1 = ln1_rows(0)
            for half in range(2):
                for op in (ln_half_ops(xT, g1, gb1, hT, half)
                           + q_half_ops(0, half) + k_ops(0, half)
                           + v_ops(0, range(half * 4, half * 4 + 4))):
                    op()
            for op in setup_ops(0):
                op()

            # -------------------- layers --------------------
            pending = None
            for l in range(L):
                run_attn(l, 0, pending)
                fillerB = chain_ops(l, 0)
                if l + 1 < L:
                    fillerB += q_half_ops(l + 1, 0)
                    fillerB += k_ops(l + 1, 0)
                    fillerB += v_ops(l + 1, range(0, 4))
                run_attn(l, 1, fillerB)
                pending = chain_ops(l, 1)
                if l + 1 < L:
                    pending += q_half_ops(l + 1, 1)
                    pending += k_ops(l + 1, 1)
                    pending += v_ops(l + 1, range(4, 8))
                    pending += setup_ops(l + 1)
            # final-LN half 1 chain (wrote h2T) + half-0 result sits in h2T too
            for op in pending:
                op()

            # ---------------- pool + classifier ----------------
            # final LN output lives in h2T (both halves)
            emb = sp.tile([P, 2], F32, tag="emb")
            for cc in range(2):
                nc.vector.reduce_sum(emb[:, cc:cc + 1], h2T[cc][:], axis=X_AXIS)
            be_in = dp.tile([P, 2], F32, tag="be_in", name="be_in")
            be_out = dp.tile([P, 2], F32, tag="be_out", name="be_out")
            nc.sync.dma_start(be_in[:], emb[:])
            if sim:
                nc.sync.dma_start(be_out[:], be_in[:])
            else:
                nc.gpsimd.collective_compute(
                    "AllReduce", Alu.add, replica_groups=REPL,
                    ins=[be_in[:].opt()], outs=[be_out[:].opt()])
            embr = sp.tile([P, 2], F32, tag="embr")
            nc.sync.dma_start(embr[:], be_out[:])

            h1ps = psM.tile([P, CLS_H // P], F32, tag="mm", name="h1ps")
            for mt in range(CLS_H // P):
                for kc in range(2):
                    nc.tensor.matmul(h1ps[:, mt:mt + 1],
                                     lhsT=wc1[:, kc, mt * P:(mt + 1) * P],
                                     rhs=embr[:, kc:kc + 1],
                                     start=(kc == 0), stop=(kc == 1))
            h1 = sp.tile([P, CLS_H // P], F32, tag="h1")
            nc.vector.tensor_add(h1[:], h1ps[:], bc1[:])
            nc.vector.tensor_scalar_max(h1[:], h1[:], 0.0)
            lps = psM.tile([1, NOUT], F32, tag="mm", name="lps")
            for j in range(CLS_H // P):
                nc.tensor.matmul(lps[:], lhsT=h1[:, j:j + 1], rhs=wc2[:, j, :],
                                 start=(j == 0), stop=(j == CLS_H // P - 1))
            lsb = sp.tile([1, NOUT], F32, tag="lsb")
            nc.vector.tensor_add(lsb[:], lps[:], bc2[:])
            mx = sp.tile([1, 1], F32, tag="mx")
            nc.vector.tensor_reduce(mx[:], lsb[:], axis=X_AXIS, op=Alu.max)
            nmx = sp.tile([1, 1], F32, tag="nmx")
            nc.vector.tensor_scalar_mul(nmx[:], mx[:], -1.0)
            esb = sp.tile([1, NOUT], F32, tag="esb")
            nc.scalar.activation(esb[:], lsb[:], Act.Exp, bias=nmx[:])
            ssum = sp.tile([1, 1], F32, tag="ssum")
            nc.vector.reduce_sum(ssum[:], esb[:], axis=X_AXIS)
            rsum = sp.tile([1, 1], F32, tag="rsum")
            nc.vector.reciprocal(rsum[:], ssum[:])
            probs = sp.tile([1, NOUT], F32, tag="probs")
            nc.vector.tensor_single_scalar(probs[:], esb[:], rsum[:], Alu.mult)
            nc.sync.dma_start(out_d[:], probs[:])

    nc.compile()
    return nc


def _prep_shared(inputs):
    """Host-side weight prepack (identical for all cores)."""
    f = lambda a: np.ascontiguousarray(np.asarray(a, dtype=np.float32))

    def pack_mat(w):  # [C_in, M] -> [128, C_in//128, M]
        ci, m = w.shape
        return np.ascontiguousarray(w.reshape(ci // P, P, m).transpose(1, 0, 2))

    def perlayer(g):  # [L, P, 2, M] -> [P, L, 2, M] contiguous
        return np.ascontiguousarray(np.stack(g).transpose(1, 0, 2, 3))

    wq3 = perlayer([pack_mat(f(inputs["Wq"][l]).transpose(1, 0, 2).reshape(C, H * HS))
                    for l in range(L)])
    wk3 = perlayer([pack_mat(f(inputs["Wk"][l]).transpose(1, 0, 2).reshape(C, H * HS))
                    for l in range(L)])
    wv3 = perlayer([pack_mat(f(inputs["Wv"][l]).transpose(1, 0, 2).reshape(C, H * HS))
                    for l in range(L)])
    wp3 = perlayer([pack_mat(f(inputs["Wproj"][l])) for l in range(L)])
    w13 = perlayer([pack_mat(f(inputs["W1"][l])) for l in range(L)])
    w23 = perlayer([pack_mat(f(inputs["W2"][l])) for l in range(L)])

    def pack_vec(v):  # [256] -> [128, 2]
        return np.ascontiguousarray(f(v).reshape(2, P).T)

    vecs = np.stack([np.stack([pack_vec(inputs[k][l]) for k in
                               ("bproj", "b1", "b2")]).transpose(1, 0, 2)
                     for l in range(L)])
    vecs = np.ascontiguousarray(vecs.transpose(1, 0, 2, 3))   # [P, L, 3, 2]
    # vrow[row(g/b), l, ln_i, cc, :]: gamma/beta as contraction rows
    vrow = np.zeros((2, L, 2, 2, P), np.float32)
    for l in range(L):
        for ln_i, (gk, bk) in enumerate((("ln1_g", "ln1_b"),
                                         ("ln2_g", "ln2_b"))):
            g = f(inputs[gk][l]).reshape(2, P)
            b = f(inputs[bk][l]).reshape(2, P)
            for cc in range(2):
                vrow[0, l, ln_i, cc] = g[cc]
                vrow[1, l, ln_i, cc] = b[cc]
    lnfrow = np.zeros((2, 2, P), np.float32)
    gf = f(inputs["lnf_g"]).reshape(2, P)
    bf = f(inputs["lnf_b"]).reshape(2, P)
    for cc in range(2):
        lnfrow[0, cc] = gf[cc]
        lnfrow[1, cc] = bf[cc]
    wc1 = pack_mat(f(inputs["Wc1"]) / T)        # fold mean-pool 1/T into Wc1
    bc1 = np.ascontiguousarray(f(inputs["bc1"]).reshape(CLS_H // P, P).T)
    wc2 = np.ascontiguousarray(f(inputs["Wc2"]).reshape(CLS_H // P, P, NOUT)
                               .transpose(1, 0, 2))
    bc2 = f(inputs["bc2"]).reshape(1, NOUT)
    tokf = f(inputs["tok_emb"])
    posf = f(inputs["pos_emb"])
    return dict(wq=wq3, wk=wk3, wv=wv3, wp=wp3, w1=w13, w2=w23, vecs=vecs,
                vrow=vrow, lnfrow=lnfrow, wc1=wc1, bc1=bc1, wc2=wc2, bc2=bc2,
                tok=tokf, pos=posf)


def _wrap_idx(ids):
    """int array [n] -> dma_gather wrapped layout [128, n//16] int16."""
    n = ids.shape[0]
    w = ids.reshape(n // 16, 16).T.astype(np.int16)     # [16, n//16]
    return np.ascontiguousarray(np.tile(w, (8, 1)))     # [128, n//16]


def _make_in_maps(inputs):
    shared = _prep_shared(inputs)
    idx = np.asarray(inputs["idx"]).astype(np.int64)
    in_maps = []
    for c in range(N_CORES):
        b, th = c // 2, c % 2
        t0 = th * TL
        idx_loc = idx[b, t0:t0 + TL]
        pos_loc = shared["pos"][t0:t0 + TL]  # [TL, C]
        posr_a = np.ascontiguousarray(
            pos_loc.reshape(TL // P, P, C).transpose(1, 0, 2))
        rem = (1 - th) * 2 * P + np.arange(2 * P, dtype=np.int64)
        m = dict(tok=shared["tok"], idxw=_wrap_idx(idx_loc), posr=posr_a,
                 remidx=_wrap_idx(rem),
                 wq=shared["wq"], wk=shared["wk"], wv=shared["wv"],
                 wp=shared["wp"], w1=shared["w1"], w2=shared["w2"],
                 vecs=shared["vecs"], vrow=shared["vrow"],
                 lnfrow=shared["lnfrow"], wc1=shared["wc1"],
                 bc1=shared["bc1"], wc2=shared["wc2"], bc2=shared["bc2"])
        in_maps.append(m)
    return in_maps


def kernel(**inputs) -> np.ndarray:
    if "nc" not in _CACHE:
        _CACHE["nc"] = _build_program()
    nc = _CACHE["nc"]
    in_maps = _make_in_maps(inputs)
    res = bass_utils.run_bass_kernel_spmd(nc, in_maps, core_ids=list(range(N_CORES)))
    out = np.zeros((B, NOUT), np.float32)
    for b in range(B):
        out[b] = res.results[2 * b]["probs"][0]
    return out


# revision 23
# speedup vs baseline: 1.1164x; 1.0030x over previous
"""Trainium2 Bass kernel for nn_EncoderWithClassifier (4-layer encoder + classifier).

Sharding: 8 cores, core c handles (batch b=c//2, sequence half th=c%2, 1024 tokens).
Canonical activation layout: x^T [C=256 (2 chunks of 128 partitions), T_local=1024].

Attention: scores transposed ([s_tile, t]) via row-packed K=32 matmuls (fp32r
single-pass PE), one exp per (st,g) over 2 heads. o^T accumulation uses a
33-column lhsT (v | ones) so each o matmul also emits the softmax row-sum in
PSUM partition rows 32/96 -- no separate row-sum matmuls; two heads pack per
accumulator via tile_position (0,0)/(0,64). The S->exp->o chain is
software-pipelined one iteration ahead so the in-order PE queue never blocks
on the Act engine's exp stream.

Layer pipeline: everything outside attention (proj, LN2, FFN, next-layer LN1 +
QKV) is per-token work, split into two 512-token half-chains and emitted as
"filler" steps interleaved into the next attention segment's instruction
stream -- PE/DVE chain work executes in the shadow of the Act-bound exp
stream. The 2-rank AllGather for the next layer's remote K/V is likewise
issued from filler, and remote-s score tiles are ordered last so the
collective latency hides under local-s compute.

LayerNorm: stats via packed matmuls; per-token affine folded into the
replication matmuls (lhsT = [g] and [g; b] rows) so the apply is 2 DVE ops per
chunk; x^2 runs on the idle GpSimd engine. A manual act-table load of the
ln+exp set at program start pins one table for the whole program (the
auto-inserted per-switch loads would otherwise cost ~23us).

PSUM budget (8 banks): S [128,1024] x2 bufs = 4, "acc" 2x[128,512] = 2,
"mm" 2x[128,512] = 2.
"""
import numpy as np

import concourse.bacc as bacc
import concourse.mybir as mybir
import concourse.tile as tile
from concourse import bass_utils, library_config
from concourse.masks import make_identity

V, C, TMAX, H, L = 32000, 256, 2048, 8, 4
HS, FFN = 32, 256
CLS_H, NOUT = 512, 10
B, T = 4, 2048
TL = 1024          # tokens per core
P = 128
EPS = 1e-5
SCALE = C ** (-0.5)
N_CORES = 8
dt = mybir.dt
F32 = dt.float32
Alu = mybir.AluOpType
Act = mybir.ActivationFunctionType
X_AXIS = mybir.AxisListType.X

LN_EXP_SET = 6     # act_info.json index of natural_log_exp_and_others

_CACHE = {}


def _build_program(sim=False):
    nc = bacc.Bacc("TRN2", target_bir_lowering=False, debug=False,
                   num_devices=1 if sim else N_CORES)

    # ---------------- dram I/O ----------------
    tok = nc.dram_tensor("tok", [V, C], F32, kind="ExternalInput")
    idxw = nc.dram_tensor("idxw", [P, TL // 16], dt.int16, kind="ExternalInput")
    posr = nc.dram_tensor("posr", [P, TL // P, C], F32, kind="ExternalInput")
    remidx = nc.dram_tensor("remidx", [P, (2 * P) // 16], dt.int16,
                            kind="ExternalInput")
    wq_d = nc.dram_tensor("wq", [P, L, 2, C], F32, kind="ExternalInput")
    wk_d = nc.dram_tensor("wk", [P, L, 2, C], F32, kind="ExternalInput")
    wv_d = nc.dram_tensor("wv", [P, L, 2, C], F32, kind="ExternalInput")
    wp_d = nc.dram_tensor("wp", [P, L, 2, C], F32, kind="ExternalInput")
    w1_d = nc.dram_tensor("w1", [P, L, 2, FFN], F32, kind="ExternalInput")
    w2_d = nc.dram_tensor("w2", [P, L, 2, C], F32, kind="ExternalInput")
    vecs_d = nc.dram_tensor("vecs", [P, L, 3, 2], F32, kind="ExternalInput")
    # vecs rows: 0 bproj, 1 b1, 2 b2
    vrow_d = nc.dram_tensor("vrow", [2, L, 2, 2, P], F32, kind="ExternalInput")
    # vrow dims: [row(g=0,b=1), l, ln_i, cc, P]  (g/b transposed to rows)
    lnfrow_d = nc.dram_tensor("lnfrow", [2, 2, P], F32, kind="ExternalInput")
    wc1_d = nc.dram_tensor("wc1", [P, 2, CLS_H], F32, kind="ExternalInput")
    bc1_d = nc.dram_tensor("bc1", [P, CLS_H // P], F32, kind="ExternalInput")
    wc2_d = nc.dram_tensor("wc2", [P, CLS_H // P, NOUT], F32, kind="ExternalInput")
    bc2_d = nc.dram_tensor("bc2", [1, NOUT], F32, kind="ExternalInput")
    out_d = nc.dram_tensor("probs", [1, NOUT], F32, kind="ExternalOutput")

    REPL = [[0, 1], [2, 3], [4, 5], [6, 7]]

    with tile.TileContext(nc) as tc:
        with (
            tc.tile_pool(name="const", bufs=1) as cp,
            tc.tile_pool(name="work", bufs=1) as wk,
            tc.tile_pool(name="exp", bufs=3) as ep,
            tc.tile_pool(name="small", bufs=1) as sp,
            tc.tile_pool(name="psS", bufs=2, space="PSUM") as psS,
            tc.tile_pool(name="psA", bufs=2, space="PSUM") as psA,
            tc.tile_pool(name="psM", bufs=2, space="PSUM") as psM,
            tc.tile_pool(name="dram", bufs=2, space="DRAM") as dp,
        ):
            nc.gpsimd.load_library(library_config.mlp)

            R32 = dt.float32r

            def mm(out, lhsT, rhs, **kw):
                """matmul with fp32r operand views: single-pass PE (4x fp32)."""
                nc.tensor.matmul(out, lhsT=lhsT.bitcast(R32),
                                 rhs=rhs.bitcast(R32), **kw)

            # pin the ln+exp act table once; Ln and Exp then never reload
            nc.scalar.add_instruction(mybir.InstLoadActFuncSet(
                name=nc.get_next_instruction_name(),
                ins=[], outs=[], act_func_set_id=LN_EXP_SET))

            # ---------------- constants / weights to SBUF ----------------
            ident = cp.tile([P, P], F32, tag="ident")
            make_identity(nc, ident[:])
            inv256 = cp.tile([P, 1], F32, tag="inv256")
            nc.vector.memset(inv256[:], 1.0 / C)
            sel4 = cp.tile([4, P], F32, tag="sel4")
            nc.gpsimd.memset(sel4[:], 0.0)
            for j in range(4):
                nc.gpsimd.memset(sel4[j:j + 1, 32 * j:32 * (j + 1)], 1.0)

            def load_const(name, dram_ap, shape, dtype=F32, eng=None):
                t = cp.tile(shape, dtype, tag=name, name=name)
                (eng or nc.sync).dma_start(t[:], dram_ap)
                return t

            # SP queue carries only the embed-critical loads; all weights go
            # as single coalesced DMAs on the Activation HWDGE queue so the
            # embed gather + pos load aren't stuck behind them.
            idx_sb = load_const("idx_sb", idxw[:], [P, TL // 16], dt.int16)
            remidx_sb = load_const("remidx_sb", remidx[:], [P, (2 * P) // 16],
                                   dt.int16)
            A = nc.scalar
            vrow_all = load_const("vrow_all", vrow_d[:], [2, L, 2, 2, P], eng=A)
            wq_all = load_const("wq_all", wq_d[:], [P, L, 2, C], eng=A)
            wk_all = load_const("wk_all", wk_d[:], [P, L, 2, C], eng=A)
            wv_all = load_const("wv_all", wv_d[:], [P, L, 2, C], eng=A)
            wp_all = load_const("wp_all", wp_d[:], [P, L, 2, C], eng=A)
            w1_all = load_const("w1_all", w1_d[:], [P, L, 2, FFN], eng=A)
            w2_all = load_const("w2_all", w2_d[:], [P, L, 2, C], eng=A)
            vecs_all = load_const("vecs_all", vecs_d[:], [P, L, 3, 2], eng=A)
            lnfrow = load_const("lnfrow", lnfrow_d[:], [2, 2, P], eng=A)
            wc1 = load_const("wc1", wc1_d[:], [P, 2, CLS_H], eng=A)
            bc1 = load_const("bc1", bc1_d[:], [P, CLS_H // P], eng=A)
            wc2 = load_const("wc2", wc2_d[:], [P, CLS_H // P, NOUT], eng=A)
            bc2 = load_const("bc2", bc2_d[:], [1, NOUT], eng=A)
            wq = [wq_all[:, l] for l in range(L)]
            wkt = [wk_all[:, l] for l in range(L)]
            wv = [wv_all[:, l] for l in range(L)]
            wp = [wp_all[:, l] for l in range(L)]
            w1 = [w1_all[:, l] for l in range(L)]
            w2 = [w2_all[:, l] for l in range(L)]
            vrow = [vrow_all[:, l] for l in range(L)]

            # vecs rows: 0 bproj, 1 b1, 2 b2
            def vap(l, row, cc):
                return vecs_all[:, l, row, cc:cc + 1]

            # persistent activations
            xT = [wk.tile([P, TL], F32, tag=f"xT{cc}", name=f"xT{cc}")
                  for cc in range(2)]

            # ---------------- embedding ----------------
            with tc.tile_pool(name="embed", bufs=1) as ebp:
                xg = ebp.tile([P, TL // P, C], F32, tag="xg")
                nc.gpsimd.dma_gather(xg[:], tok[:], idx_sb[:], TL, TL, C)
                pos_sb = ebp.tile([P, TL // P, C], F32, tag="pos_sb")
                nc.sync.dma_start(pos_sb[:], posr[:])
                nc.vector.tensor_add(xg[:], xg[:], pos_sb[:])
                for tt in range(TL // P):
                    for cc in range(2):
                        tp = psM.tile([P, P], F32, tag="mm", name="tp")
                        nc.tensor.transpose(tp[:], xg[:, tt, cc * P:(cc + 1) * P],
                                            ident[:])
                        nc.vector.tensor_copy(xT[cc][:, tt * P:(tt + 1) * P], tp[:])

            # v with trailing ones column: o matmul emits row-sums for free
            v_sb = [wk.tile([P, H, HS + 1], F32, tag=f"v{st}", name=f"v{st}")
                    for st in range(16)]
            for st in range(16):
                nc.vector.memset(v_sb[st][:, :, HS:HS + 1], 1.0)
            # [mrs; -1] rows for the g*mrs - b replication matmul
            stM = sp.tile([2, TL], F32, tag="stM", name="stM")
            nc.vector.memset(stM[1:2, :], -1.0)

            stA = sp.tile([1, TL], F32, tag="stA")   # mu
            stB = sp.tile([1, TL], F32, tag="stB")   # msq -> var -> rstd
            stT = sp.tile([1, TL], F32, tag="stT")   # musq -> ln(var)
            xsq = [sp.tile([P, TL], F32, tag=f"lnsq{cc}", name=f"lnsq{cc}")
                   for cc in range(2)]

            hT = [wk.tile([P, TL], F32, tag=f"hT{cc}", name=f"hT{cc}")
                  for cc in range(2)]
            h2T = [wk.tile([P, TL], F32, tag=f"h2T{cc}", name=f"h2T{cc}")
                   for cc in range(2)]
            fT = [wk.tile([P, TL], F32, tag=f"fT{ff}", name=f"fT{ff}")
                  for ff in range(2)]
            oT = [wk.tile([P, TL], F32, tag=f"oT{cc}", name=f"oT{cc}")
                  for cc in range(2)]
            qT = [wk.tile([P, TL], F32, tag=f"qT{mt}", name=f"qT{mt}")
                  for mt in range(2)]
            kT = [wk.tile([P, T], F32, tag=f"kT{mt}", name=f"kT{mt}")
                  for mt in range(2)]
            # remote LN1 halves: hR2[rh] holds remote s in [1024+512*rh, ...)
            hR2 = [wk.tile([P, 2, 512], F32, tag=f"hR{rh}", name=f"hR{rh}")
                   for rh in range(2)]

            # -------- per-half op-list builders (each op = one emission) -----
            def ln_half_ops(src, grow, gbrow, out, half):
                """LayerNorm of 512-token half: list of emission closures."""
                sl = slice(half * 512, (half + 1) * 512)
                ops = []
                for cc in range(2):
                    ops.append(lambda cc=cc: nc.gpsimd.tensor_mul(
                        xsq[cc][:, sl], src[cc][:, sl], src[cc][:, sl]))

                def stats(dst, inp):
                    ps = psM.tile([1, 512], F32, tag="mm", name="stat")
                    for kc in range(2):
                        mm(ps[:], lhsT=inv256[:], rhs=inp[kc][:, sl],
                           start=(kc == 0), stop=(kc == 1))
                    nc.vector.tensor_copy(dst[:, sl], ps[:])
                ops.append(lambda: stats(stA, src))
                ops.append(lambda: stats(stB, xsq))

                def rows1():
                    nc.vector.tensor_mul(stT[:, sl], stA[:, sl], stA[:, sl])
                    nc.vector.scalar_tensor_tensor(stB[:, sl], stB[:, sl], EPS,
                                                   stT[:, sl], Alu.add,
                                                   Alu.subtract)
                ops.append(rows1)

                def rows2():
                    nc.scalar.activation(stT[:, sl], stB[:, sl], Act.Ln)
                    nc.scalar.activation(stB[:, sl], stT[:, sl], Act.Exp,
                                         scale=-0.5)
                    nc.vector.tensor_mul(stM[0:1, sl], stA[:, sl], stB[:, sl])
                ops.append(rows2)

                def apply(cc):
                    rep1 = psM.tile([P, 512], F32, tag="mm", name="rep1")
                    mm(rep1[:], lhsT=grow(cc), rhs=stB[:, sl],
                       start=True, stop=True)
                    rep2 = psM.tile([P, 512], F32, tag="mm", name="rep2")
                    mm(rep2[:], lhsT=gbrow(cc), rhs=stM[:, sl],
                       start=True, stop=True)
                    nc.vector.tensor_mul(out[cc][:, sl], src[cc][:, sl], rep1[:])
                    nc.vector.tensor_sub(out[cc][:, sl], out[cc][:, sl], rep2[:])
                for cc in range(2):
                    ops.append(lambda cc=cc: apply(cc))
                return ops

            def proj_half_ops(l, half):
                sl = slice(half * 512, (half + 1) * 512)
                ops = []

                def pj(cc):
                    dpj = psM.tile([P, 512], F32, tag="mm", name="dpj")
                    for kc in range(2):
                        mm(dpj[:], lhsT=wp[l][:, kc, cc * P:(cc + 1) * P],
                           rhs=oT[kc][:, sl], start=(kc == 0), stop=(kc == 1))
                    nc.vector.scalar_tensor_tensor(xT[cc][:, sl], dpj[:],
                                                   vap(l, 0, cc), xT[cc][:, sl],
                                                   Alu.add, Alu.add)
                for cc in range(2):
                    ops.append(lambda cc=cc: pj(cc))
                return ops

            def ffn_half_ops(l, half):
                sl = slice(half * 512, (half + 1) * 512)
                ops = []

                def f1(ff):
                    fps = psM.tile([P, 512], F32, tag="mm", name="fps")
                    for kc in range(2):
                        mm(fps[:], lhsT=w1[l][:, kc, ff * P:(ff + 1) * P],
                           rhs=h2T[kc][:, sl], start=(kc == 0), stop=(kc == 1))
                    nc.vector.tensor_scalar(fT[ff][:, sl], fps[:], vap(l, 1, ff),
                                            0.0, Alu.add, Alu.max)

                def f2(cc):
                    d2 = psM.tile([P, 512], F32, tag="mm", name="d2")
                    for kc in range(2):
                        mm(d2[:], lhsT=w2[l][:, kc, cc * P:(cc + 1) * P],
                           rhs=fT[kc][:, sl], start=(kc == 0), stop=(kc == 1))
                    nc.vector.scalar_tensor_tensor(xT[cc][:, sl], d2[:],
                                                   vap(l, 2, cc), xT[cc][:, sl],
                                                   Alu.add, Alu.add)
                for ff in range(2):
                    ops.append(lambda ff=ff: f1(ff))
                for cc in range(2):
                    ops.append(lambda cc=cc: f2(cc))
                return ops

            def q_half_ops(l, half):
                sl = slice(half * 512, (half + 1) * 512)
                ops = []

                def q(mt):
                    qps = psM.tile([P, 512], F32, tag="mm", name="qps")
                    for kc in range(2):
                        mm(qps[:], lhsT=wq[l][:, kc, mt * P:(mt + 1) * P],
                           rhs=hT[kc][:, sl], start=(kc == 0), stop=(kc == 1))
                    nc.vector.tensor_copy(qT[mt][:, sl], qps[:])
                for mt in range(2):
                    ops.append(lambda mt=mt: q(mt))
                return ops

            def k_ops(l, nch):
                """kT columns nch*512.. from local (nch<2) or remote half."""
                ops = []

                def k(mt):
                    kps = psM.tile([P, 512], F32, tag="mm", name="kps")
                    for kc in range(2):
                        if nch < 2:
                            rhs = hT[kc][:, nch * 512:(nch + 1) * 512]
                        else:
                            rhs = hR2[nch - 2][:, kc, :]
                        mm(kps[:], lhsT=wkt[l][:, kc, mt * P:(mt + 1) * P],
                           rhs=rhs, start=(kc == 0), stop=(kc == 1))
                    nc.vector.tensor_copy(kT[mt][:, nch * 512:(nch + 1) * 512],
                                          kps[:])
                for mt in range(2):
                    ops.append(lambda mt=mt: k(mt))
                return ops

            def v_ops(l, sts):
                ops = []

                def v(st):
                    vps = psM.tile([P, H, HS], F32, tag="mm", name="vps")
                    for kc in range(2):
                        if st < 8:
                            lhsT = hT[kc][:, st * P:(st + 1) * P]
                        else:
                            lhsT = hR2[(st - 8) // 4][:, kc,
                                                      ((st - 8) % 4) * P:
                                                      ((st - 8) % 4 + 1) * P]
                        mm(vps[:], lhsT=lhsT, rhs=wv[l][:, kc, :],
                           start=(kc == 0), stop=(kc == 1))
                    nc.vector.tensor_copy(v_sb[st][:, :, 0:HS], vps[:])
                for st in sts:
                    ops.append(lambda st=st: v(st))
                return ops

            def setup_half_ops(l, half):
                """AllGather one 512-token half of hT -> hR2[half]."""
                b_in = dp.tile([2 * P, 512], F32, tag=f"b_in{half}",
                               name=f"b_in{half}")
                b_out = dp.tile([4 * P, 512], F32, tag=f"b_out{half}",
                                name=f"b_out{half}")
                sl = slice(half * 512, (half + 1) * 512)

                def send():
                    for cc in range(2):
                        nc.sync.dma_start(b_in[cc * P:(cc + 1) * P, :],
                                          hT[cc][:, sl])

                def coll():
                    if sim:
                        nc.sync.dma_start(b_out[:2 * P, :], b_in[:])
                        nc.sync.dma_start(b_out[2 * P:, :], b_in[:])
                    else:
                        nc.gpsimd.collective_compute(
                            "AllGather", Alu.bypass, replica_groups=REPL,
                            ins=[b_in[:].opt()], outs=[b_out[:].opt()])

                def gath():
                    nc.gpsimd.dma_gather(hR2[half][:], b_out[:], remidx_sb[:],
                                         2 * P, 2 * P, 512)
                return [send, coll, gath]

            def ln1_rows(l):
                return (lambda cc: vrow[l][0:1, 0, cc, :],
                        lambda cc: vrow[l][:, 0, cc, :])

            def ln2_rows(l):
                return (lambda cc: vrow[l][0:1, 1, cc, :],
                        lambda cc: vrow[l][:, 1, cc, :])

            def lnf_rows():
                return (lambda cc: lnfrow[0:1, cc, :],
                        lambda cc: lnfrow[:, cc, :])

            def chain_ops(l, half):
                """proj -> LN2 -> FFN -> next LN1 (or final LN) for one half."""
                ops = []
                ops += proj_half_ops(l, half)
                g2, gb2 = ln2_rows(l)
                ops += ln_half_ops(xT, g2, gb2, h2T, half)
                ops += ffn_half_ops(l, half)
                if l + 1 < L:
                    g1, gb1 = ln1_rows(l + 1)
                    ops += ln_half_ops(xT, g1, gb1, hT, half)
                else:
                    gf, gbf = lnf_rows()
                    ops += ln_half_ops(xT, gf, gbf, h2T, half)
                return ops

            # -------------------- attention --------------------
            def run_attn(l, tcn, filler):
                tsl = slice(tcn * 512, (tcn + 1) * 512)
                fit = iter(filler) if filler is not None else None

                def step(n=1):
                    if fit is not None:
                        for _ in range(n):
                            op = next(fit, None)
                            if op is not None:
                                op()

                acc = {}

                def emit_S(hp, st, g):
                    S = psS.tile([P, 2 * 512], F32, tag="S", name="S")
                    for jj in range(2):
                        j = 2 * g + jj
                        mm(S[:, jj * 512:(jj + 1) * 512],
                           lhsT=kT[hp][32 * j:32 * (j + 1),
                                       st * P:(st + 1) * P],
                           rhs=qT[hp][32 * j:32 * (j + 1), tsl],
                           start=True, stop=True,
                           tile_position=(32 * j, 0))
                    expT = ep.tile([P, 2 * 512], F32, tag="expT", name="expT")
                    nc.scalar.activation(expT[:], S[:], Act.Exp, scale=SCALE)
                    return expT

                def emit_O(hp, st, g, expT):
                    for jj in range(2):
                        j = 2 * g + jj
                        mm(acc[hp][g][64 * jj:64 * jj + HS + 1, :],
                           lhsT=v_sb[st][:, hp * 4 + j, :],
                           rhs=expT[:, jj * 512:(jj + 1) * 512],
                           start=(st == 0), stop=(st == 15),
                           tile_position=(0, 64 * jj))

                def norm(hp):
                    rs4 = sp.tile([4, 512], F32, tag="rs4")
                    for g in range(2):
                        for jj in range(2):
                            nc.vector.tensor_copy(
                                rs4[2 * g + jj:2 * g + jj + 1, :],
                                acc[hp][g][64 * jj + HS:64 * jj + HS + 1, :])
                    rec4 = sp.tile([4, 512], F32, tag="rec4")
                    nc.vector.reciprocal(rec4[:], rs4[:])
                    rrep = psM.tile([P, 512], F32, tag="mm", name="rrep")
                    mm(rrep[:], lhsT=sel4[:], rhs=rec4[:], start=True, stop=True)
                    for j in range(4):
                        nc.vector.tensor_mul(
                            oT[hp][32 * j:32 * (j + 1), tsl],
                            acc[hp][j // 2][64 * (j % 2):64 * (j % 2) + HS, :],
                            rrep[32 * j:32 * (j + 1), :])

                # st order follows data readiness: local h0, remote h0
                # (early-split collective), local h1 (chain latency), remote h1
                ST_ORDER = [0, 1, 2, 3, 8, 9, 10, 11, 4, 5, 6, 7, 12, 13, 14, 15]
                pend = None
                for hp in range(2):
                    acc[hp] = [psA.tile([P, 512], F32, tag="acc",
                                        name=f"acc{g}") for g in range(2)]
                    for st in ST_ORDER:
                        for g in range(2):
                            cur = emit_S(hp, st, g)
                            step()
                            if pend is not None:
                                emit_O(*pend)
                                if pend[0] == 0 and pend[1] == 15 and pend[2] == 1:
                                    norm(0)
                            step()
                            pend = (hp, st, g, cur)
                emit_O(*pend)
                norm(1)
                # drain leftover filler
                step(1000)

            # -------------------- prologue: layer 0 setup --------------------
            g1, gb1 = ln1_rows(0)
            for half in range(2):
                for op in (ln_half_ops(xT, g1, gb1, hT, half)
                           + setup_half_ops(0, half)
                           + q_half_ops(0, half) + k_ops(0, half)
                           + v_ops(0, range(half * 4, half * 4 + 4))):
                    op()
            for op in (k_ops(0, 2) + v_ops(0, range(8, 12))
                       + k_ops(0, 3) + v_ops(0, range(12, 16))):
                op()

            # -------------------- layers --------------------
            pending = None
            for l in range(L):
                run_attn(l, 0, pending)
                fillerB = chain_ops(l, 0)
                if l + 1 < L:
                    # LN1(l+1) h0 is ready mid-filler: launch its collective
                    # here so remote-h0 K/V are ready before attn(l+1) starts
                    fillerB += setup_half_ops(l + 1, 0)
                    fillerB += q_half_ops(l + 1, 0)
                    fillerB += k_ops(l + 1, 0)
                    fillerB += v_ops(l + 1, range(0, 4))
                run_attn(l, 1, fillerB)
                pending = []
                if l + 1 < L:
                    pending += k_ops(l + 1, 2)
                    pending += v_ops(l + 1, range(8, 12))
                pending += chain_ops(l, 1)
                if l + 1 < L:
                    pending += k_ops(l + 1, 1)
                    pending += v_ops(l + 1, range(4, 8))
                    pending += setup_half_ops(l + 1, 1)
                    pending += k_ops(l + 1, 3)
                    pending += v_ops(l + 1, range(12, 16))
                    pending += q_half_ops(l + 1, 1)
            # final-LN half 1 chain (wrote h2T) + half-0 result sits in h2T too
            for op in pending:
                op()

            # ---------------- pool + classifier ----------------
            # final LN output lives in h2T (both halves)
            emb = sp.tile([P, 2], F32, tag="emb")
            for cc in range(2):
                nc.vector.reduce_sum(emb[:, cc:cc + 1], h2T[cc][:], axis=X_AXIS)
            be_in = dp.tile([P, 2], F32, tag="be_in", name="be_in")
            be_out = dp.tile([P, 2], F32, tag="be_out", name="be_out")
            nc.sync.dma_start(be_in[:], emb[:])
            if sim:
                nc.sync.dma_start(be_out[:], be_in[:])
            else:
                nc.gpsimd.collective_compute(
                    "AllReduce", Alu.add, replica_groups=REPL,
                    ins=[be_in[:].opt()], outs=[be_out[:].opt()])
            embr = sp.tile([P, 2], F32, tag="embr")
            nc.sync.dma_start(embr[:], be_out[:])

            h1ps = psM.tile([P, CLS_H // P], F32, tag="mm", name="h1ps")
            for mt in range(CLS_H // P):
                for kc in range(2):
                    nc.tensor.matmul(h1ps[:, mt:mt + 1],
                                     lhsT=wc1[:, kc, mt * P:(mt + 1) * P],
                                     rhs=embr[:, kc:kc + 1],
                                     start=(kc == 0), stop=(kc == 1))
            h1 = sp.tile([P, CLS_H // P], F32, tag="h1")
            nc.vector.tensor_add(h1[:], h1ps[:], bc1[:])
            nc.vector.tensor_scalar_max(h1[:], h1[:], 0.0)
            lps = psM.tile([1, NOUT], F32, tag="mm", name="lps")
            for j in range(CLS_H // P):
                nc.tensor.matmul(lps[:], lhsT=h1[:, j:j + 1], rhs=wc2[:, j, :],
                                 start=(j == 0), stop=(j == CLS_H // P - 1))
            lsb = sp.tile([1, NOUT], F32, tag="lsb")
            nc.vector.tensor_add(lsb[:], lps[:], bc2[:])
            mx = sp.tile([1, 1], F32, tag="mx")
            nc.vector.tensor_reduce(mx[:], lsb[:], axis=X_AXIS, op=Alu.max)
            nmx = sp.tile([1, 1], F32, tag="nmx")
            nc.vector.tensor_scalar_mul(nmx[:], mx[:], -1.0)
            esb = sp.tile([1, NOUT], F32, tag="esb")
            nc.scalar.activation(esb[:], lsb[:], Act.Exp, bias=nmx[:])
            ssum = sp.tile([1, 1], F32, tag="ssum")
            nc.vector.reduce_sum(ssum[:], esb[:], axis=X_AXIS)
            rsum = sp.tile([1, 1], F32, tag="rsum")
            nc.vector.reciprocal(rsum[:], ssum[:])
            probs = sp.tile([1, NOUT], F32, tag="probs")
            nc.vector.tensor_single_scalar(probs[:], esb[:], rsum[:], Alu.mult)
            nc.sync.dma_start(out_d[:], probs[:])

    nc.compile()
    return nc


def _prep_shared(inputs):
    """Host-side weight prepack (identical for all cores)."""
    f = lambda a: np.ascontiguousarray(np.asarray(a, dtype=np.float32))

    def pack_mat(w):  # [C_in, M] -> [128, C_in//128, M]
        ci, m = w.shape
        return np.ascontiguousarray(w.reshape(ci // P, P, m).transpose(1, 0, 2))

    def perlayer(g):  # [L, P, 2, M] -> [P, L, 2, M] contiguous
        return np.ascontiguousarray(np.stack(g).transpose(1, 0, 2, 3))

    wq3 = perlayer([pack_mat(f(inputs["Wq"][l]).transpose(1, 0, 2).reshape(C, H * HS))
                    for l in range(L)])
    wk3 = perlayer([pack_mat(f(inputs["Wk"][l]).transpose(1, 0, 2).reshape(C, H * HS))
                    for l in range(L)])
    wv3 = perlayer([pack_mat(f(inputs["Wv"][l]).transpose(1, 0, 2).reshape(C, H * HS))
                    for l in range(L)])
    wp3 = perlayer([pack_mat(f(inputs["Wproj"][l])) for l in range(L)])
    w13 = perlayer([pack_mat(f(inputs["W1"][l])) for l in range(L)])
    w23 = perlayer([pack_mat(f(inputs["W2"][l])) for l in range(L)])

    def pack_vec(v):  # [256] -> [128, 2]
        return np.ascontiguousarray(f(v).reshape(2, P).T)

    vecs = np.stack([np.stack([pack_vec(inputs[k][l]) for k in
                               ("bproj", "b1", "b2")]).transpose(1, 0, 2)
                     for l in range(L)])
    vecs = np.ascontiguousarray(vecs.transpose(1, 0, 2, 3))   # [P, L, 3, 2]
    # vrow[row(g/b), l, ln_i, cc, :]: gamma/beta as contraction rows
    vrow = np.zeros((2, L, 2, 2, P), np.float32)
    for l in range(L):
        for ln_i, (gk, bk) in enumerate((("ln1_g", "ln1_b"),
                                         ("ln2_g", "ln2_b"))):
            g = f(inputs[gk][l]).reshape(2, P)
            b = f(inputs[bk][l]).reshape(2, P)
            for cc in range(2):
                vrow[0, l, ln_i, cc] = g[cc]
                vrow[1, l, ln_i, cc] = b[cc]
    lnfrow = np.zeros((2, 2, P), np.float32)
    gf = f(inputs["lnf_g"]).reshape(2, P)
    bf = f(inputs["lnf_b"]).reshape(2, P)
    for cc in range(2):
        lnfrow[0, cc] = gf[cc]
        lnfrow[1, cc] = bf[cc]
    wc1 = pack_mat(f(inputs["Wc1"]) / T)        # fold mean-pool 1/T into Wc1
    bc1 = np.ascontiguousarray(f(inputs["bc1"]).reshape(CLS_H // P, P).T)
    wc2 = np.ascontiguousarray(f(inputs["Wc2"]).reshape(CLS_H // P, P, NOUT)
                               .transpose(1, 0, 2))
    bc2 = f(inputs["bc2"]).reshape(1, NOUT)
    tokf = f(inputs["tok_emb"])
    posf = f(inputs["pos_emb"])
    return dict(wq=wq3, wk=wk3, wv=wv3, wp=wp3, w1=w13, w2=w23, vecs=vecs,
                vrow=vrow, lnfrow=lnfrow, wc1=wc1, bc1=bc1, wc2=wc2, bc2=bc2,
                tok=tokf, pos=posf)


def _wrap_idx(ids):
    """int array [n] -> dma_gather wrapped layout [128, n//16] int16."""
    n = ids.shape[0]
    w = ids.reshape(n // 16, 16).T.astype(np.int16)     # [16, n//16]
    return np.ascontiguousarray(np.tile(w, (8, 1)))     # [128, n//16]


def _make_in_maps(inputs):
    shared = _prep_shared(inputs)
    idx = np.asarray(inputs["idx"]).astype(np.int64)
    in_maps = []
    for c in range(N_CORES):
        b, th = c // 2, c % 2
        t0 = th * TL
        idx_loc = idx[b, t0:t0 + TL]
        pos_loc = shared["pos"][t0:t0 + TL]  # [TL, C]
        posr_a = np.ascontiguousarray(
            pos_loc.reshape(TL // P, P, C).transpose(1, 0, 2))
        rem = (1 - th) * 2 * P + np.arange(2 * P, dtype=np.int64)
        m = dict(tok=shared["tok"], idxw=_wrap_idx(idx_loc), posr=posr_a,
                 remidx=_wrap_idx(rem),
                 wq=shared["wq"], wk=shared["wk"], wv=shared["wv"],
                 wp=shared["wp"], w1=shared["w1"], w2=shared["w2"],
                 vecs=shared["vecs"], vrow=shared["vrow"],
                 lnfrow=shared["lnfrow"], wc1=shared["wc1"],
                 bc1=shared["bc1"], wc2=shared["wc2"], bc2=shared["bc2"])
        in_maps.append(m)
    return in_maps


def kernel(**inputs) -> np.ndarray:
    if "nc" not in _CACHE:
        _CACHE["nc"] = _build_program()
    nc = _CACHE["nc"]
    in_maps = _make_in_maps(inputs)
    res = bass_utils.run_bass_kernel_spmd(nc, in_maps, core_ids=list(range(N_CORES)))
    out = np.zeros((B, NOUT), np.float32)
    for b in range(B):
        out[b] = res.results[2 * b]["probs"][0]
    return out


# revision 28
# speedup vs baseline: 1.1308x; 1.0129x over previous
"""Trainium2 Bass kernel for nn_EncoderWithClassifier (4-layer encoder + classifier).

Sharding: 8 cores, core c handles (batch b=c//2, sequence half th=c%2, 1024 tokens).
Canonical activation layout: x^T [C=256 (2 chunks of 128 partitions), T_local=1024].

Attention: scores transposed ([s_tile, t]) via row-packed K=32 matmuls (fp32r
single-pass PE), one exp per (st,g) over 2 heads. o^T accumulation uses a
33-column lhsT (v | ones) so each o matmul also emits the softmax row-sum in
PSUM partition rows 32/96 -- no separate row-sum matmuls; two heads pack per
accumulator via tile_position (0,0)/(0,64). The S->exp->o chain is
software-pipelined one iteration ahead so the in-order PE queue never blocks
on the Act engine's exp stream.

Layer pipeline: everything outside attention (proj, LN2, FFN, next-layer LN1 +
QKV) is per-token work, split into two 512-token half-chains and emitted as
"filler" steps interleaved into the next attention segment's instruction
stream -- PE/DVE chain work executes in the shadow of the Act-bound exp
stream. The 2-rank AllGather for the next layer's remote K/V is likewise
issued from filler, and remote-s score tiles are ordered last so the
collective latency hides under local-s compute.

LayerNorm: stats via packed matmuls; per-token affine folded into the
replication matmuls (lhsT = [g] and [g; b] rows) so the apply is 2 DVE ops per
chunk; x^2 runs on the idle GpSimd engine. A manual act-table load of the
ln+exp set at program start pins one table for the whole program (the
auto-inserted per-switch loads would otherwise cost ~23us).

PSUM budget (8 banks): S [128,1024] x2 bufs = 4, "acc" 2x[128,512] = 2,
"mm" 2x[128,512] = 2.
"""
import numpy as np

import concourse.bacc as bacc
import concourse.mybir as mybir
import concourse.tile as tile
from concourse import bass_utils, library_config
from concourse.masks import make_identity

V, C, TMAX, H, L = 32000, 256, 2048, 8, 4
HS, FFN = 32, 256
CLS_H, NOUT = 512, 10
B, T = 4, 2048
TL = 1024          # tokens per core
P = 128
EPS = 1e-5
SCALE = C ** (-0.5)
N_CORES = 8
dt = mybir.dt
F32 = dt.float32
Alu = mybir.AluOpType
Act = mybir.ActivationFunctionType
X_AXIS = mybir.AxisListType.X

LN_EXP_SET = 6     # act_info.json index of natural_log_exp_and_others

_CACHE = {}


def _build_program(sim=False):
    nc = bacc.Bacc("TRN2", target_bir_lowering=False, debug=False,
                   num_devices=1 if sim else N_CORES)

    # ---------------- dram I/O ----------------
    tok = nc.dram_tensor("tok", [V, C], F32, kind="ExternalInput")
    idxw = nc.dram_tensor("idxw", [P, TL // 16], dt.int16, kind="ExternalInput")
    posr = nc.dram_tensor("posr", [P, TL // P, C], F32, kind="ExternalInput")
    wq_d = nc.dram_tensor("wq", [P, L, 2, C], F32, kind="ExternalInput")
    wk_d = nc.dram_tensor("wk", [P, L, 2, C], F32, kind="ExternalInput")
    wv_d = nc.dram_tensor("wv", [P, L, 2, C], F32, kind="ExternalInput")
    wp_d = nc.dram_tensor("wp", [P, L, 2, C], F32, kind="ExternalInput")
    w1_d = nc.dram_tensor("w1", [P, L, 2, FFN], F32, kind="ExternalInput")
    w2_d = nc.dram_tensor("w2", [P, L, 2, C], F32, kind="ExternalInput")
    vecs_d = nc.dram_tensor("vecs", [P, L, 3, 2], F32, kind="ExternalInput")
    # vecs rows: 0 bproj, 1 b1, 2 b2
    vrow_d = nc.dram_tensor("vrow", [2, L, 2, 2, P], F32, kind="ExternalInput")
    # vrow dims: [row(g=0,b=1), l, ln_i, cc, P]  (g/b transposed to rows)
    lnfrow_d = nc.dram_tensor("lnfrow", [2, 2, P], F32, kind="ExternalInput")
    wc1_d = nc.dram_tensor("wc1", [P, 2, CLS_H], F32, kind="ExternalInput")
    bc1_d = nc.dram_tensor("bc1", [P, CLS_H // P], F32, kind="ExternalInput")
    wc2_d = nc.dram_tensor("wc2", [P, CLS_H // P, NOUT], F32, kind="ExternalInput")
    bc2_d = nc.dram_tensor("bc2", [1, NOUT], F32, kind="ExternalInput")
    out_d = nc.dram_tensor("probs", [1, NOUT], F32, kind="ExternalOutput")

    REPL = [[0, 1], [2, 3], [4, 5], [6, 7]]

    with tile.TileContext(nc) as tc:
        with (
            tc.tile_pool(name="const", bufs=1) as cp,
            tc.tile_pool(name="work", bufs=1) as wk,
            tc.tile_pool(name="exp", bufs=3) as ep,
            tc.tile_pool(name="small", bufs=1) as sp,
            tc.tile_pool(name="psS", bufs=2, space="PSUM") as psS,
            tc.tile_pool(name="psA", bufs=2, space="PSUM") as psA,
            tc.tile_pool(name="psM", bufs=2, space="PSUM") as psM,
            tc.tile_pool(name="dram", bufs=2, space="DRAM") as dp,
        ):
            nc.gpsimd.load_library(library_config.mlp)

            R32 = dt.float32r

            def mm(out, lhsT, rhs, **kw):
                """matmul with fp32r operand views: single-pass PE (4x fp32)."""
                nc.tensor.matmul(out, lhsT=lhsT.bitcast(R32),
                                 rhs=rhs.bitcast(R32), **kw)

            # pin the ln+exp act table once; Ln and Exp then never reload
            nc.scalar.add_instruction(mybir.InstLoadActFuncSet(
                name=nc.get_next_instruction_name(),
                ins=[], outs=[], act_func_set_id=LN_EXP_SET))

            # ---------------- constants / weights to SBUF ----------------
            ident = cp.tile([P, P], F32, tag="ident")
            make_identity(nc, ident[:])
            inv256 = cp.tile([P, 1], F32, tag="inv256")
            nc.vector.memset(inv256[:], 1.0 / C)
            sel4 = cp.tile([4, P], F32, tag="sel4")
            nc.gpsimd.memset(sel4[:], 0.0)
            for j in range(4):
                nc.gpsimd.memset(sel4[j:j + 1, 32 * j:32 * (j + 1)], 1.0)

            def load_const(name, dram_ap, shape, dtype=F32, eng=None):
                t = cp.tile(shape, dtype, tag=name, name=name)
                (eng or nc.sync).dma_start(t[:], dram_ap)
                return t

            # SP queue carries only the embed-critical loads; all weights go
            # as single coalesced DMAs on the Activation HWDGE queue so the
            # embed gather + pos load aren't stuck behind them.
            idx_sb = load_const("idx_sb", idxw[:], [P, TL // 16], dt.int16)
            A = nc.scalar
            vrow_all = load_const("vrow_all", vrow_d[:], [2, L, 2, 2, P], eng=A)
            wq_all = load_const("wq_all", wq_d[:], [P, L, 2, C], eng=A)
            wk_all = load_const("wk_all", wk_d[:], [P, L, 2, C], eng=A)
            wv_all = load_const("wv_all", wv_d[:], [P, L, 2, C], eng=A)
            wp_all = load_const("wp_all", wp_d[:], [P, L, 2, C], eng=A)
            w1_all = load_const("w1_all", w1_d[:], [P, L, 2, FFN], eng=A)
            w2_all = load_const("w2_all", w2_d[:], [P, L, 2, C], eng=A)
            vecs_all = load_const("vecs_all", vecs_d[:], [P, L, 3, 2], eng=A)
            lnfrow = load_const("lnfrow", lnfrow_d[:], [2, 2, P], eng=A)
            wc1 = load_const("wc1", wc1_d[:], [P, 2, CLS_H], eng=A)
            bc1 = load_const("bc1", bc1_d[:], [P, CLS_H // P], eng=A)
            wc2 = load_const("wc2", wc2_d[:], [P, CLS_H // P, NOUT], eng=A)
            bc2 = load_const("bc2", bc2_d[:], [1, NOUT], eng=A)
            wq = [wq_all[:, l] for l in range(L)]
            wkt = [wk_all[:, l] for l in range(L)]
            wv = [wv_all[:, l] for l in range(L)]
            wp = [wp_all[:, l] for l in range(L)]
            w1 = [w1_all[:, l] for l in range(L)]
            w2 = [w2_all[:, l] for l in range(L)]
            vrow = [vrow_all[:, l] for l in range(L)]

            # vecs rows: 0 bproj, 1 b1, 2 b2
            def vap(l, row, cc):
                return vecs_all[:, l, row, cc:cc + 1]

            # persistent activations
            xT = [wk.tile([P, TL], F32, tag=f"xT{cc}", name=f"xT{cc}")
                  for cc in range(2)]

            # ---------------- embedding ----------------
            with tc.tile_pool(name="embed", bufs=1) as ebp:
                xg = ebp.tile([P, TL // P, C], F32, tag="xg")
                nc.gpsimd.dma_gather(xg[:], tok[:], idx_sb[:], TL, TL, C)
                pos_sb = ebp.tile([P, TL // P, C], F32, tag="pos_sb")
                nc.sync.dma_start(pos_sb[:], posr[:])
                nc.vector.tensor_add(xg[:], xg[:], pos_sb[:])
                for tt in range(TL // P):
                    for cc in range(2):
                        tp = psM.tile([P, P], F32, tag="mm", name="tp")
                        nc.tensor.transpose(tp[:], xg[:, tt, cc * P:(cc + 1) * P],
                                            ident[:])
                        nc.vector.tensor_copy(xT[cc][:, tt * P:(tt + 1) * P], tp[:])

            # v with trailing ones column: o matmul emits row-sums for free
            v_sb = [wk.tile([P, H, HS + 1], F32, tag=f"v{st}", name=f"v{st}")
                    for st in range(16)]
            for st in range(16):
                nc.vector.memset(v_sb[st][:, :, HS:HS + 1], 1.0)
            # [mrs; -1] rows for the g*mrs - b replication matmul
            stM = sp.tile([2, TL], F32, tag="stM", name="stM")
            nc.vector.memset(stM[1:2, :], -1.0)

            stA = sp.tile([1, TL], F32, tag="stA")   # mu
            stB = sp.tile([1, TL], F32, tag="stB")   # msq -> var -> rstd
            stT = sp.tile([1, TL], F32, tag="stT")   # musq -> ln(var)
            xsq = [sp.tile([P, TL], F32, tag=f"lnsq{cc}", name=f"lnsq{cc}")
                   for cc in range(2)]

            hT = [wk.tile([P, TL], F32, tag=f"hT{cc}", name=f"hT{cc}")
                  for cc in range(2)]
            h2T = [wk.tile([P, TL], F32, tag=f"h2T{cc}", name=f"h2T{cc}")
                   for cc in range(2)]
            fT = [wk.tile([P, TL], F32, tag=f"fT{ff}", name=f"fT{ff}")
                  for ff in range(2)]
            oT = [wk.tile([P, TL], F32, tag=f"oT{cc}", name=f"oT{cc}")
                  for cc in range(2)]
            qT = [wk.tile([P, TL], F32, tag=f"qT{mt}", name=f"qT{mt}")
                  for mt in range(2)]
            kT = [wk.tile([P, T], F32, tag=f"kT{mt}", name=f"kT{mt}")
                  for mt in range(2)]
            # allreduced LN1 halves: hs_sb[rh][cc] = h_local + h_remote for
            # 512-token half rh, channel chunk cc. Remote-half K/V are then
            # (W^T hsum) - (W^T h_local), with the subtract replacing the
            # PSUM->SBUF copy -- no index gather needed.
            hs_sb = [[wk.tile([P, 512], F32, tag=f"hs{rh}{cc}",
                              name=f"hs{rh}{cc}") for cc in range(2)]
                     for rh in range(2)]

            # -------- per-half op-list builders (each op = one emission) -----
            def ln_half_ops(src, grow, gbrow, out, half):
                """LayerNorm of 512-token half: list of emission closures."""
                sl = slice(half * 512, (half + 1) * 512)
                ops = []
                for cc in range(2):
                    ops.append(lambda cc=cc: nc.gpsimd.tensor_mul(
                        xsq[cc][:, sl], src[cc][:, sl], src[cc][:, sl]))

                def stats(dst, inp):
                    ps = psM.tile([1, 512], F32, tag="mm", name="stat")
                    for kc in range(2):
                        mm(ps[:], lhsT=inv256[:], rhs=inp[kc][:, sl],
                           start=(kc == 0), stop=(kc == 1))
                    nc.vector.tensor_copy(dst[:, sl], ps[:])
                ops.append(lambda: stats(stA, src))
                ops.append(lambda: stats(stB, xsq))

                def rows1():
                    nc.vector.tensor_mul(stT[:, sl], stA[:, sl], stA[:, sl])
                    nc.vector.scalar_tensor_tensor(stB[:, sl], stB[:, sl], EPS,
                                                   stT[:, sl], Alu.add,
                                                   Alu.subtract)
                ops.append(rows1)

                def rows2():
                    nc.scalar.activation(stT[:, sl], stB[:, sl], Act.Ln)
                    nc.scalar.activation(stB[:, sl], stT[:, sl], Act.Exp,
                                         scale=-0.5)
                    nc.vector.tensor_mul(stM[0:1, sl], stA[:, sl], stB[:, sl])
                ops.append(rows2)

                def apply(cc):
                    rep1 = psM.tile([P, 512], F32, tag="mm", name="rep1")
                    mm(rep1[:], lhsT=grow(cc), rhs=stB[:, sl],
                       start=True, stop=True)
                    rep2 = psM.tile([P, 512], F32, tag="mm", name="rep2")
                    mm(rep2[:], lhsT=gbrow(cc), rhs=stM[:, sl],
                       start=True, stop=True)
                    nc.vector.tensor_mul(out[cc][:, sl], src[cc][:, sl], rep1[:])
                    nc.vector.tensor_sub(out[cc][:, sl], out[cc][:, sl], rep2[:])
                for cc in range(2):
                    ops.append(lambda cc=cc: apply(cc))
                return ops

            def proj_half_ops(l, half):
                sl = slice(half * 512, (half + 1) * 512)
                ops = []

                def pj(cc):
                    dpj = psM.tile([P, 512], F32, tag="mm", name="dpj")
                    for kc in range(2):
                        mm(dpj[:], lhsT=wp[l][:, kc, cc * P:(cc + 1) * P],
                           rhs=oT[kc][:, sl], start=(kc == 0), stop=(kc == 1))
                    nc.vector.scalar_tensor_tensor(xT[cc][:, sl], dpj[:],
                                                   vap(l, 0, cc), xT[cc][:, sl],
                                                   Alu.add, Alu.add)
                for cc in range(2):
                    ops.append(lambda cc=cc: pj(cc))
                return ops

            def ffn_half_ops(l, half):
                sl = slice(half * 512, (half + 1) * 512)
                ops = []

                def f1(ff):
                    fps = psM.tile([P, 512], F32, tag="mm", name="fps")
                    for kc in range(2):
                        mm(fps[:], lhsT=w1[l][:, kc, ff * P:(ff + 1) * P],
                           rhs=h2T[kc][:, sl], start=(kc == 0), stop=(kc == 1))
                    nc.vector.tensor_scalar(fT[ff][:, sl], fps[:], vap(l, 1, ff),
                                            0.0, Alu.add, Alu.max)

                def f2(cc):
                    d2 = psM.tile([P, 512], F32, tag="mm", name="d2")
                    for kc in range(2):
                        mm(d2[:], lhsT=w2[l][:, kc, cc * P:(cc + 1) * P],
                           rhs=fT[kc][:, sl], start=(kc == 0), stop=(kc == 1))
                    nc.vector.scalar_tensor_tensor(xT[cc][:, sl], d2[:],
                                                   vap(l, 2, cc), xT[cc][:, sl],
                                                   Alu.add, Alu.add)
                for ff in range(2):
                    ops.append(lambda ff=ff: f1(ff))
                for cc in range(2):
                    ops.append(lambda cc=cc: f2(cc))
                return ops

            def q_half_ops(l, half):
                sl = slice(half * 512, (half + 1) * 512)
                ops = []

                def q(mt):
                    qps = psM.tile([P, 512], F32, tag="mm", name="qps")
                    for kc in range(2):
                        mm(qps[:], lhsT=wq[l][:, kc, mt * P:(mt + 1) * P],
                           rhs=hT[kc][:, sl], start=(kc == 0), stop=(kc == 1))
                    nc.vector.tensor_copy(qT[mt][:, sl], qps[:])
                for mt in range(2):
                    ops.append(lambda mt=mt: q(mt))
                return ops

            def k_ops(l, nch):
                """kT columns nch*512.. from local (nch<2) or remote half."""
                ops = []

                def k(mt):
                    kps = psM.tile([P, 512], F32, tag="mm", name="kps")
                    for kc in range(2):
                        if nch < 2:
                            rhs = hT[kc][:, nch * 512:(nch + 1) * 512]
                        else:
                            rhs = hs_sb[nch - 2][kc][:]
                        mm(kps[:], lhsT=wkt[l][:, kc, mt * P:(mt + 1) * P],
                           rhs=rhs, start=(kc == 0), stop=(kc == 1))
                    out_sl = slice(nch * 512, (nch + 1) * 512)
                    if nch < 2:
                        nc.vector.tensor_copy(kT[mt][:, out_sl], kps[:])
                    else:
                        loc_sl = slice((nch - 2) * 512, (nch - 1) * 512)
                        nc.vector.tensor_sub(kT[mt][:, out_sl], kps[:],
                                             kT[mt][:, loc_sl])
                for mt in range(2):
                    ops.append(lambda mt=mt: k(mt))
                return ops

            def v_ops(l, sts):
                ops = []

                def v(st):
                    vps = psM.tile([P, H, HS], F32, tag="mm", name="vps")
                    for kc in range(2):
                        if st < 8:
                            lhsT = hT[kc][:, st * P:(st + 1) * P]
                        else:
                            lhsT = hs_sb[(st - 8) // 4][kc][
                                :, ((st - 8) % 4) * P:((st - 8) % 4 + 1) * P]
                        mm(vps[:], lhsT=lhsT, rhs=wv[l][:, kc, :],
                           start=(kc == 0), stop=(kc == 1))
                    if st < 8:
                        nc.vector.tensor_copy(v_sb[st][:, :, 0:HS], vps[:])
                    else:
                        nc.vector.tensor_sub(v_sb[st][:, :, 0:HS], vps[:],
                                             v_sb[st - 8][:, :, 0:HS])
                for st in sts:
                    ops.append(lambda st=st: v(st))
                return ops

            def setup_half_ops(l, half):
                """AllReduce one 512-token half of hT -> hs_sb[half]."""
                b_in = dp.tile([2, P, 512], F32, tag=f"b_in{half}",
                               name=f"b_in{half}")
                b_out = dp.tile([2, P, 512], F32, tag=f"b_out{half}",
                                name=f"b_out{half}")
                sl = slice(half * 512, (half + 1) * 512)

                def send():
                    for cc in range(2):
                        nc.sync.dma_start(b_in[cc], hT[cc][:, sl])

                def coll():
                    if sim:
                        nc.sync.dma_start(b_out[:], b_in[:])
                    else:
                        nc.gpsimd.collective_compute(
                            "AllReduce", Alu.add, replica_groups=REPL,
                            ins=[b_in[:].opt()], outs=[b_out[:].opt()])

                def load():
                    for cc in range(2):
                        nc.sync.dma_start(hs_sb[half][cc][:], b_out[cc])
                return [send, coll, load]

            def ln1_rows(l):
                return (lambda cc: vrow[l][0:1, 0, cc, :],
                        lambda cc: vrow[l][:, 0, cc, :])

            def ln2_rows(l):
                return (lambda cc: vrow[l][0:1, 1, cc, :],
                        lambda cc: vrow[l][:, 1, cc, :])

            def lnf_rows():
                return (lambda cc: lnfrow[0:1, cc, :],
                        lambda cc: lnfrow[:, cc, :])

            def chain_ops(l, half):
                """proj -> LN2 -> FFN -> next LN1 (or final LN) for one half."""
                ops = []
                ops += proj_half_ops(l, half)
                g2, gb2 = ln2_rows(l)
                ops += ln_half_ops(xT, g2, gb2, h2T, half)
                ops += ffn_half_ops(l, half)
                if l + 1 < L:
                    g1, gb1 = ln1_rows(l + 1)
                    ops += ln_half_ops(xT, g1, gb1, hT, half)
                else:
                    gf, gbf = lnf_rows()
                    ops += ln_half_ops(xT, gf, gbf, h2T, half)
                return ops

            # -------------------- attention --------------------
            def run_attn(l, tcn, filler):
                tsl = slice(tcn * 512, (tcn + 1) * 512)
                fit = iter(filler) if filler is not None else None

                def step(n=1):
                    if fit is not None:
                        for _ in range(n):
                            op = next(fit, None)
                            if op is not None:
                                op()

                acc = {}

                def emit_S(hp, st, g):
                    S = psS.tile([P, 2 * 512], F32, tag="S", name="S")
                    for jj in range(2):
                        j = 2 * g + jj
                        mm(S[:, jj * 512:(jj + 1) * 512],
                           lhsT=kT[hp][32 * j:32 * (j + 1),
                                       st * P:(st + 1) * P],
                           rhs=qT[hp][32 * j:32 * (j + 1), tsl],
                           start=True, stop=True,
                           tile_position=(32 * j, 0))
                    expT = ep.tile([P, 2 * 512], F32, tag="expT", name="expT")
                    nc.scalar.activation(expT[:], S[:], Act.Exp, scale=SCALE)
                    return expT

                def emit_O(hp, st, g, expT):
                    for jj in range(2):
                        j = 2 * g + jj
                        mm(acc[hp][g][64 * jj:64 * jj + HS + 1, :],
                           lhsT=v_sb[st][:, hp * 4 + j, :],
                           rhs=expT[:, jj * 512:(jj + 1) * 512],
                           start=(st == 0), stop=(st == 15),
                           tile_position=(0, 64 * jj))

                def norm(hp):
                    rs4 = sp.tile([4, 512], F32, tag="rs4")
                    for g in range(2):
                        for jj in range(2):
                            nc.vector.tensor_copy(
                                rs4[2 * g + jj:2 * g + jj + 1, :],
                                acc[hp][g][64 * jj + HS:64 * jj + HS + 1, :])
                    rec4 = sp.tile([4, 512], F32, tag="rec4")
                    nc.vector.reciprocal(rec4[:], rs4[:])
                    rrep = psM.tile([P, 512], F32, tag="mm", name="rrep")
                    mm(rrep[:], lhsT=sel4[:], rhs=rec4[:], start=True, stop=True)
                    for j in range(4):
                        nc.vector.tensor_mul(
                            oT[hp][32 * j:32 * (j + 1), tsl],
                            acc[hp][j // 2][64 * (j % 2):64 * (j % 2) + HS, :],
                            rrep[32 * j:32 * (j + 1), :])

                # st order follows data readiness: local h0, remote h0
                # (early-split collective), local h1 (chain latency), remote h1
                ST_ORDER = [0, 1, 2, 3, 8, 9, 10, 11, 4, 5, 6, 7, 12, 13, 14, 15]
                pend = None
                for hp in range(2):
                    acc[hp] = [psA.tile([P, 512], F32, tag="acc",
                                        name=f"acc{g}") for g in range(2)]
                    for st in ST_ORDER:
                        for g in range(2):
                            cur = emit_S(hp, st, g)
                            step()
                            if pend is not None:
                                emit_O(*pend)
                                if pend[0] == 0 and pend[1] == 15 and pend[2] == 1:
                                    norm(0)
                            step()
                            pend = (hp, st, g, cur)
                emit_O(*pend)
                norm(1)
                # drain leftover filler
                step(1000)

            # -------------------- prologue: layer 0 setup --------------------
            g1, gb1 = ln1_rows(0)
            for half in range(2):
                for op in (ln_half_ops(xT, g1, gb1, hT, half)
                           + setup_half_ops(0, half)
                           + q_half_ops(0, half) + k_ops(0, half)
                           + v_ops(0, range(half * 4, half * 4 + 4))):
                    op()
            for op in (k_ops(0, 2) + v_ops(0, range(8, 12))
                       + k_ops(0, 3) + v_ops(0, range(12, 16))):
                op()

            # -------------------- layers --------------------
            pending = None
            for l in range(L):
                run_attn(l, 0, pending)
                fillerB = chain_ops(l, 0)
                if l + 1 < L:
                    # LN1(l+1) h0 is ready mid-filler: launch its collective
                    # here so remote-h0 K/V are ready before attn(l+1) starts
                    fillerB += setup_half_ops(l + 1, 0)
                    fillerB += q_half_ops(l + 1, 0)
                    fillerB += k_ops(l + 1, 0)
                    fillerB += v_ops(l + 1, range(0, 4))
                run_attn(l, 1, fillerB)
                pending = []
                if l + 1 < L:
                    pending += k_ops(l + 1, 2)
                    pending += v_ops(l + 1, range(8, 12))
                pending += chain_ops(l, 1)
                if l + 1 < L:
                    pending += k_ops(l + 1, 1)
                    pending += v_ops(l + 1, range(4, 8))
                    pending += setup_half_ops(l + 1, 1)
                    pending += k_ops(l + 1, 3)
                    pending += v_ops(l + 1, range(12, 16))
                    pending += q_half_ops(l + 1, 1)
            # final-LN half 1 chain (wrote h2T) + half-0 result sits in h2T too
            for op in pending:
                op()

            # ---------------- pool + classifier ----------------
            # final LN output lives in h2T (both halves)
            emb = sp.tile([P, 2], F32, tag="emb")
            for cc in range(2):
                nc.vector.reduce_sum(emb[:, cc:cc + 1], h2T[cc][:], axis=X_AXIS)
            be_in = dp.tile([P, 2], F32, tag="be_in", name="be_in")
            be_out = dp.tile([P, 2], F32, tag="be_out", name="be_out")
            nc.sync.dma_start(be_in[:], emb[:])
            if sim:
                nc.sync.dma_start(be_out[:], be_in[:])
            else:
                nc.gpsimd.collective_compute(
                    "AllReduce", Alu.add, replica_groups=REPL,
                    ins=[be_in[:].opt()], outs=[be_out[:].opt()])
            embr = sp.tile([P, 2], F32, tag="embr")
            nc.sync.dma_start(embr[:], be_out[:])

            h1ps = psM.tile([P, CLS_H // P], F32, tag="mm", name="h1ps")
            for mt in range(CLS_H // P):
                for kc in range(2):
                    nc.tensor.matmul(h1ps[:, mt:mt + 1],
                                     lhsT=wc1[:, kc, mt * P:(mt + 1) * P],
                                     rhs=embr[:, kc:kc + 1],
                                     start=(kc == 0), stop=(kc == 1))
            h1 = sp.tile([P, CLS_H // P], F32, tag="h1")
            nc.vector.tensor_add(h1[:], h1ps[:], bc1[:])
            nc.vector.tensor_scalar_max(h1[:], h1[:], 0.0)
            lps = psM.tile([1, NOUT], F32, tag="mm", name="lps")
            for j in range(CLS_H // P):
                nc.tensor.matmul(lps[:], lhsT=h1[:, j:j + 1], rhs=wc2[:, j, :],
                                 start=(j == 0), stop=(j == CLS_H // P - 1))
            lsb = sp.tile([1, NOUT], F32, tag="lsb")
            nc.vector.tensor_add(lsb[:], lps[:], bc2[:])
            mx = sp.tile([1, 1], F32, tag="mx")
            nc.vector.tensor_reduce(mx[:], lsb[:], axis=X_AXIS, op=Alu.max)
            nmx = sp.tile([1, 1], F32, tag="nmx")
            nc.vector.tensor_scalar_mul(nmx[:], mx[:], -1.0)
            esb = sp.tile([1, NOUT], F32, tag="esb")
            nc.scalar.activation(esb[:], lsb[:], Act.Exp, bias=nmx[:])
            ssum = sp.tile([1, 1], F32, tag="ssum")
            nc.vector.reduce_sum(ssum[:], esb[:], axis=X_AXIS)
            rsum = sp.tile([1, 1], F32, tag="rsum")
            nc.vector.reciprocal(rsum[:], ssum[:])
            probs = sp.tile([1, NOUT], F32, tag="probs")
            nc.vector.tensor_single_scalar(probs[:], esb[:], rsum[:], Alu.mult)
            nc.sync.dma_start(out_d[:], probs[:])

    nc.compile()
    return nc


def _prep_shared(inputs):
    """Host-side weight prepack (identical for all cores)."""
    f = lambda a: np.ascontiguousarray(np.asarray(a, dtype=np.float32))

    def pack_mat(w):  # [C_in, M] -> [128, C_in//128, M]
        ci, m = w.shape
        return np.ascontiguousarray(w.reshape(ci // P, P, m).transpose(1, 0, 2))

    def perlayer(g):  # [L, P, 2, M] -> [P, L, 2, M] contiguous
        return np.ascontiguousarray(np.stack(g).transpose(1, 0, 2, 3))

    wq3 = perlayer([pack_mat(f(inputs["Wq"][l]).transpose(1, 0, 2).reshape(C, H * HS))
                    for l in range(L)])
    wk3 = perlayer([pack_mat(f(inputs["Wk"][l]).transpose(1, 0, 2).reshape(C, H * HS))
                    for l in range(L)])
    wv3 = perlayer([pack_mat(f(inputs["Wv"][l]).transpose(1, 0, 2).reshape(C, H * HS))
                    for l in range(L)])
    wp3 = perlayer([pack_mat(f(inputs["Wproj"][l])) for l in range(L)])
    w13 = perlayer([pack_mat(f(inputs["W1"][l])) for l in range(L)])
    w23 = perlayer([pack_mat(f(inputs["W2"][l])) for l in range(L)])

    def pack_vec(v):  # [256] -> [128, 2]
        return np.ascontiguousarray(f(v).reshape(2, P).T)

    vecs = np.stack([np.stack([pack_vec(inputs[k][l]) for k in
                               ("bproj", "b1", "b2")]).transpose(1, 0, 2)
                     for l in range(L)])
    vecs = np.ascontiguousarray(vecs.transpose(1, 0, 2, 3))   # [P, L, 3, 2]
    # vrow[row(g/b), l, ln_i, cc, :]: gamma/beta as contraction rows
    vrow = np.zeros((2, L, 2, 2, P), np.float32)
    for l in range(L):
        for ln_i, (gk, bk) in enumerate((("ln1_g", "ln1_b"),
                                         ("ln2_g", "ln2_b"))):
            g = f(inputs[gk][l]).reshape(2, P)
            b = f(inputs[bk][l]).reshape(2, P)
            for cc in range(2):
                vrow[0, l, ln_i, cc] = g[cc]
                vrow[1, l, ln_i, cc] = b[cc]
    lnfrow = np.zeros((2, 2, P), np.float32)
    gf = f(inputs["lnf_g"]).reshape(2, P)
    bf = f(inputs["lnf_b"]).reshape(2, P)
    for cc in range(2):
        lnfrow[0, cc] = gf[cc]
        lnfrow[1, cc] = bf[cc]
    wc1 = pack_mat(f(inputs["Wc1"]) / T)        # fold mean-pool 1/T into Wc1
    bc1 = np.ascontiguousarray(f(inputs["bc1"]).reshape(CLS_H // P, P).T)
    wc2 = np.ascontiguousarray(f(inputs["Wc2"]).reshape(CLS_H // P, P, NOUT)
                               .transpose(1, 0, 2))
    bc2 = f(inputs["bc2"]).reshape(1, NOUT)
    tokf = f(inputs["tok_emb"])
    posf = f(inputs["pos_emb"])
    return dict(wq=wq3, wk=wk3, wv=wv3, wp=wp3, w1=w13, w2=w23, vecs=vecs,
                vrow=vrow, lnfrow=lnfrow, wc1=wc1, bc1=bc1, wc2=wc2, bc2=bc2,
                tok=tokf, pos=posf)


def _wrap_idx(ids):
    """int array [n] -> dma_gather wrapped layout [128, n//16] int16."""
    n = ids.shape[0]
    w = ids.reshape(n // 16, 16).T.astype(np.int16)     # [16, n//16]
    return np.ascontiguousarray(np.tile(w, (8, 1)))     # [128, n//16]


def _make_in_maps(inputs):
    shared = _prep_shared(inputs)
    idx = np.asarray(inputs["idx"]).astype(np.int64)
    in_maps = []
    for c in range(N_CORES):
        b, th = c // 2, c % 2
        t0 = th * TL
        idx_loc = idx[b, t0:t0 + TL]
        pos_loc = shared["pos"][t0:t0 + TL]  # [TL, C]
        posr_a = np.ascontiguousarray(
            pos_loc.reshape(TL // P, P, C).transpose(1, 0, 2))
        m = dict(tok=shared["tok"], idxw=_wrap_idx(idx_loc), posr=posr_a,
                 wq=shared["wq"], wk=shared["wk"], wv=shared["wv"],
                 wp=shared["wp"], w1=shared["w1"], w2=shared["w2"],
                 vecs=shared["vecs"], vrow=shared["vrow"],
                 lnfrow=shared["lnfrow"], wc1=shared["wc1"],
                 bc1=shared["bc1"], wc2=shared["wc2"], bc2=shared["bc2"])
        in_maps.append(m)
    return in_maps


def kernel(**inputs) -> np.ndarray:
    if "nc" not in _CACHE:
        _CACHE["nc"] = _build_program()
    nc = _CACHE["nc"]
    in_maps = _make_in_maps(inputs)
    res = bass_utils.run_bass_kernel_spmd(nc, in_maps, core_ids=list(range(N_CORES)))
    out = np.zeros((B, NOUT), np.float32)
    for b in range(B):
        out[b] = res.results[2 * b]["probs"][0]
    return out


# revision 35
# speedup vs baseline: 1.1979x; 1.0593x over previous
"""Trainium2 Bass kernel for nn_EncoderWithClassifier (4-layer encoder + classifier).

Sharding: 8 cores, core c handles (batch b=c//2, sequence half th=c%2, 1024 tokens).
Canonical activation layout: x^T [C=256 (2 chunks of 128 partitions), T_local=1024].

Attention: scores transposed ([s_tile, t]) via row-packed K=32 matmuls (fp32r
single-pass PE), one exp per (st,g) over 2 heads. o^T accumulation uses a
33-column lhsT (v | ones) so each o matmul also emits the softmax row-sum in
PSUM partition rows 32/96 -- no separate row-sum matmuls; two heads pack per
accumulator via tile_position (0,0)/(0,64). The S->exp->o chain is
software-pipelined one iteration ahead so the in-order PE queue never blocks
on the Act engine's exp stream.

Layer pipeline: everything outside attention (proj, LN2, FFN, next-layer LN1 +
QKV) is per-token work, split into two 512-token half-chains and emitted as
"filler" steps interleaved into the next attention segment's instruction
stream -- PE/DVE chain work executes in the shadow of the Act-bound exp
stream. The 2-rank AllGather for the next layer's remote K/V is likewise
issued from filler, and remote-s score tiles are ordered last so the
collective latency hides under local-s compute.

LayerNorm: stats via packed matmuls; per-token affine folded into the
replication matmuls (lhsT = [g] and [g; b] rows) so the apply is 2 DVE ops per
chunk; x^2 runs on the idle GpSimd engine. A manual act-table load of the
ln+exp set at program start pins one table for the whole program (the
auto-inserted per-switch loads would otherwise cost ~23us).

PSUM budget (8 banks): S [128,1024] x2 bufs = 4, "acc" 2x[128,512] = 2,
"mm" 2x[128,512] = 2.
"""
import numpy as np

import concourse.bacc as bacc
import concourse.mybir as mybir
import concourse.tile as tile
from concourse import bass_utils, library_config
from concourse.masks import make_identity

V, C, TMAX, H, L = 32000, 256, 2048, 8, 4
HS, FFN = 32, 256
CLS_H, NOUT = 512, 10
B, T = 4, 2048
TL = 1024          # tokens per core
P = 128
EPS = 1e-5
SCALE = C ** (-0.5)
N_CORES = 8
dt = mybir.dt
F32 = dt.float32
Alu = mybir.AluOpType
Act = mybir.ActivationFunctionType
X_AXIS = mybir.AxisListType.X

LN_EXP_SET = 6     # act_info.json index of natural_log_exp_and_others

_CACHE = {}


def _build_program(sim=False):
    nc = bacc.Bacc("TRN2", target_bir_lowering=False, debug=False,
                   num_devices=1 if sim else N_CORES)

    # ---------------- dram I/O ----------------
    tok = nc.dram_tensor("tok", [V, C], F32, kind="ExternalInput")
    idxw = nc.dram_tensor("idxw", [P, TL // 16], dt.int16, kind="ExternalInput")
    posr = nc.dram_tensor("posr", [P, TL // P, C], F32, kind="ExternalInput")
    wq_d = nc.dram_tensor("wq", [P, L, 2, C], F32, kind="ExternalInput")
    wk_d = nc.dram_tensor("wk", [P, L, 2, C], F32, kind="ExternalInput")
    wv_d = nc.dram_tensor("wv", [P, L, 2, C], F32, kind="ExternalInput")
    wp_d = nc.dram_tensor("wp", [P, L, 4, C], F32, kind="ExternalInput")
    w1_d = nc.dram_tensor("w1", [P, L, 2, FFN], F32, kind="ExternalInput")
    w2_d = nc.dram_tensor("w2", [P, L, 2, C], F32, kind="ExternalInput")
    vecs_d = nc.dram_tensor("vecs", [P, L, 3, 2], F32, kind="ExternalInput")
    # vecs rows: 0 bproj, 1 b1, 2 b2
    vrow_d = nc.dram_tensor("vrow", [2, L, 2, 2, P], F32, kind="ExternalInput")
    # vrow dims: [row(g=0,b=1), l, ln_i, cc, P]  (g/b transposed to rows)
    lnfrow_d = nc.dram_tensor("lnfrow", [2, 2, P], F32, kind="ExternalInput")
    wc1_d = nc.dram_tensor("wc1", [P, 2, CLS_H], F32, kind="ExternalInput")
    bc1_d = nc.dram_tensor("bc1", [P, CLS_H // P], F32, kind="ExternalInput")
    wc2_d = nc.dram_tensor("wc2", [P, CLS_H // P, NOUT], F32, kind="ExternalInput")
    bc2_d = nc.dram_tensor("bc2", [1, NOUT], F32, kind="ExternalInput")
    out_d = nc.dram_tensor("probs", [1, NOUT], F32, kind="ExternalOutput")

    REPL = [[0, 1], [2, 3], [4, 5], [6, 7]]

    with tile.TileContext(nc) as tc:
        with (
            tc.tile_pool(name="const", bufs=1) as cp,
            tc.tile_pool(name="work", bufs=1) as wk,
            tc.tile_pool(name="exp", bufs=3) as ep,
            tc.tile_pool(name="small", bufs=1) as sp,
            tc.tile_pool(name="psS", bufs=2, space="PSUM") as psS,
            tc.tile_pool(name="psA", bufs=2, space="PSUM") as psA,
            tc.tile_pool(name="psM", bufs=2, space="PSUM") as psM,
            tc.tile_pool(name="dram", bufs=2, space="DRAM") as dp,
        ):
            nc.gpsimd.load_library(library_config.mlp)

            R32 = dt.float32r

            def mm(out, lhsT, rhs, **kw):
                """matmul with fp32r operand views: single-pass PE (4x fp32)."""
                nc.tensor.matmul(out, lhsT=lhsT.bitcast(R32),
                                 rhs=rhs.bitcast(R32), **kw)

            # pin the ln+exp act table once; Ln and Exp then never reload
            nc.scalar.add_instruction(mybir.InstLoadActFuncSet(
                name=nc.get_next_instruction_name(),
                ins=[], outs=[], act_func_set_id=LN_EXP_SET))

            # ---------------- constants / weights to SBUF ----------------
            ident = cp.tile([P, P], F32, tag="ident")
            make_identity(nc, ident[:])
            inv256 = cp.tile([P, 1], F32, tag="inv256")
            nc.vector.memset(inv256[:], 1.0 / C)

            def load_const(name, dram_ap, shape, dtype=F32, eng=None):
                t = cp.tile(shape, dtype, tag=name, name=name)
                (eng or nc.sync).dma_start(t[:], dram_ap)
                return t

            # SP queue carries only the embed-critical loads; all weights go
            # as single coalesced DMAs on the Activation HWDGE queue so the
            # embed gather + pos load aren't stuck behind them.
            idx_sb = load_const("idx_sb", idxw[:], [P, TL // 16], dt.int16)
            A = nc.scalar
            vrow_all = load_const("vrow_all", vrow_d[:], [2, L, 2, 2, P], eng=A)
            wq_all = load_const("wq_all", wq_d[:], [P, L, 2, C], eng=A)
            wk_all = load_const("wk_all", wk_d[:], [P, L, 2, C], eng=A)
            wv_all = load_const("wv_all", wv_d[:], [P, L, 2, C], eng=A)
            wp_all = load_const("wp_all", wp_d[:], [P, L, 4, C], eng=A)
            w1_all = load_const("w1_all", w1_d[:], [P, L, 2, FFN], eng=A)
            w2_all = load_const("w2_all", w2_d[:], [P, L, 2, C], eng=A)
            vecs_all = load_const("vecs_all", vecs_d[:], [P, L, 3, 2], eng=A)
            lnfrow = load_const("lnfrow", lnfrow_d[:], [2, 2, P], eng=A)
            wc1 = load_const("wc1", wc1_d[:], [P, 2, CLS_H], eng=A)
            bc1 = load_const("bc1", bc1_d[:], [P, CLS_H // P], eng=A)
            wc2 = load_const("wc2", wc2_d[:], [P, CLS_H // P, NOUT], eng=A)
            bc2 = load_const("bc2", bc2_d[:], [1, NOUT], eng=A)
            wq = [wq_all[:, l] for l in range(L)]
            wkt = [wk_all[:, l] for l in range(L)]
            wv = [wv_all[:, l] for l in range(L)]
            wp = [wp_all[:, l] for l in range(L)]
            w1 = [w1_all[:, l] for l in range(L)]
            w2 = [w2_all[:, l] for l in range(L)]
            vrow = [vrow_all[:, l] for l in range(L)]

            # vecs rows: 0 bproj, 1 b1, 2 b2
            def vap(l, row, cc):
                return vecs_all[:, l, row, cc:cc + 1]

            # persistent activations
            xT = [wk.tile([P, TL], F32, tag=f"xT{cc}", name=f"xT{cc}")
                  for cc in range(2)]

            # ---------------- embedding ----------------
            with tc.tile_pool(name="embed", bufs=1) as ebp:
                xg = ebp.tile([P, TL // P, C], F32, tag="xg")
                nc.gpsimd.dma_gather(xg[:], tok[:], idx_sb[:], TL, TL, C)
                pos_sb = ebp.tile([P, TL // P, C], F32, tag="pos_sb")
                nc.sync.dma_start(pos_sb[:], posr[:])
                nc.vector.tensor_add(xg[:], xg[:], pos_sb[:])
                for tt in range(TL // P):
                    for cc in range(2):
                        tp = psM.tile([P, P], F32, tag="mm", name="tp")
                        nc.tensor.transpose(tp[:], xg[:, tt, cc * P:(cc + 1) * P],
                                            ident[:])
                        nc.vector.tensor_copy(xT[cc][:, tt * P:(tt + 1) * P], tp[:])

            # v with trailing ones column: o matmul emits row-sums for free
            v_sb = [wk.tile([P, H, HS + 1], F32, tag=f"v{st}", name=f"v{st}")
                    for st in range(16)]
            for st in range(16):
                nc.vector.memset(v_sb[st][:, :, HS:HS + 1], 1.0)
            # [mrs; -1] rows for the g*mrs - b replication matmul (per half)
            stM = sp.tile([2, 512], F32, tag="stM", name="stM")
            nc.vector.memset(stM[1:2, :], -1.0)

            stA = sp.tile([1, 512], F32, tag="stA")   # mu
            stB = sp.tile([1, 512], F32, tag="stB")   # msq -> var -> rstd
            stT = sp.tile([1, 512], F32, tag="stT")   # musq -> ln(var)
            xsq = [sp.tile([P, 512], F32, tag=f"lnsq{cc}", name=f"lnsq{cc}")
                   for cc in range(2)]

            # softmax reciprocal scratch: only rows 32/96 carry data; junk
            # rows stay at the memset value so the sel_acc contraction reads
            # finite zeros-by-selection
            rec_sb = [wk.tile([P, 512], F32, tag=f"rec{g}", name=f"rec{g}")
                      for g in range(2)]
            for g in range(2):
                nc.vector.memset(rec_sb[g][:], 1.0)
            for _ in range(2):
                accz = psA.tile([P, 512], F32, tag="acc", name="accz")
                nc.vector.memset(accz[:], 0.0)
            # sel_acc replicates acc row 32 -> rows 0-31, row 96 -> rows 64-95
            sel_acc = cp.tile([P, P], F32, tag="sel_acc")
            nc.gpsimd.memset(sel_acc[:], 0.0)
            nc.gpsimd.memset(sel_acc[HS:HS + 1, 0:HS], 1.0)
            nc.gpsimd.memset(sel_acc[64 + HS:64 + HS + 1, 64:64 + HS], 1.0)

            hT = [wk.tile([P, TL], F32, tag=f"hT{cc}", name=f"hT{cc}")
                  for cc in range(2)]
            h2T = [wk.tile([P, TL], F32, tag=f"h2T{cc}", name=f"h2T{cc}")
                   for cc in range(2)]
            fT = [wk.tile([P, 512], F32, tag=f"fT{ff}", name=f"fT{ff}")
                  for ff in range(2)]
            # oT in accumulator layout: tile pc = hp*2+g holds head hp*4+2g at
            # rows 0-31 and head hp*4+2g+1 at rows 64-95 (junk elsewhere);
            # proj weights are zero-padded on the junk rows host-side
            oT = [wk.tile([P, TL], F32, tag=f"oT{pc}", name=f"oT{pc}")
                  for pc in range(4)]
            qT = [wk.tile([P, TL], F32, tag=f"qT{mt}", name=f"qT{mt}")
                  for mt in range(2)]
            kT = [wk.tile([P, T], F32, tag=f"kT{mt}", name=f"kT{mt}")
                  for mt in range(2)]
            # allreduced LN1 halves: hs_sb[rh][cc] = h_local + h_remote for
            # 512-token half rh, channel chunk cc. Remote-half K/V are then
            # (W^T hsum) - (W^T h_local), with the subtract replacing the
            # PSUM->SBUF copy -- no index gather needed.
            hs_sb = [[wk.tile([P, 512], F32, tag=f"hs{rh}{cc}",
                              name=f"hs{rh}{cc}") for cc in range(2)]
                     for rh in range(2)]

            # -------- per-half op-list builders (each op = one emission) -----
            def ln_half_ops(src, grow, gbrow, out, half):
                """LayerNorm of 512-token half: list of emission closures."""
                sl = slice(half * 512, (half + 1) * 512)
                ops = []
                for cc in range(2):
                    ops.append(lambda cc=cc: nc.vector.tensor_mul(
                        xsq[cc][:], src[cc][:, sl], src[cc][:, sl]))

                def stats(dst, inp, islice):
                    ps = psM.tile([1, 512], F32, tag="mm", name="stat")
                    for kc in range(2):
                        mm(ps[:], lhsT=inv256[:], rhs=inp[kc][islice],
                           start=(kc == 0), stop=(kc == 1))
                    nc.vector.tensor_copy(dst[:], ps[:])
                ops.append(lambda: stats(stA, src, (slice(None), sl)))
                ops.append(lambda: stats(stB, xsq, slice(None)))

                def rows1():
                    nc.vector.tensor_mul(stT[:], stA[:], stA[:])
                    nc.vector.scalar_tensor_tensor(stB[:], stB[:], EPS,
                                                   stT[:], Alu.add,
                                                   Alu.subtract)
                ops.append(rows1)

                def rows2():
                    nc.scalar.activation(stT[:], stB[:], Act.Ln)
                    nc.scalar.activation(stB[:], stT[:], Act.Exp,
                                         scale=-0.5)
                    nc.vector.tensor_mul(stM[0:1, :], stA[:], stB[:])
                ops.append(rows2)

                def apply(cc):
                    rep1 = psM.tile([P, 512], F32, tag="mm", name="rep1")
                    mm(rep1[:], lhsT=grow(cc), rhs=stB[:],
                       start=True, stop=True)
                    rep2 = psM.tile([P, 512], F32, tag="mm", name="rep2")
                    mm(rep2[:], lhsT=gbrow(cc), rhs=stM[:],
                       start=True, stop=True)
                    nc.vector.tensor_mul(out[cc][:, sl], src[cc][:, sl], rep1[:])
                    nc.vector.tensor_sub(out[cc][:, sl], out[cc][:, sl], rep2[:])
                for cc in range(2):
                    ops.append(lambda cc=cc: apply(cc))
                return ops

            def proj_half_ops(l, half):
                sl = slice(half * 512, (half + 1) * 512)
                ops = []

                def pj(cc):
                    dpj = psM.tile([P, 512], F32, tag="mm", name="dpj")
                    for pc in range(4):
                        mm(dpj[:], lhsT=wp[l][:, pc, cc * P:(cc + 1) * P],
                           rhs=oT[pc][:, sl], start=(pc == 0), stop=(pc == 3))
                    nc.vector.scalar_tensor_tensor(xT[cc][:, sl], dpj[:],
                                                   vap(l, 0, cc), xT[cc][:, sl],
                                                   Alu.add, Alu.add)
                for cc in range(2):
                    ops.append(lambda cc=cc: pj(cc))
                return ops

            def ffn_half_ops(l, half):
                sl = slice(half * 512, (half + 1) * 512)
                ops = []

                def f1(ff):
                    fps = psM.tile([P, 512], F32, tag="mm", name="fps")
                    for kc in range(2):
                        mm(fps[:], lhsT=w1[l][:, kc, ff * P:(ff + 1) * P],
                           rhs=h2T[kc][:, sl], start=(kc == 0), stop=(kc == 1))
                    nc.vector.tensor_scalar(fT[ff][:], fps[:], vap(l, 1, ff),
                                            0.0, Alu.add, Alu.max)

                def f2(cc):
                    d2 = psM.tile([P, 512], F32, tag="mm", name="d2")
                    for kc in range(2):
                        mm(d2[:], lhsT=w2[l][:, kc, cc * P:(cc + 1) * P],
                           rhs=fT[kc][:], start=(kc == 0), stop=(kc == 1))
                    nc.vector.scalar_tensor_tensor(xT[cc][:, sl], d2[:],
                                                   vap(l, 2, cc), xT[cc][:, sl],
                                                   Alu.add, Alu.add)
                for ff in range(2):
                    ops.append(lambda ff=ff: f1(ff))
                for cc in range(2):
                    ops.append(lambda cc=cc: f2(cc))
                return ops

            def q_half_ops(l, half):
                sl = slice(half * 512, (half + 1) * 512)
                ops = []

                def q(mt):
                    qps = psM.tile([P, 512], F32, tag="mm", name="qps")
                    for kc in range(2):
                        mm(qps[:], lhsT=wq[l][:, kc, mt * P:(mt + 1) * P],
                           rhs=hT[kc][:, sl], start=(kc == 0), stop=(kc == 1))
                    nc.vector.tensor_copy(qT[mt][:, sl], qps[:])
                for mt in range(2):
                    ops.append(lambda mt=mt: q(mt))
                return ops

            def k_ops(l, nch):
                """kT columns nch*512.. from local (nch<2) or remote half."""
                ops = []

                def k(mt):
                    kps = psM.tile([P, 512], F32, tag="mm", name="kps")
                    for kc in range(2):
                        if nch < 2:
                            rhs = hT[kc][:, nch * 512:(nch + 1) * 512]
                        else:
                            rhs = hs_sb[nch - 2][kc][:]
                        mm(kps[:], lhsT=wkt[l][:, kc, mt * P:(mt + 1) * P],
                           rhs=rhs, start=(kc == 0), stop=(kc == 1))
                    out_sl = slice(nch * 512, (nch + 1) * 512)
                    if nch < 2:
                        nc.vector.tensor_copy(kT[mt][:, out_sl], kps[:])
                    else:
                        loc_sl = slice((nch - 2) * 512, (nch - 1) * 512)
                        nc.vector.tensor_sub(kT[mt][:, out_sl], kps[:],
                                             kT[mt][:, loc_sl])
                for mt in range(2):
                    ops.append(lambda mt=mt: k(mt))
                return ops

            def v_ops(l, sts):
                ops = []

                def v(st):
                    vps = psM.tile([P, H, HS], F32, tag="mm", name="vps")
                    for kc in range(2):
                        if st < 8:
                            lhsT = hT[kc][:, st * P:(st + 1) * P]
                        else:
                            lhsT = hs_sb[(st - 8) // 4][kc][
                                :, ((st - 8) % 4) * P:((st - 8) % 4 + 1) * P]
                        mm(vps[:], lhsT=lhsT, rhs=wv[l][:, kc, :],
                           start=(kc == 0), stop=(kc == 1))
                    if st < 8:
                        nc.vector.tensor_copy(v_sb[st][:, :, 0:HS], vps[:])
                    else:
                        nc.vector.tensor_sub(v_sb[st][:, :, 0:HS], vps[:],
                                             v_sb[st - 8][:, :, 0:HS])
                for st in sts:
                    ops.append(lambda st=st: v(st))
                return ops

            def setup_half_ops(l, half):
                """AllReduce one 512-token half of hT -> hs_sb[half]."""
                b_in = dp.tile([2, P, 512], F32, tag=f"b_in{half}",
                               name=f"b_in{half}")
                b_out = dp.tile([2, P, 512], F32, tag=f"b_out{half}",
                                name=f"b_out{half}")
                sl = slice(half * 512, (half + 1) * 512)

                def send():
                    for cc in range(2):
                        nc.sync.dma_start(b_in[cc], hT[cc][:, sl])

                def coll():
                    if sim:
                        nc.sync.dma_start(b_out[:], b_in[:])
                    else:
                        nc.gpsimd.collective_compute(
                            "AllReduce", Alu.add, replica_groups=REPL,
                            ins=[b_in[:].opt()], outs=[b_out[:].opt()])

                def load():
                    for cc in range(2):
                        nc.sync.dma_start(hs_sb[half][cc][:], b_out[cc])
                return [send, coll, load]

            def ln1_rows(l):
                return (lambda cc: vrow[l][0:1, 0, cc, :],
                        lambda cc: vrow[l][:, 0, cc, :])

            def ln2_rows(l):
                return (lambda cc: vrow[l][0:1, 1, cc, :],
                        lambda cc: vrow[l][:, 1, cc, :])

            def lnf_rows():
                return (lambda cc: lnfrow[0:1, cc, :],
                        lambda cc: lnfrow[:, cc, :])

            def chain_ops(l, half):
                """proj -> LN2 -> FFN -> next LN1 (or final LN) for one half."""
                ops = []
                ops += proj_half_ops(l, half)
                g2, gb2 = ln2_rows(l)
                ops += ln_half_ops(xT, g2, gb2, h2T, half)
                ops += ffn_half_ops(l, half)
                if l + 1 < L:
                    g1, gb1 = ln1_rows(l + 1)
                    ops += ln_half_ops(xT, g1, gb1, hT, half)
                else:
                    gf, gbf = lnf_rows()
                    ops += ln_half_ops(xT, gf, gbf, h2T, half)
                return ops

            # -------------------- attention --------------------
            def run_attn(l, tcn, filler):
                tsl = slice(tcn * 512, (tcn + 1) * 512)
                fit = iter(filler) if filler is not None else None

                def step(n=1):
                    if fit is not None:
                        for _ in range(n):
                            op = next(fit, None)
                            if op is not None:
                                op()

                acc = {}

                def emit_S(hp, st, g):
                    S = psS.tile([P, 2 * 512], F32, tag="S", name="S")
                    for jj in range(2):
                        j = 2 * g + jj
                        mm(S[:, jj * 512:(jj + 1) * 512],
                           lhsT=kT[hp][32 * j:32 * (j + 1),
                                       st * P:(st + 1) * P],
                           rhs=qT[hp][32 * j:32 * (j + 1), tsl],
                           start=True, stop=True,
                           tile_position=(32 * j, 0))
                    expT = ep.tile([P, 2 * 512], F32, tag="expT", name="expT")
                    nc.scalar.activation(expT[:], S[:], Act.Exp, scale=SCALE)
                    return expT

                def emit_O(hp, st, g, expT):
                    for jj in range(2):
                        j = 2 * g + jj
                        mm(acc[hp][g][64 * jj:64 * jj + HS + 1, :],
                           lhsT=v_sb[st][:, hp * 4 + j, :],
                           rhs=expT[:, jj * 512:(jj + 1) * 512],
                           start=(st == 0), stop=(st == 15),
                           tile_position=(0, 64 * jj))

                def norm(hp):
                    # row-sums sit at partition rows 32/96 of each acc; take
                    # reciprocals in place (partition-aligned), replicate to
                    # rows 0-31/64-95 via sel_acc, normalize the whole tile
                    for g in range(2):
                        nc.vector.reciprocal(rec_sb[g][HS:HS + 1, :],
                                             acc[hp][g][HS:HS + 1, :])
                        nc.vector.reciprocal(rec_sb[g][64 + HS:64 + HS + 1, :],
                                             acc[hp][g][64 + HS:64 + HS + 1, :])
                        rrep = psM.tile([P, 512], F32, tag="mm", name="rrep")
                        mm(rrep[:], lhsT=sel_acc[:], rhs=rec_sb[g][:],
                           start=True, stop=True)
                        nc.vector.tensor_mul(oT[hp * 2 + g][:, tsl],
                                             acc[hp][g][:], rrep[:])

                # st order follows data readiness: local h0, remote h0
                # (early-split collective), local h1 (chain latency), remote h1
                ST_ORDER = [0, 1, 2, 3, 8, 9, 10, 11, 4, 5, 6, 7, 12, 13, 14, 15]
                pend = None
                for hp in range(2):
                    acc[hp] = [psA.tile([P, 512], F32, tag="acc",
                                        name=f"acc{g}") for g in range(2)]
                    for st in ST_ORDER:
                        for g in range(2):
                            cur = emit_S(hp, st, g)
                            step()
                            if pend is not None:
                                emit_O(*pend)
                                if pend[0] == 0 and pend[1] == 15 and pend[2] == 1:
                                    norm(0)
                            step()
                            pend = (hp, st, g, cur)
                emit_O(*pend)
                norm(1)
                # drain leftover filler
                step(1000)

            # -------------------- prologue: layer 0 setup --------------------
            g1, gb1 = ln1_rows(0)
            for half in range(2):
                for op in (ln_half_ops(xT, g1, gb1, hT, half)
                           + setup_half_ops(0, half)
                           + q_half_ops(0, half) + k_ops(0, half)
                           + v_ops(0, range(half * 4, half * 4 + 4))):
                    op()
            for op in (k_ops(0, 2) + v_ops(0, range(8, 12))
                       + k_ops(0, 3) + v_ops(0, range(12, 16))):
                op()

            # -------------------- layers --------------------
            pending = None
            for l in range(L):
                run_attn(l, 0, pending)
                fillerB = chain_ops(l, 0)
                if l + 1 < L:
                    # LN1(l+1) h0 is ready mid-filler: launch its collective
                    # here so remote-h0 K/V are ready before attn(l+1) starts
                    fillerB += setup_half_ops(l + 1, 0)
                    fillerB += q_half_ops(l + 1, 0)
                    fillerB += k_ops(l + 1, 0)
                    fillerB += v_ops(l + 1, range(0, 4))
                run_attn(l, 1, fillerB)
                pending = []
                if l + 1 < L:
                    pending += k_ops(l + 1, 2)
                    pending += v_ops(l + 1, range(8, 12))
                pending += chain_ops(l, 1)
                if l + 1 < L:
                    pending += k_ops(l + 1, 1)
                    pending += v_ops(l + 1, range(4, 8))
                    pending += setup_half_ops(l + 1, 1)
                    pending += k_ops(l + 1, 3)
                    pending += v_ops(l + 1, range(12, 16))
                    pending += q_half_ops(l + 1, 1)
            # final-LN half 1 chain (wrote h2T) + half-0 result sits in h2T too
            for op in pending:
                op()

            # ---------------- pool + classifier ----------------
            # final LN output lives in h2T (both halves)
            emb = sp.tile([P, 2], F32, tag="emb")
            for cc in range(2):
                nc.vector.reduce_sum(emb[:, cc:cc + 1], h2T[cc][:], axis=X_AXIS)
            be_in = dp.tile([P, 2], F32, tag="be_in", name="be_in")
            be_out = dp.tile([P, 2], F32, tag="be_out", name="be_out")
            nc.sync.dma_start(be_in[:], emb[:])
            if sim:
                nc.sync.dma_start(be_out[:], be_in[:])
            else:
                nc.gpsimd.collective_compute(
                    "AllReduce", Alu.add, replica_groups=REPL,
                    ins=[be_in[:].opt()], outs=[be_out[:].opt()])
            embr = sp.tile([P, 2], F32, tag="embr")
            nc.sync.dma_start(embr[:], be_out[:])

            h1ps = psM.tile([P, CLS_H // P], F32, tag="mm", name="h1ps")
            for mt in range(CLS_H // P):
                for kc in range(2):
                    nc.tensor.matmul(h1ps[:, mt:mt + 1],
                                     lhsT=wc1[:, kc, mt * P:(mt + 1) * P],
                                     rhs=embr[:, kc:kc + 1],
                                     start=(kc == 0), stop=(kc == 1))
            h1 = sp.tile([P, CLS_H // P], F32, tag="h1")
            nc.vector.tensor_add(h1[:], h1ps[:], bc1[:])
            nc.vector.tensor_scalar_max(h1[:], h1[:], 0.0)
            lps = psM.tile([1, NOUT], F32, tag="mm", name="lps")
            for j in range(CLS_H // P):
                nc.tensor.matmul(lps[:], lhsT=h1[:, j:j + 1], rhs=wc2[:, j, :],
                                 start=(j == 0), stop=(j == CLS_H // P - 1))
            lsb = sp.tile([1, NOUT], F32, tag="lsb")
            nc.vector.tensor_add(lsb[:], lps[:], bc2[:])
            mx = sp.tile([1, 1], F32, tag="mx")
            nc.vector.tensor_reduce(mx[:], lsb[:], axis=X_AXIS, op=Alu.max)
            nmx = sp.tile([1, 1], F32, tag="nmx")
            nc.vector.tensor_scalar_mul(nmx[:], mx[:], -1.0)
            esb = sp.tile([1, NOUT], F32, tag="esb")
            nc.scalar.activation(esb[:], lsb[:], Act.Exp, bias=nmx[:])
            ssum = sp.tile([1, 1], F32, tag="ssum")
            nc.vector.reduce_sum(ssum[:], esb[:], axis=X_AXIS)
            rsum = sp.tile([1, 1], F32, tag="rsum")
            nc.vector.reciprocal(rsum[:], ssum[:])
            probs = sp.tile([1, NOUT], F32, tag="probs")
            nc.vector.tensor_single_scalar(probs[:], esb[:], rsum[:], Alu.mult)
            nc.sync.dma_start(out_d[:], probs[:])

    nc.compile()
    return nc


def _prep_shared(inputs):
    """Host-side weight prepack (identical for all cores)."""
    f = lambda a: np.ascontiguousarray(np.asarray(a, dtype=np.float32))

    def pack_mat(w):  # [C_in, M] -> [128, C_in//128, M]
        ci, m = w.shape
        return np.ascontiguousarray(w.reshape(ci // P, P, m).transpose(1, 0, 2))

    def perlayer(g):  # [L, P, 2, M] -> [P, L, 2, M] contiguous
        return np.ascontiguousarray(np.stack(g).transpose(1, 0, 2, 3))

    wq3 = perlayer([pack_mat(f(inputs["Wq"][l]).transpose(1, 0, 2).reshape(C, H * HS))
                    for l in range(L)])
    wk3 = perlayer([pack_mat(f(inputs["Wk"][l]).transpose(1, 0, 2).reshape(C, H * HS))
                    for l in range(L)])
    wv3 = perlayer([pack_mat(f(inputs["Wv"][l]).transpose(1, 0, 2).reshape(C, H * HS))
                    for l in range(L)])
    def pack_wp(w):  # [C, C] -> [P, 4, C] acc-layout, junk rows zeroed
        out = np.zeros((P, 4, C), np.float32)
        for pc in range(4):
            hh = (pc // 2) * 4 + 2 * (pc % 2)
            out[0:HS, pc, :] = w[hh * HS:(hh + 1) * HS, :]
            out[64:64 + HS, pc, :] = w[(hh + 1) * HS:(hh + 2) * HS, :]
        return out

    wp3 = perlayer([pack_wp(f(inputs["Wproj"][l])) for l in range(L)])
    w13 = perlayer([pack_mat(f(inputs["W1"][l])) for l in range(L)])
    w23 = perlayer([pack_mat(f(inputs["W2"][l])) for l in range(L)])

    def pack_vec(v):  # [256] -> [128, 2]
        return np.ascontiguousarray(f(v).reshape(2, P).T)

    vecs = np.stack([np.stack([pack_vec(inputs[k][l]) for k in
                               ("bproj", "b1", "b2")]).transpose(1, 0, 2)
                     for l in range(L)])
    vecs = np.ascontiguousarray(vecs.transpose(1, 0, 2, 3))   # [P, L, 3, 2]
    # vrow[row(g/b), l, ln_i, cc, :]: gamma/beta as contraction rows
    vrow = np.zeros((2, L, 2, 2, P), np.float32)
    for l in range(L):
        for ln_i, (gk, bk) in enumerate((("ln1_g", "ln1_b"),
                                         ("ln2_g", "ln2_b"))):
            g = f(inputs[gk][l]).reshape(2, P)
            b = f(inputs[bk][l]).reshape(2, P)
            for cc in range(2):
                vrow[0, l, ln_i, cc] = g[cc]
                vrow[1, l, ln_i, cc] = b[cc]
    lnfrow = np.zeros((2, 2, P), np.float32)
    gf = f(inputs["lnf_g"]).reshape(2, P)
    bf = f(inputs["lnf_b"]).reshape(2, P)
    for cc in range(2):
        lnfrow[0, cc] = gf[cc]
        lnfrow[1, cc] = bf[cc]
    wc1 = pack_mat(f(inputs["Wc1"]) / T)        # fold mean-pool 1/T into Wc1
    bc1 = np.ascontiguousarray(f(inputs["bc1"]).reshape(CLS_H // P, P).T)
    wc2 = np.ascontiguousarray(f(inputs["Wc2"]).reshape(CLS_H // P, P, NOUT)
                               .transpose(1, 0, 2))
    bc2 = f(inputs["bc2"]).reshape(1, NOUT)
    tokf = f(inputs["tok_emb"])
    posf = f(inputs["pos_emb"])
    return dict(wq=wq3, wk=wk3, wv=wv3, wp=wp3, w1=w13, w2=w23, vecs=vecs,
                vrow=vrow, lnfrow=lnfrow, wc1=wc1, bc1=bc1, wc2=wc2, bc2=bc2,
                tok=tokf, pos=posf)


def _wrap_idx(ids):
    """int array [n] -> dma_gather wrapped layout [128, n//16] int16."""
    n = ids.shape[0]
    w = ids.reshape(n // 16, 16).T.astype(np.int16)     # [16, n//16]
    return np.ascontiguousarray(np.tile(w, (8, 1)))     # [128, n//16]


def _make_in_maps(inputs):
    shared = _prep_shared(inputs)
    idx = np.asarray(inputs["idx"]).astype(np.int64)
    in_maps = []
    for c in range(N_CORES):
        b, th = c // 2, c % 2
        t0 = th * TL
        idx_loc = idx[b, t0:t0 + TL]
        pos_loc = shared["pos"][t0:t0 + TL]  # [TL, C]
        posr_a = np.ascontiguousarray(
            pos_loc.reshape(TL // P, P, C).transpose(1, 0, 2))
        m = dict(tok=shared["tok"], idxw=_wrap_idx(idx_loc), posr=posr_a,
                 wq=shared["wq"], wk=shared["wk"], wv=shared["wv"],
                 wp=shared["wp"], w1=shared["w1"], w2=shared["w2"],
                 vecs=shared["vecs"], vrow=shared["vrow"],
                 lnfrow=shared["lnfrow"], wc1=shared["wc1"],
                 bc1=shared["bc1"], wc2=shared["wc2"], bc2=shared["bc2"])
        in_maps.append(m)
    return in_maps


def kernel(**inputs) -> np.ndarray:
    if "nc" not in _CACHE:
        _CACHE["nc"] = _build_program()
    nc = _CACHE["nc"]
    in_maps = _make_in_maps(inputs)
    res = bass_utils.run_bass_kernel_spmd(nc, in_maps, core_ids=list(range(N_CORES)))
    out = np.zeros((B, NOUT), np.float32)
    for b in range(B):
        out[b] = res.results[2 * b]["probs"][0]
    return out
